# revision 55
# baseline (speedup 1.0000x reference)
"""Trainium2 Bass kernel for nn_MESGM_15857019256842.

Data-parallel over batch: 16 batches -> 8 cores x 2 batches.
Per core: gather clause tokens (indirect DMA), 2 GCN layers, max/avg pooling,
projection, 8-head self-attention over 32 clauses/batch, FFN, label decoder,
soft-label KL loss. Each core emits (sum kl*mask, sum mask); host combines.

v2: trace-driven rewrite of the baseline.
 - host packs block-diag transposed adjacency + bias vectors (layout prep)
 - one big 3D DMA per weight matrix, issued early and spread across queues
 - bf16 PE transposes for the gathered tokens
 - pooling done per 512-col block, split across DVE and GpSimd, overlapped
   with GCN matmuls; no H2T materialization (pool consumes z2 chunks)
 - z2 runs c-outer so each pooled chunk immediately feeds the projection
 - attention: full-row exp (no b-split/memset), transposed-ctx path that
   feeds ao directly, FFN produces inter transposed (no 24 transposes)
"""
import sys
sys.path.insert(0, '/opt/trn_rl_repo')
import numpy as np

from concourse import bass, mybir, tile
from concourse import bass_utils
from concourse.masks import make_identity
from concourse.vector_clock import ScopedClock

F32 = mybir.dt.float32
BF16 = mybir.dt.bfloat16
I32 = mybir.dt.int32
AF = mybir.ActivationFunctionType
AX = mybir.AxisListType
ALU = mybir.AluOpType

B, S, H, M, LC, NL, II, NH, DH = 16, 512, 768, 32, 32, 7, 3072, 8, 96
NCORES = 8
BB = B // NCORES          # 2 batches per core
NCL = BB * M              # 64 clauses per core
NROW = NCL * LC           # 2048 clause-token rows per core
RT = NROW // 128          # 16 row tiles
HC = H // 128             # 6 H chunks
IC = II // 128            # 24 intermediate chunks
LN_EPS = 1e-12
SQD = float(np.sqrt(DH))
NBC = 5 * H + NL          # broadcast-packed bias columns

_MAX_WAITS = 1


def _patched_drain_and_barrier(self, tick_clock, wait_clock):
    nc = self.nc
    drain_inst = nc.sync.drain()
    wait_clock.add_sem_waits(
        drain_inst.ins, ScopedClock({None: tick_clock.global_clock})
    )
    si = drain_inst.ins.sync_info
    waits = list(si.on_wait or [])
    if len(waits) > _MAX_WAITS:
        si.on_wait = waits[:_MAX_WAITS]
        rest = waits[_MAX_WAITS:]
        for i in range(0, len(rest), _MAX_WAITS):
            nop = nc.sync.nop(nofuse=True)
            nop.ins.sync_info = mybir.SyncInfo(
                on_wait=rest[i : i + _MAX_WAITS], on_update=[]
            )
    nc.all_engine_barrier()
    assert self.sems is not None
    popped = nc._tile_sem_poison_stack.pop()
    assert popped is self._sem_poison
    nc.clear_and_free_semaphores(list(self.sems.allocated().values()))
    nc.all_engine_barrier()


tile.TileContext._drain_and_barrier = _patched_drain_and_barrier


def legalize_waits(nc, limit=1):
    """TRN2 instructions carry at most one sem wait; hoist extras onto nops."""
    nfix = 0
    for blk in nc.main_func.blocks:
        insts = list(blk.instructions)
        pos = 0
        for inst in insts:
            si = inst.sync_info
            waits = list(si.on_wait) if si is not None and si.on_wait else []
            if len(waits) > limit:
                si.on_wait = waits[-limit:]
                rest = waits[:-limit]
                eng = nc.engines[inst.engine]
                for j in range(0, len(rest), limit):
                    nop = eng.nop(nofuse=True)
                    nop.ins.sync_info = mybir.SyncInfo(
                        on_wait=rest[j : j + limit], on_update=[]
                    )
                    src_blk = nc.cur_bb.bb
                    popped = src_blk.instructions.pop()
                    assert popped.name == nop.ins.name
                    blk.instructions.insert(pos, nop.ins)
                    pos += 1
                nfix += 1
            pos += 1
    return nfix


DEBUG = False


def build_program():
    nc = bass.Bass(trn_type="TRN2")

    # ---- DRAM I/O --------------------------------------------------------
    enc = nc.dram_tensor("enc", [BB * S, H], F32, kind="ExternalInput")
    enc_s = [nc.dram_tensor(f"enc_s{i}", [BB * S, 192], F32, kind="ExternalInput")
             for i in range(4)]
    gidx = nc.dram_tensor("gidx", [128, RT], I32, kind="ExternalInput")
    wrm = nc.dram_tensor("wrm", [128, RT], F32, kind="ExternalInput")
    wrm_b = nc.dram_tensor("wrm_b", [NROW], BF16, kind="ExternalInput")
    lens = nc.dram_tensor("lens", [NCL], F32, kind="ExternalInput")
    wseg = nc.dram_tensor("wseg", [128, RT * 4], BF16, kind="ExternalInput")
    cnm = nc.dram_tensor("cnm", [NCL], F32, kind="ExternalInput")
    amask = nc.dram_tensor("amask", [NCL, NH * NCL], BF16, kind="ExternalInput")
    adjt = nc.dram_tensor("adjt", [128, RT, 128], BF16, kind="ExternalInput")
    tgt = nc.dram_tensor("tgt", [NCL, NL], F32, kind="ExternalInput")
    bias_pk = nc.dram_tensor("bias_pk", [128, 42], F32, kind="ExternalInput")
    qkb_pk = nc.dram_tensor("qkb_pk", [DH, 16], F32, kind="ExternalInput")
    bcast_pk = nc.dram_tensor("bcast_pk", [NBC], BF16, kind="ExternalInput")

    # all weights arrive host-packed in their SBUF layout: [128, chunks*cols]
    w = {}
    for name, shp, dt in [
        ("gc1_w", [128, HC * H], BF16), ("gc2_w", [128, HC * H], BF16),
        ("proj_w", [128, IC * H], BF16),
        ("q_w", [128, HC * H], BF16), ("k_w", [128, HC * H], BF16),
        ("v_w", [128, HC * H], BF16),
        ("ao_w", [DH, NH * H], BF16),
        ("int_w", [128, HC * II], BF16), ("out_w", [128, IC * H], BF16),
        ("dec_w", [128, HC * NL], F32),
    ]:
        w[name] = nc.dram_tensor(name, shp, dt, kind="ExternalInput")

    out_d = nc.dram_tensor("out", [2], F32, kind="ExternalOutput")
    dbg = {}
    if DEBUG:
        dbg["cv"] = nc.dram_tensor("dbg_cv", [NCL, H], F32, kind="ExternalOutput")
        dbg["attn"] = nc.dram_tensor("dbg_attn", [NCL, H], F32, kind="ExternalOutput")
        dbg["pred"] = nc.dram_tensor("dbg_pred", [NCL, NL], F32, kind="ExternalOutput")
        dbg["pool"] = nc.dram_tensor("dbg_pool", [128, 24, NCL], F32, kind="ExternalOutput")

    with tile.TileContext(nc) as tc:
        _body(nc, tc, enc, enc_s, gidx, wrm, wrm_b, lens, wseg, cnm, amask, adjt, tgt,
              bias_pk, qkb_pk, bcast_pk, w, out_d, dbg)

    nfix = legalize_waits(nc)
    return nc, nfix


def _body(nc, tc, enc, enc_s, gidx, wrm, wrm_b, lens, wseg, cnm, amask, adjt, tgt,
          bias_pk, qkb_pk, bcast_pk, w, out_d, dbg):
    from contextlib import ExitStack
    ctx = ExitStack()
    with ctx:
        # pool stack (LIFO). Pools reserve their full footprint at open, so
        # big pools open only for their live window:
        #   pp -> wat -> pjps -> wgcn -> yn16 -> pjwp -> ph1 -> pxm
        #   closes: pxm (GCN1 end), ph1 (y2 end), pjwp/yn16/wgcn (z2 end),
        #   pjps (after cvT), then p2 opens for the attention phase.
        pp = ctx.enter_context(tc.tile_pool(name="persist", bufs=1))

        # identities
        ident = pp.tile([128, 128], F32, tag="ident")
        make_identity(nc, ident[:])
        ident_b = pp.tile([128, 128], BF16, tag="identb")
        nc.vector.tensor_copy(out=ident_b[:], in_=ident[:])
        tprime = pp.tile([1, 1], F32, tag="tprime")
        nc.scalar.copy(out=tprime[:], in_=ident[0:1, 0:1])  # prime ACT table

        # --- input / small-tensor DMAs (spread across queues) -------------
        gidx_t = pp.tile([128, RT], I32, tag="gidx")
        nc.sync.dma_start(out=gidx_t[:], in_=bass.AP(tensor=gidx, offset=0, ap=[[RT, 128], [1, RT]]))
        wrm_pp = pp.tile([128, RT], F32, tag="wrmpp")
        nc.sync.dma_start(out=wrm_pp[:], in_=bass.AP(tensor=wrm, offset=0, ap=[[RT, 128], [1, RT]]))
        wseg_sb = pp.tile([128, RT, 4], BF16, tag="wseg")
        nc.sync.dma_start(out=wseg_sb[:].rearrange("p a b -> p (a b)"), in_=bass.AP(tensor=wseg, offset=0, ap=[[RT * 4, 128], [1, RT * 4]]))

        # phase-2 matmul weights + cvT (live to the end; reserved at open)
        wat = ctx.enter_context(tc.tile_pool(name="wat", bufs=1))

        # gcn weights + adjacency (space frees after z2)
        wgcn_stack = ExitStack()
        wg = wgcn_stack.enter_context(tc.tile_pool(name="wgcn", bufs=1))
        gc1w = wg.tile([128, HC, H], BF16, tag="gc1w")
        nc.sync.dma_start(out=gc1w[:].rearrange("p a b -> p (a b)"), in_=bass.AP(tensor=w["gc1_w"], offset=0, ap=[[HC * H, 128], [1, HC * H]]))
        adjt_sb = wg.tile([128, RT, 128], BF16, tag="adjt")
        nc.sync.dma_start(out=adjt_sb[:].rearrange("p a b -> p (a b)"), in_=bass.AP(tensor=adjt, offset=0, ap=[[RT * 128, 128], [1, RT * 128]]))
        gc2w = wg.tile([128, HC, H], BF16, tag="gc2w")
        nc.sync.dma_start(out=gc2w[:].rearrange("p a b -> p (a b)"), in_=bass.AP(tensor=w["gc2_w"], offset=0, ap=[[HC * H, 128], [1, HC * H]]))
        bcast_all = pp.tile([NCL, NBC], BF16, tag="bcall")
        nc.sync.dma_start(out=bcast_all[:], in_=bass.AP(tensor=bcast_pk, offset=0, ap=[[0, NCL], [1, NBC]]))
        amask8 = pp.tile([NCL, NH, NCL], BF16, tag="amask8")
        nc.sync.dma_start(out=amask8[:].rearrange("p a b -> p (a b)"), in_=bass.AP(tensor=amask, offset=0, ap=[[NH * NCL, NCL], [1, NH * NCL]]))
        wrm_bcb = pp.tile([128, NROW], BF16, tag="wrmbcb")
        nc.sync.dma_start(out=wrm_bcb[:], in_=bass.AP(tensor=wrm_b, offset=0, ap=[[0, 128], [1, NROW]]))
        # small tensors on the sync queue, priority order; scalar queue is
        # kept free for the gather masks
        bias_sb = pp.tile([128, 42], F32, tag="biaspk")
        nc.sync.dma_start(out=bias_sb[:], in_=bass.AP(tensor=bias_pk, offset=0, ap=[[42, 128], [1, 42]]))
        qkb_sb = pp.tile([DH, 16], F32, tag="qkb")
        nc.sync.dma_start(out=qkb_sb[:], in_=bass.AP(tensor=qkb_pk, offset=0, ap=[[16, DH], [1, 16]]))
        cnm_pp = pp.tile([NCL, 1], F32, tag="cnmpp")
        nc.sync.dma_start(out=cnm_pp[:], in_=cnm[:, None])
        lens_bc = pp.tile([128, NCL], F32, tag="lensbc")
        nc.sync.dma_start(out=lens_bc[:], in_=bass.AP(tensor=lens, offset=0, ap=[[0, 128], [1, NCL]]))
        tgt_sb = pp.tile([NCL, NL], F32, tag="tgtsb")
        nc.sync.dma_start(out=tgt_sb[:], in_=tgt[:, :])
        eps_t = pp.tile([NCL, 1], F32, tag="epst")
        nc.vector.memset(eps_t[:], LN_EPS)
        ones_t = pp.tile([NCL, 1], F32, tag="onest")
        nc.vector.memset(ones_t[:], 1.0)

        # bias_pk columns: gc1_b 0:6, gc2_b 6:12, proj_b 12:18, int_b 18:42
        gb1 = bias_sb[:, 0:HC]
        gb2 = bias_sb[:, HC : 2 * HC]
        projb = bias_sb[:, 2 * HC : 3 * HC]

        PT = pp.tile([128, 24, NCL], BF16, tag="PT")

        # attention weights early: wat space is reserved from open, so these
        # carry no space-reuse waits and transfer during the gather
        qw = wat.tile([128, HC, H], BF16, tag="qw")
        nc.sync.dma_start(out=qw[:].rearrange("p a b -> p (a b)"), in_=bass.AP(tensor=w["q_w"], offset=0, ap=[[HC * H, 128], [1, HC * H]]))
        kw = wat.tile([128, HC, H], BF16, tag="kw")
        nc.sync.dma_start(out=kw[:].rearrange("p a b -> p (a b)"), in_=bass.AP(tensor=w["k_w"], offset=0, ap=[[HC * H, 128], [1, HC * H]]))
        dw = wat.tile([128, HC, NL], F32, tag="dw")
        nc.sync.dma_start(out=dw[:].rearrange("p a b -> p (a b)"), in_=bass.AP(tensor=w["dec_w"], offset=0, ap=[[HC * NL, 128], [1, HC * NL]]))

        # big transposed activation tiles + rotating proj_w quarters
        pjwp_stack = ExitStack()
        pjwp = pjwp_stack.enter_context(tc.tile_pool(name="pjwp", bufs=3))
        ph1_stack = ExitStack()
        ph1 = ph1_stack.enter_context(tc.tile_pool(name="ph1", bufs=1))
        H1T = ph1.tile([128, HC, NROW], BF16, tag="H1T")
        pxm_stack = ExitStack()
        pxm = pxm_stack.enter_context(tc.tile_pool(name="pxm", bufs=1))
        XmT = pxm.tile([128, HC, NROW], BF16, tag="XmT")

        # =================== phase 1: gather + transpose + x-pool =========
        def y_block(XT, wt, g, tag, ypool, psum_pool, evac="mixed"):
            yns = []
            for rr in range(4):
                r = 4 * g + rr
                p1 = psum_pool.tile([128, 512], F32, tag="y1", name=f"y1_{tag}{r}")
                p2 = psum_pool.tile([128, 256], F32, tag="y2", name=f"y2_{tag}{r}")
                for c in range(HC):
                    lhs = XT[:, c, r * 128 : r * 128 + 128]
                    nc.tensor.matmul(out=p1[:], lhsT=lhs, rhs=wt[:, c, 0:512],
                                     start=(c == 0), stop=(c == HC - 1))
                    nc.tensor.matmul(out=p2[:], lhsT=lhs, rhs=wt[:, c, 512:768],
                                     start=(c == 0), stop=(c == HC - 1))
                yr = ypool.tile([128, H], BF16, tag=f"yn{rr}", name=f"yn_{tag}{r}")
                if evac == "scalar":
                    nc.scalar.copy(out=yr[:, 0:512], in_=p1[:])
                else:
                    nc.vector.tensor_copy(out=yr[:, 0:512], in_=p1[:])
                nc.scalar.copy(out=yr[:, 512:768], in_=p2[:])
                yns.append(yr)
            return yns

        with tc.tile_pool(name="xg", bufs=2) as xgp, \
             tc.tile_pool(name="xb", bufs=2) as xbp, \
             tc.tile_pool(name="xps", bufs=1, space="PSUM") as xps, \
             tc.tile_pool(name="tps", bufs=2, space="PSUM") as tps, \
             tc.tile_pool(name="ynat", bufs=2) as gcn_yn, \
             tc.tile_pool(name="gps", bufs=2, space="PSUM") as gps, \
             tc.tile_pool(name="zps", bufs=1, space="PSUM") as zps:

            def z1_block(g, yns):
                for c in range(HC):
                    zp = zps.tile([128, 512], F32, tag="z", name=f"z_l1{g}_{c}")
                    for rr in range(4):
                        nc.tensor.matmul(
                            out=zp[:, rr * 128 : rr * 128 + 128],
                            lhsT=yns[rr][:, c * 128 : c * 128 + 128],
                            rhs=adjt_sb[:, 4 * g + rr, :],
                            start=True, stop=True,
                        )
                    nc.scalar.activation(
                        out=H1T[:, c, g * 512 : g * 512 + 512], in_=zp[:],
                        func=AF.Relu, bias=gb1[:, c : c + 1], scale=1.0,
                    )

            prev = None
            for g in range(4):
                xt = xgp.tile([128, 4, H], F32, tag="xg", name=f"xg{g}")
                xb = xbp.tile([128, 4, H], BF16, tag="xb", name=f"xb{g}")
                for rr in range(4):
                    r = 4 * g + rr
                    if r == 0:
                        for piece in range(4):
                            a, b = piece * 192, piece * 192 + 192
                            nc.gpsimd.indirect_dma_start(
                                out=xt[:, rr, a:b], out_offset=None, in_=enc_s[piece][:],
                                in_offset=bass.IndirectOffsetOnAxis(ap=gidx_t[:, r : r + 1], axis=0),
                            )
                            nc.scalar.mul(out=xb[:, rr, a:b], in_=xt[:, rr, a:b],
                                          mul=wrm_pp[:, r : r + 1])
                    else:
                        nc.gpsimd.indirect_dma_start(
                            out=xt[:, rr, :], out_offset=None, in_=enc[:],
                            in_offset=bass.IndirectOffsetOnAxis(ap=gidx_t[:, r : r + 1], axis=0),
                        )
                        nc.scalar.mul(out=xb[:, rr, :], in_=xt[:, rr, :], mul=wrm_pp[:, r : r + 1])
                for c in range(HC):
                    ps = tps.tile([128, 512], BF16, tag="tp", name=f"tp{g}_{c}")
                    for rr in range(4):
                        nc.tensor.transpose(
                            out=ps[:, rr * 128 : rr * 128 + 128],
                            in_=xb[:, rr, c * 128 : c * 128 + 128],
                            identity=ident_b[:],
                        )
                    psx = xps.tile([128, 16], F32, tag="psx", name=f"psx{g}_{c}")
                    for rr in range(4):
                        nc.tensor.matmul(out=psx[:, rr * 4 : rr * 4 + 4],
                                         lhsT=xb[:, rr, c * 128 : c * 128 + 128],
                                         rhs=wseg_sb[:, 4 * g + rr, :],
                                         start=True, stop=True)
                    if (c + g) % 2 == 0:
                        nc.vector.tensor_copy(out=XmT[:, c, g * 512 : g * 512 + 512], in_=ps[:])
                        nc.scalar.copy(out=PT[:, 12 + c, g * 16 : g * 16 + 16], in_=psx[:])
                    else:
                        nc.scalar.copy(out=XmT[:, c, g * 512 : g * 512 + 512], in_=ps[:])
                        nc.vector.tensor_copy(out=PT[:, 12 + c, g * 16 : g * 16 + 16], in_=psx[:])
                    v = XmT[:, c, g * 512 : g * 512 + 512].rearrange("p (n l) -> p n l", l=LC)
                    nc.vector.reduce_max(out=PT[:, c, g * 16 : g * 16 + 16], in_=v, axis=AX.X)
                yns = y_block(XmT, gc1w, g, "l1", gcn_yn, gps)
                if prev is not None:
                    z1_block(prev[0], prev[1])
                prev = (g, yns)
            z1_block(prev[0], prev[1])

        # proj_w quarters 0-2 (sync queue; gpsimd slots stay free for gathers)
        pjq = []
        for q in range(3):
            t = pjwp.tile([128, HC, H], BF16, tag="pjw", name=f"pjw{q}")
            nc.sync.dma_start(out=t[:].rearrange("p a b -> p (a b)"), in_=bass.AP(
                tensor=w["proj_w"], offset=q * HC * H,
                ap=[[IC * H, 128], [1, HC * H]]))
            pjq.append(t)

        pxm_stack.close()

        # projection psum (phase-1 psum pools closed now; spans GCN2..cvT)
        pjps_stack = ExitStack()
        pjps = pjps_stack.enter_context(tc.tile_pool(name="pjps", bufs=1, space="PSUM"))
        pcs = pjps.tile([128, HC, NCL], F32, tag="pj")

        # v / ao weights
        vw = wat.tile([128, HC, H], BF16, tag="vw")
        nc.scalar.dma_start(out=vw[:].rearrange("p a b -> p (a b)"), in_=bass.AP(tensor=w["v_w"], offset=0, ap=[[HC * H, 128], [1, HC * H]]))
        aow = wat.tile([DH, NH, H], BF16, tag="aow")
        nc.scalar.dma_start(out=aow[:].rearrange("p a b -> p (a b)"), in_=bass.AP(tensor=w["ao_w"], offset=0, ap=[[NH * H, DH], [1, NH * H]]))

        korder = list(range(12, 18)) + list(range(0, 6)) + \
            [k for c in range(HC) for k in (6 + c, 18 + c)]

        def proj_chunk(k, ki):
            kq, kr = divmod(k, HC)
            for m in range(HC):
                nc.tensor.matmul(out=pcs[:, m, :], lhsT=pjq[kq][:, kr, m * 128 : m * 128 + 128],
                                 rhs=PT[:, k, :], start=(ki == 0), stop=(ki == 23))
        for ki in range(12):
            proj_chunk(korder[ki], ki)
        t = pjwp.tile([128, HC, H], BF16, tag="pjw", name="pjw3")
        nc.sync.dma_start(out=t[:].rearrange("p a b -> p (a b)"), in_=bass.AP(
            tensor=w["proj_w"], offset=3 * HC * H, ap=[[IC * H, 128], [1, HC * H]]))
        pjq.append(t)

        # layer 2: per group g, y2(g) then z2 blocks (c, g) + pooling;
        # proj h2 chunks fire inside the last group as chunks complete.
        with tc.tile_pool(name="gps2", bufs=2, space="PSUM") as gps2, \
             tc.tile_pool(name="yn16", bufs=2) as yn16, \
             tc.tile_pool(name="h2b", bufs=3) as h2bp, \
             tc.tile_pool(name="h2m", bufs=3) as h2mp, \
             tc.tile_pool(name="xs2", bufs=3) as xs2p, \
             tc.tile_pool(name="z2ps", bufs=3, space="PSUM") as z2ps:
            for g in range(4):
                yg = y_block(H1T, gc2w, g, "l2", yn16, gps2, evac="scalar")
                for c in range(HC):
                    zp = z2ps.tile([128, 512], F32, tag="z2", name=f"z_l2{g}_{c}")
                    for rr in range(4):
                        nc.tensor.matmul(
                            out=zp[:, rr * 128 : rr * 128 + 128],
                            lhsT=yg[rr][:, c * 128 : c * 128 + 128],
                            rhs=adjt_sb[:, 4 * g + rr, :],
                            start=True, stop=True,
                        )
                    hb = h2bp.tile([128, 512], BF16, tag="h2b", name=f"h2b{g}_{c}")
                    nc.scalar.activation(out=hb[:], in_=zp[:], func=AF.Relu,
                                         bias=gb2[:, c : c + 1], scale=1.0)
                    hm = h2mp.tile([128, 512], BF16, tag="h2m", name=f"h2m{g}_{c}")
                    nc.gpsimd.tensor_tensor(out=hm[:], in0=hb[:],
                                            in1=wrm_bcb[:, g * 512 : g * 512 + 512], op=ALU.mult)
                    v = hm[:].rearrange("p (n l) -> p n l", l=LC)
                    nc.vector.reduce_max(out=PT[:, 6 + c, g * 16 : g * 16 + 16], in_=v, axis=AX.X)
                    xs = xs2p.tile([128, 16], F32, tag="xs2", name=f"xs2{g}_{c}")
                    nc.vector.reduce_sum(out=xs[:], in_=v, axis=AX.X)
                    nc.gpsimd.tensor_tensor(
                        out=PT[:, 18 + c, g * 16 : g * 16 + 16], in0=xs[:],
                        in1=lens_bc[:, g * 16 : g * 16 + 16], op=ALU.mult)
                    if g == 3:
                        proj_chunk(6 + c, 12 + 2 * c)
                        proj_chunk(18 + c, 13 + 2 * c)

        ph1_stack.close()

        # cv^T = relu(proj + b) straight out of the proj psum
        cvT = wat.tile([128, HC, NCL], BF16, tag="cvT")
        for m in range(HC):
            nc.scalar.activation(out=cvT[:, m, :], in_=pcs[:, m, :], func=AF.Relu,
                                 bias=projb[:, m : m + 1], scale=1.0)
        pjps_stack.close()
        pjwp_stack.close()
        wgcn_stack.close()
        if DEBUG:
            with tc.tile_pool(name="dbgp", bufs=1) as dp:
                ptf = dp.tile([128, 24, NCL], F32, tag="ptdbg")
                nc.vector.tensor_copy(out=ptf[:], in_=PT[:])
                nc.sync.dma_start(out=dbg["pool"][:, :, :], in_=ptf[:])

        # =================== phase 2: attention + FFN + KL ================
        p2 = ctx.enter_context(tc.tile_pool(name="p2", bufs=1))
        # bcast_pk: ao_b, v_b, out_b, ln1_g, ln1_b, ln2_g, ln2_b, bdec
        aob_bc = bcast_all[:, 0:H]
        vb_bc = bcast_all[:, H : 2 * H]
        outb_bc = bcast_all[:, 2 * H : 3 * H]
        l1g_bc = bcast_all[:, 3 * H : 4 * H]
        l1b_bc = bcast_all[:, 4 * H : 5 * H]
        decb_bc = bcast_all[:, 5 * H : 5 * H + NL]

        # FFN weights stream into space freed by H1T/yn16/pjw
        wf_stack = ExitStack()
        wf = wf_stack.enter_context(tc.tile_pool(name="wf", bufs=1))
        iw = wf.tile([128, HC, II], BF16, tag="iw")
        for half in range(2):
            nc.sync.dma_start(
                out=iw[:, 3 * half : 3 * half + 3, :].rearrange("p a b -> p (a b)"),
                in_=bass.AP(tensor=w["int_w"], offset=half * 3 * II,
                            ap=[[HC * II, 128], [1, 3 * II]]))
        ow = wf.tile([128, IC, H], BF16, tag="ow")
        for qtr in range(4):
            nc.sync.dma_start(
                out=ow[:, 6 * qtr : 6 * qtr + 6, :].rearrange("p a b -> p (a b)"),
                in_=bass.AP(tensor=w["out_w"], offset=qtr * 6 * H,
                            ap=[[IC * H, 128], [1, 6 * H]]))

        QT = p2.tile([DH, NH, NCL], BF16, tag="QT")
        KT = p2.tile([DH, NH, NCL], BF16, tag="KT")
        Vn = p2.tile([NCL, H], BF16, tag="Vn")
        att8 = p2.tile([NCL, NH, NCL], BF16, tag="att8")
        attS = p2.tile([NCL, NH, NCL], BF16, tag="attS")
        sums_t = p2.tile([NCL, NH], F32, tag="sums")
        recip_t = p2.tile([NCL, NH], F32, tag="recip")
        negmax = p2.tile([NCL, NH], F32, tag="negmax")
        cv_pa = p2.tile([NCL, H], F32, tag="cvpa")

        # scores psum prefilled with the attention mask (matmuls accumulate
        # on top); the copy is issued early so it never gates the scores.
        sc_stack = ExitStack()
        scps = sc_stack.enter_context(tc.tile_pool(name="scps", bufs=1, space="PSUM"))
        pss = scps.tile([NCL, NH, NCL], F32, tag="scores")
        nc.vector.tensor_copy(out=pss[:], in_=amask8[:])

        with tc.tile_pool(name="qkps", bufs=2, space="PSUM") as qkps, \
             tc.tile_pool(name="vps", bufs=1, space="PSUM") as vps, \
             tc.tile_pool(name="cvt2", bufs=3, space="PSUM") as cvt2:
            psq = qkps.tile([DH, NH * NCL], F32, tag="qk", name="psq")
            for h in range(NH):
                for c in range(HC):
                    nc.tensor.matmul(out=psq[:, h * NCL : h * NCL + NCL],
                                     lhsT=qw[:, c, h * DH : h * DH + DH],
                                     rhs=cvT[:, c, :], start=(c == 0), stop=(c == HC - 1))
            # Q evacs on DVE: (psq + q_b) / sqrt(dh)
            for h in range(NH):
                nc.vector.tensor_scalar(out=QT[:, h, :], in0=psq[:, h * NCL : h * NCL + NCL],
                                        scalar1=qkb_sb[:, h : h + 1], scalar2=1.0 / SQD,
                                        op0=ALU.add, op1=ALU.mult)
            psk = qkps.tile([DH, NH * NCL], F32, tag="qk", name="psk")
            for h in range(NH):
                for c in range(HC):
                    nc.tensor.matmul(out=psk[:, h * NCL : h * NCL + NCL],
                                     lhsT=kw[:, c, h * DH : h * DH + DH],
                                     rhs=cvT[:, c, :], start=(c == 0), stop=(c == HC - 1))
            for h in range(NH):
                nc.scalar.activation(out=KT[:, h, :], in_=psk[:, h * NCL : h * NCL + NCL],
                                     func=AF.Identity, bias=qkb_sb[:, 8 + h : 9 + h], scale=1.0)

            # scores straight after K; V runs on the PE behind them
            for h in range(NH):
                nc.tensor.matmul(out=pss[:, h, :], lhsT=QT[:, h, :], rhs=KT[:, h, :],
                                 start=False, stop=True, skip_group_check=True)
            pv1 = vps.tile([NCL, 512], F32, tag="v1")
            pv2 = vps.tile([NCL, 256], F32, tag="v2")
            for c in range(HC):
                nc.tensor.matmul(out=pv1[:], lhsT=cvT[:, c, :], rhs=vw[:, c, 0:512],
                                 start=(c == 0), stop=(c == HC - 1))
                nc.tensor.matmul(out=pv2[:], lhsT=cvT[:, c, :], rhs=vw[:, c, 512:768],
                                 start=(c == 0), stop=(c == HC - 1))
            # cv natural + ao_b residual base (PE transposes behind V)
            for c in range(HC):
                ps = cvt2.tile([NCL, 128], BF16, tag="cvn", name=f"cvn{c}")
                nc.tensor.transpose(out=ps[:], in_=cvT[:, c, :], identity=ident_b[:])
                nc.vector.tensor_tensor(out=cv_pa[:, c * 128 : c * 128 + 128], in0=ps[:],
                                        in1=aob_bc[:, c * 128 : c * 128 + 128], op=ALU.add)
            nc.vector.tensor_tensor(out=Vn[:, 0:512], in0=pv1[:], in1=vb_bc[:, 0:512], op=ALU.add)
            nc.vector.tensor_tensor(out=Vn[:, 512:768], in0=pv2[:], in1=vb_bc[:, 512:768], op=ALU.add)

        nc.vector.tensor_reduce(out=negmax[:], in_=pss[:], axis=AX.X,
                                op=ALU.max, negate=True)

        # fused per-head softmax + context pipeline: head h's transpose and
        # context matmul run while head h+1 is still exponentiating
        ctxT = p2.tile([DH, NH, NCL], BF16, tag="ctxT")
        attn_out = p2.tile([NCL, H], F32, tag="attnout")
        xhat = p2.tile([NCL, H], F32, tag="xhat")
        ln_in = p2.tile([NCL, H], F32, tag="lnin1")
        with tc.tile_pool(name="aops", bufs=1, space="PSUM") as aops:
            pa1 = aops.tile([NCL, 512], F32, tag="ao1")
            pa2 = aops.tile([NCL, 256], F32, tag="ao2")
            with tc.tile_pool(name="ctps", bufs=2, space="PSUM") as ctps, \
                 tc.tile_pool(name="atts", bufs=3) as atts:
                for h in range(NH):
                    nc.scalar.activation(
                        out=att8[:, h, :], in_=pss[:, h, :], func=AF.Exp,
                        bias=negmax[:, h : h + 1], scale=1.0,
                    )
                    nc.vector.reduce_sum(out=sums_t[:, h : h + 1], in_=att8[:, h, :], axis=AX.X)
                    nc.vector.reciprocal(out=recip_t[:, h : h + 1], in_=sums_t[:, h : h + 1])
                    nc.scalar.mul(out=attS[:, h, :], in_=att8[:, h, :], mul=recip_t[:, h : h + 1])
                    pst = ctps.tile([NCL, NCL], BF16, tag="attT", name=f"attT{h}")
                    nc.tensor.transpose(out=pst[:], in_=attS[:, h, :], identity=ident_b[:64, :64])
                    asb = atts.tile([NCL, NCL], BF16, tag="attTs", name=f"attTs{h}")
                    nc.vector.tensor_copy(out=asb[:], in_=pst[:])
                    pctx = ctps.tile([DH, NCL], F32, tag="ctx", name=f"ctx{h}")
                    nc.tensor.matmul(out=pctx[:], lhsT=Vn[:, h * DH : h * DH + DH], rhs=asb[:],
                                     start=True, stop=True)
                    nc.vector.tensor_copy(out=ctxT[:, h, :], in_=pctx[:])
                    nc.tensor.matmul(out=pa1[:], lhsT=ctxT[:, h, :], rhs=aow[:, h, 0:512],
                                     start=(h == 0), stop=(h == NH - 1))
                    nc.tensor.matmul(out=pa2[:], lhsT=ctxT[:, h, :], rhs=aow[:, h, 512:768],
                                     start=(h == 0), stop=(h == NH - 1))

            def layer_norm(x_nat, g_bc, b_bc, y_nat, lnp):
                stats = lnp.tile([NCL, 3, 6], F32, tag="lnstats")
                for i in range(3):
                    nc.vector.bn_stats(out=stats[:, i, :], in_=x_nat[:, i * 256 : i * 256 + 256])
                mv = lnp.tile([NCL, 2], F32, tag="lnmv")
                nc.vector.bn_aggr(out=mv[:], in_=stats[:])
                sd = lnp.tile([NCL, 1], F32, tag="lnsd")
                nc.scalar.activation(out=sd[:], in_=mv[:, 1:2], func=AF.Sqrt, bias=eps_t[:, :1], scale=1.0)
                rstd = lnp.tile([NCL, 1], F32, tag="lnrstd")
                nc.vector.reciprocal(out=rstd[:], in_=sd[:])
                xc = lnp.tile([NCL, H], F32, tag="lnxc")
                nc.vector.tensor_scalar(out=xc[:], in0=x_nat[:], scalar1=mv[:, 0:1],
                                        scalar2=rstd[:, :1], op0=ALU.subtract, op1=ALU.mult)
                nc.vector.tensor_tensor(out=xc[:], in0=xc[:], in1=g_bc, op=ALU.mult)
                nc.vector.tensor_tensor(out=y_nat[:], in0=xc[:], in1=b_bc, op=ALU.add)

            with tc.tile_pool(name="ln1p", bufs=1) as lnp:
                nc.vector.tensor_tensor(out=ln_in[:, 0:512], in0=pa1[:], in1=cv_pa[:, 0:512], op=ALU.add)
                nc.vector.tensor_tensor(out=ln_in[:, 512:768], in0=pa2[:], in1=cv_pa[:, 512:768], op=ALU.add)
                # LN1 split: FFN consumes xhat directly (ln1_g/ln1_b are
                # host-folded into int_w/int_b); the residual (xhat*g+b)
                # is computed off the critical path.
                stats = lnp.tile([NCL, 3, 6], F32, tag="lnstats")
                for i in range(3):
                    nc.vector.bn_stats(out=stats[:, i, :], in_=ln_in[:, i * 256 : i * 256 + 256])
                mv = lnp.tile([NCL, 2], F32, tag="lnmv")
                nc.vector.bn_aggr(out=mv[:], in_=stats[:])
                sd = lnp.tile([NCL, 1], F32, tag="lnsd")
                nc.scalar.activation(out=sd[:], in_=mv[:, 1:2], func=AF.Sqrt, bias=eps_t[:, :1], scale=1.0)
                rstd = lnp.tile([NCL, 1], F32, tag="lnrstd")
                nc.vector.reciprocal(out=rstd[:], in_=sd[:])
                nc.vector.tensor_scalar(out=xhat[:], in0=ln_in[:], scalar1=mv[:, 0:1],
                                        scalar2=rstd[:, :1], op0=ALU.subtract, op1=ALU.mult)
        sc_stack.close()
        if DEBUG:
            nc.sync.dma_start(out=dbg["attn"][:, :], in_=attn_out[:])

        # aoT = xhat^T (bf16); the residual attn_out = xhat*g1 + b1 runs on
        # DVE in parallel with the FFN matmuls
        aoT = p2.tile([128, HC, NCL], BF16, tag="aoT")
        with tc.tile_pool(name="aotps", bufs=1, space="PSUM") as aotps:
            psA = aotps.tile([128, HC * NCL], F32, tag="psA")
            for c in range(HC):
                nc.tensor.transpose(out=psA[:, c * NCL : c * NCL + NCL],
                                    in_=xhat[:, c * 128 : c * 128 + 128],
                                    identity=ident[:64, :64])
            nc.vector.tensor_copy(out=aoT[:].rearrange("p a b -> p (a b)"), in_=psA[:])
        nc.vector.tensor_tensor(out=attn_out[:], in0=xhat[:], in1=l1g_bc, op=ALU.mult)
        nc.vector.tensor_tensor(out=attn_out[:], in0=attn_out[:], in1=l1b_bc, op=ALU.add)

        # FFN: inter produced transposed, chunk by chunk, feeding out-proj
        ln_in2 = p2.tile([NCL, H], F32, tag="lnin2")
        with tc.tile_pool(name="fips", bufs=4, space="PSUM") as fips, \
             tc.tile_pool(name="fit", bufs=4) as fit, \
             tc.tile_pool(name="fops", bufs=1, space="PSUM") as fops, \
             tc.tile_pool(name="ln2p", bufs=1) as lnp2:
            po1 = fops.tile([NCL, 512], F32, tag="o1")
            po2 = fops.tile([NCL, 256], F32, tag="o2")
            for cc in range(IC):
                ip = fips.tile([128, NCL], F32, tag="fi", name=f"fi{cc}")
                for c in range(HC):
                    nc.tensor.matmul(out=ip[:], lhsT=iw[:, c, cc * 128 : cc * 128 + 128],
                                     rhs=aoT[:, c, :], start=(c == 0), stop=(c == HC - 1))
                it = fit.tile([128, NCL], BF16, tag="it", name=f"it{cc}")
                nc.scalar.activation(out=it[:], in_=ip[:], func=AF.Gelu,
                                     bias=bias_sb[:, 18 + cc : 19 + cc], scale=1.0)
                nc.tensor.matmul(out=po1[:], lhsT=it[:], rhs=ow[:, cc, 0:512],
                                 start=(cc == 0), stop=(cc == IC - 1))
                nc.tensor.matmul(out=po2[:], lhsT=it[:], rhs=ow[:, cc, 512:768],
                                 start=(cc == 0), stop=(cc == IC - 1))
            nc.vector.tensor_tensor(out=ln_in2[:, 0:512], in0=po1[:], in1=attn_out[:, 0:512], op=ALU.add)
            nc.vector.tensor_tensor(out=ln_in2[:, 512:768], in0=po2[:], in1=attn_out[:, 512:768], op=ALU.add)
            nc.vector.tensor_tensor(out=ln_in2[:], in0=ln_in2[:], in1=outb_bc[:], op=ALU.add)
            # LN2 folded into the decoder: pred = rstd*(ln_in2 @ gdec2) + bdec
            # (gdec2 = ln2_g*dec_w - colsum/H host-folded). Only the rstd
            # stats chain remains; the transpose runs in parallel with it.
            stats2 = lnp2.tile([NCL, 3, 6], F32, tag="lnstats2")
            for i in range(3):
                nc.vector.bn_stats(out=stats2[:, i, :], in_=ln_in2[:, i * 256 : i * 256 + 256])
            mv2 = lnp2.tile([NCL, 2], F32, tag="lnmv2")
            nc.vector.bn_aggr(out=mv2[:], in_=stats2[:])
            sd2 = lnp2.tile([NCL, 1], F32, tag="lnsd2")
            nc.scalar.activation(out=sd2[:], in_=mv2[:, 1:2], func=AF.Sqrt, bias=eps_t[:, :1], scale=1.0)
            rstd2 = p2.tile([NCL, 1], F32, tag="lnrstd2")
            nc.vector.reciprocal(out=rstd2[:], in_=sd2[:])
        wf_stack.close()

        # decoder + KL
        outT = p2.tile([128, HC, NCL], F32, tag="outT")
        with tc.tile_pool(name="otps", bufs=1, space="PSUM") as otps:
            psO = otps.tile([128, HC * NCL], F32, tag="psO")
            for c in range(HC):
                nc.tensor.transpose(out=psO[:, c * NCL : c * NCL + NCL],
                                    in_=ln_in2[:, c * 128 : c * 128 + 128],
                                    identity=ident[:64, :64])
            nc.vector.tensor_copy(out=outT[:].rearrange("p a b -> p (a b)"), in_=psO[:])

        pair = p2.tile([NCL, 2], F32, tag="pair")
        fin_sb = p2.tile([2, 1], F32, tag="fin")
        with tc.tile_pool(name="klps", bufs=1, space="PSUM") as klps, \
             tc.tile_pool(name="klsc", bufs=1) as klsc:
            pd = klps.tile([NCL, NL], F32, tag="pred")
            for c in range(HC):
                nc.tensor.matmul(out=pd[:], lhsT=outT[:, c, :], rhs=dw[:, c, :],
                                 start=(c == 0), stop=(c == HC - 1))
            predm = klsc.tile([NCL, NL], F32, tag="predm")
            nc.scalar.mul(out=predm[:], in_=pd[:], mul=rstd2[:, :1])
            pred = klsc.tile([NCL, NL], F32, tag="pred_sb")
            nc.vector.tensor_tensor(out=pred[:], in0=predm[:], in1=decb_bc[:], op=ALU.add)
            if DEBUG:
                nc.sync.dma_start(out=dbg["pred"][:, :], in_=pred[:])
            # KL with sum_l(t)=1: kl = sum_l t*(ln t - pred) - negm + ln(ssum)
            negm = klsc.tile([NCL, 1], F32, tag="negm")
            nc.vector.tensor_reduce(out=negm[:], in_=pred[:], axis=AX.X, op=ALU.max, negate=True)
            esc = klsc.tile([NCL, NL], F32, tag="esc")
            ssum = klsc.tile([NCL, 1], F32, tag="ssum")
            nc.scalar.activation(out=esc[:], in_=pred[:], func=AF.Exp,
                                 bias=negm[:, :1], scale=1.0, accum_out=ssum[:, :1])
            lnS = klsc.tile([NCL, 1], F32, tag="lnS")
            nc.scalar.activation(out=lnS[:], in_=ssum[:], func=AF.Ln)
            lnt = klsc.tile([NCL, NL], F32, tag="lnt")
            nc.scalar.activation(out=lnt[:], in_=tgt_sb[:], func=AF.Ln)
            a1 = klsc.tile([NCL, NL], F32, tag="a1")
            nc.vector.tensor_tensor(out=a1[:], in0=lnt[:], in1=pred[:], op=ALU.subtract)
            nc.vector.tensor_tensor(out=a1[:], in0=a1[:], in1=tgt_sb[:], op=ALU.mult)
            kl = klsc.tile([NCL, 1], F32, tag="kl")
            nc.vector.reduce_sum(out=kl[:], in_=a1[:], axis=AX.X)
            nc.vector.tensor_tensor(out=kl[:], in0=kl[:], in1=negm[:], op=ALU.subtract)
            nc.vector.tensor_tensor(out=kl[:], in0=kl[:], in1=lnS[:], op=ALU.add)
            nc.vector.tensor_tensor(out=pair[:, 0:1], in0=kl[:], in1=cnm_pp[:], op=ALU.mult)
            nc.vector.tensor_copy(out=pair[:, 1:2], in_=cnm_pp[:])
            pf = klps.tile([2, 1], F32, tag="fin_ps")
            nc.tensor.matmul(out=pf[:], lhsT=pair[:], rhs=ones_t[:], start=True, stop=True)
            nc.vector.tensor_copy(out=fin_sb[:], in_=pf[:])
            nc.sync.dma_start(out=out_d[:, None], in_=fin_sb[:])


_CACHE = {}


def _get_program():
    if "nc" not in _CACHE:
        nc, nfix = build_program()
        _CACHE["nc"] = nc
    return _CACHE["nc"]


def shard_inputs(inputs):
    import ml_dtypes
    bf16 = ml_dtypes.bfloat16
    enc = np.ascontiguousarray(inputs["encoder_hs"], dtype=np.float32)
    wr = np.asarray(inputs["word_recovery"], dtype=np.int32)
    wm = np.asarray(inputs["word_recovery_mask"], dtype=np.int32)
    cn = np.asarray(inputs["clause_num_mask"], dtype=np.int32)
    adj = np.ascontiguousarray(inputs["adj_matrix"], dtype=np.float32)
    tl = np.ascontiguousarray(inputs["target_labels"], dtype=np.float32)

    f32w = lambda k: np.asarray(inputs[k], dtype=np.float32)
    bf16w = lambda k: np.ascontiguousarray(f32w(k).astype(bf16))

    # shared (identical across cores) weight arrays. All matmul weights are
    # host-packed into their SBUF layout [128, chunks*cols] so each DMA is
    # 128 contiguous descriptors. LayerNorm affine params are folded into
    # the consumers: ln1_g/ln1_b into int_w/int_b, ln2_g/ln2_b + dec_b into
    # the decoder (gdec2 / bdec).
    def pack128(a):
        # [C*128, X] -> [128, C*X]
        C = a.shape[0] // 128
        return np.ascontiguousarray(a.reshape(C, 128, a.shape[1]).transpose(1, 0, 2).reshape(128, -1))

    shared = {}
    for k in ("gc1_w", "gc2_w", "proj_w", "q_w", "k_w", "v_w", "out_w"):
        shared[k] = pack128(bf16w(k))
    ao96 = f32w("ao_w").reshape(NH, DH, H).transpose(1, 0, 2).reshape(DH, NH * H)
    shared["ao_w"] = np.ascontiguousarray(ao96.astype(bf16))
    int_w2 = f32w("ln1_g")[:, None] * f32w("int_w")
    shared["int_w"] = pack128(np.ascontiguousarray(int_w2.astype(bf16)))
    intb2 = f32w("ln1_b") @ f32w("int_w") + f32w("int_b")
    gdec = f32w("ln2_g")[:, None] * f32w("dec_w")
    gdec2 = gdec - gdec.sum(0, keepdims=True) / H
    shared["dec_w"] = pack128(np.ascontiguousarray(gdec2.astype(np.float32)))
    bdec = f32w("ln2_b") @ f32w("dec_w") + f32w("dec_b")
    bias_pk = np.concatenate([f32w("gc1_b"), f32w("gc2_b"), f32w("proj_b"),
                              intb2]).reshape(42, 128).T
    shared["bias_pk"] = np.ascontiguousarray(bias_pk.astype(np.float32))
    qkb = np.concatenate([f32w("q_b"), f32w("k_b")]).reshape(16, DH).T
    shared["qkb_pk"] = np.ascontiguousarray(qkb)
    bcast = np.concatenate([f32w("ao_b"), f32w("v_b"), f32w("out_b"),
                            f32w("ln1_g"), f32w("ln1_b"), bdec])
    shared["bcast_pk"] = np.ascontiguousarray(bcast.astype(bf16))

    in_maps = []
    boff = (np.arange(BB) * S).astype(np.int32)[:, None, None]
    for i in range(NCORES):
        sl = slice(BB * i, BB * i + BB)
        cnm_i = cn[sl].astype(np.float32).reshape(NCL)
        am = np.zeros((NCL, NCL), dtype=np.float32)
        for b in range(BB):
            blk = (1.0 - cnm_i[b * M : (b + 1) * M]) * -10000.0
            am[b * M : (b + 1) * M, b * M : (b + 1) * M] = blk[None, :]
        wrm_i = wm[sl].astype(np.float32).reshape(NROW)
        lens_i = wrm_i.reshape(NCL, LC).sum(-1)
        lens_r = 1.0 / np.maximum(lens_i, 1.0)
        # block-diag wrm*lens_r for PE avg-pool: wseg[p, r, j] nonzero only
        # when row 128r+p belongs to clause 4r+j (j = p//32)
        wseg = np.zeros((128, RT, 4), dtype=np.float32)
        pidx = np.arange(128)
        for r in range(RT):
            rows = 128 * r + pidx
            j = pidx // 32
            wseg[pidx, r, j] = wrm_i[rows] * lens_r[rows // LC]
        adj_i = adj[sl].reshape(NCL, LC, LC)
        adjt = np.zeros((128, RT, 128), dtype=np.float32)
        for b in range(NCL):
            c, t = b % 4, b // 4
            adjt[32 * c : 32 * c + 32, t, 32 * c : 32 * c + 32] = adj_i[b].T
        enc_i = enc[sl].reshape(BB * S, H)
        d = dict(
            enc=np.ascontiguousarray(enc_i),
            gidx=np.ascontiguousarray((wr[sl] + boff).reshape(NROW).reshape(RT, 128).T),
            **{f"enc_s{i}": np.ascontiguousarray(enc_i[:, 192 * i : 192 * i + 192])
               for i in range(4)},
            wrm=np.ascontiguousarray(wrm_i.reshape(RT, 128).T),
            wrm_b=np.ascontiguousarray(wrm_i.astype(bf16)),
            lens=np.ascontiguousarray(lens_r),
            wseg=np.ascontiguousarray(wseg.astype(bf16).reshape(128, RT * 4)),
            cnm=np.ascontiguousarray(cnm_i),
            amask=np.ascontiguousarray(np.tile(am, (1, NH)).astype(bf16)),
            adjt=np.ascontiguousarray(adjt.astype(bf16)),
            tgt=np.ascontiguousarray(tl[sl].reshape(NCL, NL)),
        )
        d.update(shared)
        in_maps.append(d)
    return in_maps


def run_spmd(inputs, trace=False):
    nc = _get_program()
    in_maps = shard_inputs(inputs)
    kw = {}
    if trace:
        import types
        from trn_agent_boot.trn_boot import _ntff_profile_via_ctypes
        mod = types.ModuleType("antenv.axon_hooks")
        hook = _ntff_profile_via_ctypes("/opt/axon/libaxon_pjrt.so")
        mod.get_axon_ntff_profile_hook = lambda: hook
        mod.set_axon_ntff_profile_hook = lambda h: None
        sys.modules["antenv.axon_hooks"] = mod
        bass_utils.upload_artifacts = lambda tmpdir: "local://" + tmpdir
        kw["trace"] = True
    res = bass_utils.run_bass_kernel_spmd(nc, in_maps, core_ids=list(range(NCORES)), **kw)
    return res


def kernel(**inputs):
    res = run_spmd(inputs)
    num = 0.0
    den = 0.0
    for i in range(NCORES):
        o = res.results[i]["out"]
        num += float(o[0])
        den += float(o[1])
    loss = (num / NL) / den
    return np.asarray(loss, dtype=np.float32)


# revision 56
# speedup vs baseline: 1.1412x; 1.1412x over previous
"""Trainium2 Bass kernel for nn_MESGM_15857019256842.

Data-parallel over batch: 16 batches -> 8 cores x 2 batches.
Per core: gather clause tokens (indirect DMA), 2 GCN layers, max/avg pooling,
projection, 8-head self-attention over 32 clauses/batch, FFN, label decoder,
soft-label KL loss. Each core emits (sum kl*mask, sum mask); host combines.

v2: trace-driven rewrite of the baseline.
 - host packs block-diag transposed adjacency + bias vectors (layout prep)
 - one big 3D DMA per weight matrix, issued early and spread across queues
 - bf16 PE transposes for the gathered tokens
 - pooling done per 512-col block, split across DVE and GpSimd, overlapped
   with GCN matmuls; no H2T materialization (pool consumes z2 chunks)
 - z2 runs c-outer so each pooled chunk immediately feeds the projection
 - attention: full-row exp (no b-split/memset), transposed-ctx path that
   feeds ao directly, FFN produces inter transposed (no 24 transposes)
"""
import sys
sys.path.insert(0, '/opt/trn_rl_repo')
import numpy as np

from concourse import bass, mybir, tile
from concourse import bass_utils
from concourse.masks import make_identity
from concourse.vector_clock import ScopedClock

F32 = mybir.dt.float32
BF16 = mybir.dt.bfloat16
I32 = mybir.dt.int32
AF = mybir.ActivationFunctionType
AX = mybir.AxisListType
ALU = mybir.AluOpType

B, S, H, M, LC, NL, II, NH, DH = 16, 512, 768, 32, 32, 7, 3072, 8, 96
NCORES = 8
BB = B // NCORES          # 2 batches per core
NCL = BB * M              # 64 clauses per core
NROW = NCL * LC           # 2048 clause-token rows per core
RT = NROW // 128          # 16 row tiles
HC = H // 128             # 6 H chunks
IC = II // 128            # 24 intermediate chunks
LN_EPS = 1e-12
SQD = float(np.sqrt(DH))
NBC = 5 * H + NL          # broadcast-packed bias columns

_MAX_WAITS = 1


def _patched_drain_and_barrier(self, tick_clock, wait_clock):
    nc = self.nc
    drain_inst = nc.sync.drain()
    wait_clock.add_sem_waits(
        drain_inst.ins, ScopedClock({None: tick_clock.global_clock})
    )
    si = drain_inst.ins.sync_info
    waits = list(si.on_wait or [])
    if len(waits) > _MAX_WAITS:
        si.on_wait = waits[:_MAX_WAITS]
        rest = waits[_MAX_WAITS:]
        for i in range(0, len(rest), _MAX_WAITS):
            nop = nc.sync.nop(nofuse=True)
            nop.ins.sync_info = mybir.SyncInfo(
                on_wait=rest[i : i + _MAX_WAITS], on_update=[]
            )
    nc.all_engine_barrier()
    assert self.sems is not None
    popped = nc._tile_sem_poison_stack.pop()
    assert popped is self._sem_poison
    nc.clear_and_free_semaphores(list(self.sems.allocated().values()))
    nc.all_engine_barrier()


tile.TileContext._drain_and_barrier = _patched_drain_and_barrier


def legalize_waits(nc, limit=1):
    """TRN2 instructions carry at most one sem wait; hoist extras onto nops."""
    nfix = 0
    for blk in nc.main_func.blocks:
        insts = list(blk.instructions)
        pos = 0
        for inst in insts:
            si = inst.sync_info
            waits = list(si.on_wait) if si is not None and si.on_wait else []
            if len(waits) > limit:
                si.on_wait = waits[-limit:]
                rest = waits[:-limit]
                eng = nc.engines[inst.engine]
                for j in range(0, len(rest), limit):
                    nop = eng.nop(nofuse=True)
                    nop.ins.sync_info = mybir.SyncInfo(
                        on_wait=rest[j : j + limit], on_update=[]
                    )
                    src_blk = nc.cur_bb.bb
                    popped = src_blk.instructions.pop()
                    assert popped.name == nop.ins.name
                    blk.instructions.insert(pos, nop.ins)
                    pos += 1
                nfix += 1
            pos += 1
    return nfix


DEBUG = False


def build_program():
    nc = bass.Bass(trn_type="TRN2")

    # ---- DRAM I/O --------------------------------------------------------
    enc = nc.dram_tensor("enc", [BB * S, H], F32, kind="ExternalInput")
    enc_s = [nc.dram_tensor(f"enc_s{i}", [BB * S, 192], F32, kind="ExternalInput")
             for i in range(4)]
    gidx = nc.dram_tensor("gidx", [128, RT], I32, kind="ExternalInput")
    wrm = nc.dram_tensor("wrm", [128, RT], F32, kind="ExternalInput")
    wrm_b = nc.dram_tensor("wrm_b", [NROW], BF16, kind="ExternalInput")
    lens = nc.dram_tensor("lens", [NCL], F32, kind="ExternalInput")
    wseg = nc.dram_tensor("wseg", [128, RT * 4], BF16, kind="ExternalInput")
    cnm = nc.dram_tensor("cnm", [NCL], F32, kind="ExternalInput")
    amask = nc.dram_tensor("amask", [NCL, NH * NCL], BF16, kind="ExternalInput")
    adjt = nc.dram_tensor("adjt", [128, RT, 128], BF16, kind="ExternalInput")
    tgt = nc.dram_tensor("tgt", [NCL, NL], F32, kind="ExternalInput")
    bias_pk = nc.dram_tensor("bias_pk", [128, 42], F32, kind="ExternalInput")
    qkb_pk = nc.dram_tensor("qkb_pk", [DH, 16], F32, kind="ExternalInput")
    bcast_pk = nc.dram_tensor("bcast_pk", [NBC], BF16, kind="ExternalInput")

    # all weights arrive host-packed in their SBUF layout: [128, chunks*cols]
    w = {}
    for name, shp, dt in [
        ("gc1_w", [128, HC * H], BF16), ("gc2_w", [128, HC * H], BF16),
        ("proj_w", [128, IC * H], BF16),
        ("q_w", [128, HC * H], BF16), ("k_w", [128, HC * H], BF16),
        ("v_w", [128, HC * H], BF16),
        ("ao_w", [DH, NH * H], BF16),
        ("int_w", [128, HC * II], BF16), ("out_w", [128, IC * H], BF16),
        ("dec_w", [128, HC * NL], F32),
    ]:
        w[name] = nc.dram_tensor(name, shp, dt, kind="ExternalInput")

    out_d = nc.dram_tensor("out", [2], F32, kind="ExternalOutput")
    dbg = {}
    if DEBUG:
        dbg["cv"] = nc.dram_tensor("dbg_cv", [NCL, H], F32, kind="ExternalOutput")
        dbg["attn"] = nc.dram_tensor("dbg_attn", [NCL, H], F32, kind="ExternalOutput")
        dbg["pred"] = nc.dram_tensor("dbg_pred", [NCL, NL], F32, kind="ExternalOutput")
        dbg["pool"] = nc.dram_tensor("dbg_pool", [128, 24, NCL], F32, kind="ExternalOutput")

    with tile.TileContext(nc) as tc:
        _body(nc, tc, enc, enc_s, gidx, wrm, wrm_b, lens, wseg, cnm, amask, adjt, tgt,
              bias_pk, qkb_pk, bcast_pk, w, out_d, dbg)

    nfix = legalize_waits(nc)
    return nc, nfix


def _body(nc, tc, enc, enc_s, gidx, wrm, wrm_b, lens, wseg, cnm, amask, adjt, tgt,
          bias_pk, qkb_pk, bcast_pk, w, out_d, dbg):
    from contextlib import ExitStack
    ctx = ExitStack()
    with ctx:
        # pool stack (LIFO). Pools reserve their full footprint at open, so
        # big pools open only for their live window:
        #   pp -> wat -> pjps -> wgcn -> yn16 -> pjwp -> ph1 -> pxm
        #   closes: pxm (GCN1 end), ph1 (y2 end), pjwp/yn16/wgcn (z2 end),
        #   pjps (after cvT), then p2 opens for the attention phase.
        pp = ctx.enter_context(tc.tile_pool(name="persist", bufs=1))

        # identities
        ident = pp.tile([128, 128], F32, tag="ident")
        make_identity(nc, ident[:])
        ident_b = pp.tile([128, 128], BF16, tag="identb")
        nc.vector.tensor_copy(out=ident_b[:], in_=ident[:])
        tprime = pp.tile([1, 1], F32, tag="tprime")
        nc.scalar.copy(out=tprime[:], in_=ident[0:1, 0:1])  # prime ACT table

        # --- input / small-tensor DMAs (spread across queues) -------------
        gidx_t = pp.tile([128, RT], I32, tag="gidx")
        nc.sync.dma_start(out=gidx_t[:], in_=bass.AP(tensor=gidx, offset=0, ap=[[RT, 128], [1, RT]]))
        wrm_pp = pp.tile([128, RT], F32, tag="wrmpp")
        nc.sync.dma_start(out=wrm_pp[:], in_=bass.AP(tensor=wrm, offset=0, ap=[[RT, 128], [1, RT]]))
        wseg_sb = pp.tile([128, RT, 4], BF16, tag="wseg")
        nc.sync.dma_start(out=wseg_sb[:].rearrange("p a b -> p (a b)"), in_=bass.AP(tensor=wseg, offset=0, ap=[[RT * 4, 128], [1, RT * 4]]))

        # phase-2 matmul weights + cvT (live to the end; reserved at open)
        wat = ctx.enter_context(tc.tile_pool(name="wat", bufs=1))

        # gcn weights + adjacency (space frees after z2)
        wgcn_stack = ExitStack()
        wg = wgcn_stack.enter_context(tc.tile_pool(name="wgcn", bufs=1))
        gc1w = wg.tile([128, HC, H], BF16, tag="gc1w")
        nc.sync.dma_start(out=gc1w[:].rearrange("p a b -> p (a b)"), in_=bass.AP(tensor=w["gc1_w"], offset=0, ap=[[HC * H, 128], [1, HC * H]]))
        adjt_sb = wg.tile([128, RT, 128], BF16, tag="adjt")
        nc.sync.dma_start(out=adjt_sb[:].rearrange("p a b -> p (a b)"), in_=bass.AP(tensor=adjt, offset=0, ap=[[RT * 128, 128], [1, RT * 128]]))
        gc2w = wg.tile([128, HC, H], BF16, tag="gc2w")
        nc.sync.dma_start(out=gc2w[:].rearrange("p a b -> p (a b)"), in_=bass.AP(tensor=w["gc2_w"], offset=0, ap=[[HC * H, 128], [1, HC * H]]))
        bcast_all = pp.tile([NCL, NBC], BF16, tag="bcall")
        nc.sync.dma_start(out=bcast_all[:], in_=bass.AP(tensor=bcast_pk, offset=0, ap=[[0, NCL], [1, NBC]]))
        amask8 = pp.tile([NCL, NH, NCL], BF16, tag="amask8")
        nc.sync.dma_start(out=amask8[:].rearrange("p a b -> p (a b)"), in_=bass.AP(tensor=amask, offset=0, ap=[[NH * NCL, NCL], [1, NH * NCL]]))
        wrm_bcb = pp.tile([128, NROW], BF16, tag="wrmbcb")
        nc.sync.dma_start(out=wrm_bcb[:], in_=bass.AP(tensor=wrm_b, offset=0, ap=[[0, 128], [1, NROW]]))
        # small tensors on the sync queue, priority order; scalar queue is
        # kept free for the gather masks
        bias_sb = pp.tile([128, 42], F32, tag="biaspk")
        nc.sync.dma_start(out=bias_sb[:], in_=bass.AP(tensor=bias_pk, offset=0, ap=[[42, 128], [1, 42]]))
        qkb_sb = pp.tile([DH, 16], F32, tag="qkb")
        nc.sync.dma_start(out=qkb_sb[:], in_=bass.AP(tensor=qkb_pk, offset=0, ap=[[16, DH], [1, 16]]))
        cnm_pp = pp.tile([NCL, 1], F32, tag="cnmpp")
        nc.sync.dma_start(out=cnm_pp[:], in_=cnm[:, None])
        lens_bc = pp.tile([128, NCL], F32, tag="lensbc")
        nc.sync.dma_start(out=lens_bc[:], in_=bass.AP(tensor=lens, offset=0, ap=[[0, 128], [1, NCL]]))
        tgt_sb = pp.tile([NCL, NL], F32, tag="tgtsb")
        nc.sync.dma_start(out=tgt_sb[:], in_=tgt[:, :])
        eps_t = pp.tile([NCL, 1], F32, tag="epst")
        nc.vector.memset(eps_t[:], LN_EPS)
        ones_t = pp.tile([NCL, 1], F32, tag="onest")
        nc.vector.memset(ones_t[:], 1.0)

        # bias_pk columns: gc1_b 0:6, gc2_b 6:12, proj_b 12:18, int_b 18:42
        gb1 = bias_sb[:, 0:HC]
        gb2 = bias_sb[:, HC : 2 * HC]
        projb = bias_sb[:, 2 * HC : 3 * HC]

        PT = pp.tile([128, 24, NCL], BF16, tag="PT")

        # attention weights early: wat space is reserved from open, so these
        # carry no space-reuse waits and transfer during the gather
        qw = wat.tile([128, HC, H], BF16, tag="qw")
        nc.sync.dma_start(out=qw[:].rearrange("p a b -> p (a b)"), in_=bass.AP(tensor=w["q_w"], offset=0, ap=[[HC * H, 128], [1, HC * H]]))
        kw = wat.tile([128, HC, H], BF16, tag="kw")
        nc.sync.dma_start(out=kw[:].rearrange("p a b -> p (a b)"), in_=bass.AP(tensor=w["k_w"], offset=0, ap=[[HC * H, 128], [1, HC * H]]))
        dw = wat.tile([128, HC, NL], F32, tag="dw")
        nc.sync.dma_start(out=dw[:].rearrange("p a b -> p (a b)"), in_=bass.AP(tensor=w["dec_w"], offset=0, ap=[[HC * NL, 128], [1, HC * NL]]))

        # big transposed activation tiles + rotating proj_w quarters
        pjwp_stack = ExitStack()
        pjwp = pjwp_stack.enter_context(tc.tile_pool(name="pjwp", bufs=3))
        ph1_stack = ExitStack()
        ph1 = ph1_stack.enter_context(tc.tile_pool(name="ph1", bufs=1))
        H1T = ph1.tile([128, HC, NROW], BF16, tag="H1T")
        pxm_stack = ExitStack()
        pxm = pxm_stack.enter_context(tc.tile_pool(name="pxm", bufs=1))
        XmT = pxm.tile([128, HC, NROW], BF16, tag="XmT")

        # =================== phase 1: gather + transpose + x-pool =========
        def y_block(XT, wt, g, tag, ypool, psum_pool, evac="mixed"):
            yns = []
            for rr in range(4):
                r = 4 * g + rr
                p1 = psum_pool.tile([128, 512], F32, tag="y1", name=f"y1_{tag}{r}")
                p2 = psum_pool.tile([128, 256], F32, tag="y2", name=f"y2_{tag}{r}")
                for c in range(HC):
                    lhs = XT[:, c, r * 128 : r * 128 + 128]
                    nc.tensor.matmul(out=p1[:], lhsT=lhs, rhs=wt[:, c, 0:512],
                                     start=(c == 0), stop=(c == HC - 1))
                    nc.tensor.matmul(out=p2[:], lhsT=lhs, rhs=wt[:, c, 512:768],
                                     start=(c == 0), stop=(c == HC - 1))
                yr = ypool.tile([128, H], BF16, tag=f"yn{rr}", name=f"yn_{tag}{r}")
                if evac == "scalar":
                    nc.scalar.copy(out=yr[:, 0:512], in_=p1[:])
                else:
                    nc.vector.tensor_copy(out=yr[:, 0:512], in_=p1[:])
                nc.scalar.copy(out=yr[:, 512:768], in_=p2[:])
                yns.append(yr)
            return yns

        with tc.tile_pool(name="xg", bufs=2) as xgp, \
             tc.tile_pool(name="xb", bufs=2) as xbp, \
             tc.tile_pool(name="xps", bufs=1, space="PSUM") as xps, \
             tc.tile_pool(name="tps", bufs=2, space="PSUM") as tps, \
             tc.tile_pool(name="ynat", bufs=2) as gcn_yn, \
             tc.tile_pool(name="gps", bufs=2, space="PSUM") as gps, \
             tc.tile_pool(name="zps", bufs=1, space="PSUM") as zps:

            def z1_block(g, yns):
                for c in range(HC):
                    zp = zps.tile([128, 512], F32, tag="z", name=f"z_l1{g}_{c}")
                    for rr in range(4):
                        nc.tensor.matmul(
                            out=zp[:, rr * 128 : rr * 128 + 128],
                            lhsT=yns[rr][:, c * 128 : c * 128 + 128],
                            rhs=adjt_sb[:, 4 * g + rr, :],
                            start=True, stop=True,
                        )
                    nc.scalar.activation(
                        out=H1T[:, c, g * 512 : g * 512 + 512], in_=zp[:],
                        func=AF.Relu, bias=gb1[:, c : c + 1], scale=1.0,
                    )

            prev = None
            for g in range(4):
                xt = xgp.tile([128, 4, H], F32, tag="xg", name=f"xg{g}")
                xb = xbp.tile([128, 4, H], BF16, tag="xb", name=f"xb{g}")
                for rr in range(4):
                    r = 4 * g + rr
                    if r == 0:
                        for piece in range(4):
                            a, b = piece * 192, piece * 192 + 192
                            nc.gpsimd.indirect_dma_start(
                                out=xt[:, rr, a:b], out_offset=None, in_=enc_s[piece][:],
                                in_offset=bass.IndirectOffsetOnAxis(ap=gidx_t[:, r : r + 1], axis=0),
                            )
                            nc.scalar.mul(out=xb[:, rr, a:b], in_=xt[:, rr, a:b],
                                          mul=wrm_pp[:, r : r + 1])
                    else:
                        nc.gpsimd.indirect_dma_start(
                            out=xt[:, rr, :], out_offset=None, in_=enc[:],
                            in_offset=bass.IndirectOffsetOnAxis(ap=gidx_t[:, r : r + 1], axis=0),
                        )
                        nc.scalar.mul(out=xb[:, rr, :], in_=xt[:, rr, :], mul=wrm_pp[:, r : r + 1])
                for c in range(HC):
                    ps = tps.tile([128, 512], BF16, tag="tp", name=f"tp{g}_{c}")
                    for rr in range(4):
                        nc.tensor.transpose(
                            out=ps[:, rr * 128 : rr * 128 + 128],
                            in_=xb[:, rr, c * 128 : c * 128 + 128],
                            identity=ident_b[:],
                        )
                    psx = xps.tile([128, 16], F32, tag="psx", name=f"psx{g}_{c}")
                    for rr in range(4):
                        nc.tensor.matmul(out=psx[:, rr * 4 : rr * 4 + 4],
                                         lhsT=xb[:, rr, c * 128 : c * 128 + 128],
                                         rhs=wseg_sb[:, 4 * g + rr, :],
                                         start=True, stop=True)
                    if (c + g) % 2 == 0:
                        nc.vector.tensor_copy(out=XmT[:, c, g * 512 : g * 512 + 512], in_=ps[:])
                        nc.scalar.copy(out=PT[:, 12 + c, g * 16 : g * 16 + 16], in_=psx[:])
                    else:
                        nc.scalar.copy(out=XmT[:, c, g * 512 : g * 512 + 512], in_=ps[:])
                        nc.vector.tensor_copy(out=PT[:, 12 + c, g * 16 : g * 16 + 16], in_=psx[:])
                    v = XmT[:, c, g * 512 : g * 512 + 512].rearrange("p (n l) -> p n l", l=LC)
                    nc.vector.reduce_max(out=PT[:, c, g * 16 : g * 16 + 16], in_=v, axis=AX.X)
                yns = y_block(XmT, gc1w, g, "l1", gcn_yn, gps)
                if prev is not None:
                    z1_block(prev[0], prev[1])
                prev = (g, yns)
            z1_block(prev[0], prev[1])

        # proj_w quarters 0-2 (sync queue; gpsimd slots stay free for gathers)
        pjq = []
        for q in range(3):
            t = pjwp.tile([128, HC, H], BF16, tag="pjw", name=f"pjw{q}")
            nc.sync.dma_start(out=t[:].rearrange("p a b -> p (a b)"), in_=bass.AP(
                tensor=w["proj_w"], offset=q * HC * H,
                ap=[[IC * H, 128], [1, HC * H]]))
            pjq.append(t)

        pxm_stack.close()

        # projection psum (phase-1 psum pools closed now; spans GCN2..cvT)
        pjps_stack = ExitStack()
        pjps = pjps_stack.enter_context(tc.tile_pool(name="pjps", bufs=1, space="PSUM"))
        pcs = pjps.tile([128, HC, NCL], F32, tag="pj")

        # v / ao weights
        vw = wat.tile([128, HC, H], BF16, tag="vw")
        nc.scalar.dma_start(out=vw[:].rearrange("p a b -> p (a b)"), in_=bass.AP(tensor=w["v_w"], offset=0, ap=[[HC * H, 128], [1, HC * H]]))
        aow = wat.tile([DH, NH, H], BF16, tag="aow")
        nc.scalar.dma_start(out=aow[:].rearrange("p a b -> p (a b)"), in_=bass.AP(tensor=w["ao_w"], offset=0, ap=[[NH * H, DH], [1, NH * H]]))

        korder = list(range(12, 18)) + list(range(0, 6)) + \
            [k for c in range(HC) for k in (6 + c, 18 + c)]

        def proj_chunk(k, ki):
            kq, kr = divmod(k, HC)
            for m in range(HC):
                nc.tensor.matmul(out=pcs[:, m, :], lhsT=pjq[kq][:, kr, m * 128 : m * 128 + 128],
                                 rhs=PT[:, k, :], start=(ki == 0), stop=(ki == 23))
        for ki in range(12):
            proj_chunk(korder[ki], ki)
        t = pjwp.tile([128, HC, H], BF16, tag="pjw", name="pjw3")
        nc.sync.dma_start(out=t[:].rearrange("p a b -> p (a b)"), in_=bass.AP(
            tensor=w["proj_w"], offset=3 * HC * H, ap=[[IC * H, 128], [1, HC * H]]))
        pjq.append(t)

        # layer 2: per group g, y2(g) then z2 blocks (c, g) + pooling;
        # proj h2 chunks fire inside the last group as chunks complete.
        with tc.tile_pool(name="gps2", bufs=2, space="PSUM") as gps2, \
             tc.tile_pool(name="yn16", bufs=2) as yn16, \
             tc.tile_pool(name="h2b", bufs=3) as h2bp, \
             tc.tile_pool(name="h2m", bufs=3) as h2mp, \
             tc.tile_pool(name="xs2", bufs=3) as xs2p, \
             tc.tile_pool(name="z2ps", bufs=3, space="PSUM") as z2ps:
            for g in range(4):
                yg = y_block(H1T, gc2w, g, "l2", yn16, gps2, evac="scalar")
                for c in range(HC):
                    zp = z2ps.tile([128, 512], F32, tag="z2", name=f"z_l2{g}_{c}")
                    for rr in range(4):
                        nc.tensor.matmul(
                            out=zp[:, rr * 128 : rr * 128 + 128],
                            lhsT=yg[rr][:, c * 128 : c * 128 + 128],
                            rhs=adjt_sb[:, 4 * g + rr, :],
                            start=True, stop=True,
                        )
                    hb = h2bp.tile([128, 512], BF16, tag="h2b", name=f"h2b{g}_{c}")
                    nc.scalar.activation(out=hb[:], in_=zp[:], func=AF.Relu,
                                         bias=gb2[:, c : c + 1], scale=1.0)
                    hm = h2mp.tile([128, 512], BF16, tag="h2m", name=f"h2m{g}_{c}")
                    nc.gpsimd.tensor_tensor(out=hm[:], in0=hb[:],
                                            in1=wrm_bcb[:, g * 512 : g * 512 + 512], op=ALU.mult)
                    v = hm[:].rearrange("p (n l) -> p n l", l=LC)
                    nc.vector.reduce_max(out=PT[:, 6 + c, g * 16 : g * 16 + 16], in_=v, axis=AX.X)
                    xs = xs2p.tile([128, 16], F32, tag="xs2", name=f"xs2{g}_{c}")
                    nc.vector.reduce_sum(out=xs[:], in_=v, axis=AX.X)
                    nc.gpsimd.tensor_tensor(
                        out=PT[:, 18 + c, g * 16 : g * 16 + 16], in0=xs[:],
                        in1=lens_bc[:, g * 16 : g * 16 + 16], op=ALU.mult)
                    if g == 3:
                        proj_chunk(6 + c, 12 + 2 * c)
                        proj_chunk(18 + c, 13 + 2 * c)

        ph1_stack.close()

        # cv^T = relu(proj + b) straight out of the proj psum
        cvT = wat.tile([128, HC, NCL], BF16, tag="cvT")
        for m in range(HC):
            nc.scalar.activation(out=cvT[:, m, :], in_=pcs[:, m, :], func=AF.Relu,
                                 bias=projb[:, m : m + 1], scale=1.0)
        pjps_stack.close()
        pjwp_stack.close()
        wgcn_stack.close()
        if DEBUG:
            with tc.tile_pool(name="dbgp", bufs=1) as dp:
                ptf = dp.tile([128, 24, NCL], F32, tag="ptdbg")
                nc.vector.tensor_copy(out=ptf[:], in_=PT[:])
                nc.sync.dma_start(out=dbg["pool"][:, :, :], in_=ptf[:])

        # =================== phase 2: attention + FFN + KL ================
        p2 = ctx.enter_context(tc.tile_pool(name="p2", bufs=1))
        # bcast_pk: ao_b, v_b, out_b, ln1_g, ln1_b, ln2_g, ln2_b, bdec
        aob_bc = bcast_all[:, 0:H]
        vb_bc = bcast_all[:, H : 2 * H]
        outb_bc = bcast_all[:, 2 * H : 3 * H]
        l1g_bc = bcast_all[:, 3 * H : 4 * H]
        l1b_bc = bcast_all[:, 4 * H : 5 * H]
        decb_bc = bcast_all[:, 5 * H : 5 * H + NL]

        # FFN weights stream into space freed by H1T/yn16/pjw
        wf_stack = ExitStack()
        wf = wf_stack.enter_context(tc.tile_pool(name="wf", bufs=1))
        iw = wf.tile([128, HC, II], BF16, tag="iw")
        for half in range(2):
            nc.sync.dma_start(
                out=iw[:, 3 * half : 3 * half + 3, :].rearrange("p a b -> p (a b)"),
                in_=bass.AP(tensor=w["int_w"], offset=half * 3 * II,
                            ap=[[HC * II, 128], [1, 3 * II]]))
        ow = wf.tile([128, IC, H], BF16, tag="ow")
        for qtr in range(4):
            nc.sync.dma_start(
                out=ow[:, 6 * qtr : 6 * qtr + 6, :].rearrange("p a b -> p (a b)"),
                in_=bass.AP(tensor=w["out_w"], offset=qtr * 6 * H,
                            ap=[[IC * H, 128], [1, 6 * H]]))

        QT = p2.tile([DH, NH, NCL], BF16, tag="QT")
        KT = p2.tile([DH, NH, NCL], BF16, tag="KT")
        Vn = p2.tile([NCL, H], BF16, tag="Vn")
        att8 = p2.tile([NCL, NH, NCL], BF16, tag="att8")
        attS = p2.tile([NCL, NH, NCL], BF16, tag="attS")
        sums_t = p2.tile([NCL, NH], F32, tag="sums")
        recip_t = p2.tile([NCL, NH], F32, tag="recip")
        negmax = p2.tile([NCL, NH], F32, tag="negmax")
        cv_pa = p2.tile([NCL, H], F32, tag="cvpa")

        # scores psum prefilled with the attention mask (matmuls accumulate
        # on top); the copy is issued early so it never gates the scores.
        sc_stack = ExitStack()
        scps = sc_stack.enter_context(tc.tile_pool(name="scps", bufs=1, space="PSUM"))
        pss = scps.tile([NCL, NH, NCL], F32, tag="scores")
        nc.vector.tensor_copy(out=pss[:], in_=amask8[:])

        with tc.tile_pool(name="qkps", bufs=2, space="PSUM") as qkps, \
             tc.tile_pool(name="vps", bufs=1, space="PSUM") as vps, \
             tc.tile_pool(name="cvt2", bufs=3, space="PSUM") as cvt2:
            psq = qkps.tile([DH, NH * NCL], F32, tag="qk", name="psq")
            for h in range(NH):
                for c in range(HC):
                    nc.tensor.matmul(out=psq[:, h * NCL : h * NCL + NCL],
                                     lhsT=qw[:, c, h * DH : h * DH + DH],
                                     rhs=cvT[:, c, :], start=(c == 0), stop=(c == HC - 1))
            # Q evacs on DVE: (psq + q_b) / sqrt(dh)
            for h in range(NH):
                nc.vector.tensor_scalar(out=QT[:, h, :], in0=psq[:, h * NCL : h * NCL + NCL],
                                        scalar1=qkb_sb[:, h : h + 1], scalar2=1.0 / SQD,
                                        op0=ALU.add, op1=ALU.mult)
            psk = qkps.tile([DH, NH * NCL], F32, tag="qk", name="psk")
            for h in range(NH):
                for c in range(HC):
                    nc.tensor.matmul(out=psk[:, h * NCL : h * NCL + NCL],
                                     lhsT=kw[:, c, h * DH : h * DH + DH],
                                     rhs=cvT[:, c, :], start=(c == 0), stop=(c == HC - 1))
            for h in range(NH):
                nc.scalar.activation(out=KT[:, h, :], in_=psk[:, h * NCL : h * NCL + NCL],
                                     func=AF.Identity, bias=qkb_sb[:, 8 + h : 9 + h], scale=1.0)

            # scores straight after K; V runs on the PE behind them
            for h in range(NH):
                nc.tensor.matmul(out=pss[:, h, :], lhsT=QT[:, h, :], rhs=KT[:, h, :],
                                 start=False, stop=True, skip_group_check=True)
            pv1 = vps.tile([NCL, 512], F32, tag="v1")
            pv2 = vps.tile([NCL, 256], F32, tag="v2")
            for c in range(HC):
                nc.tensor.matmul(out=pv1[:], lhsT=cvT[:, c, :], rhs=vw[:, c, 0:512],
                                 start=(c == 0), stop=(c == HC - 1))
                nc.tensor.matmul(out=pv2[:], lhsT=cvT[:, c, :], rhs=vw[:, c, 512:768],
                                 start=(c == 0), stop=(c == HC - 1))
            # cv natural + ao_b residual base (PE transposes behind V)
            for c in range(HC):
                ps = cvt2.tile([NCL, 128], BF16, tag="cvn", name=f"cvn{c}")
                nc.tensor.transpose(out=ps[:], in_=cvT[:, c, :], identity=ident_b[:])
                nc.vector.tensor_tensor(out=cv_pa[:, c * 128 : c * 128 + 128], in0=ps[:],
                                        in1=aob_bc[:, c * 128 : c * 128 + 128], op=ALU.add)
            nc.vector.tensor_tensor(out=Vn[:, 0:512], in0=pv1[:], in1=vb_bc[:, 0:512], op=ALU.add)
            nc.vector.tensor_tensor(out=Vn[:, 512:768], in0=pv2[:], in1=vb_bc[:, 512:768], op=ALU.add)

        nc.vector.tensor_reduce(out=negmax[:], in_=pss[:], axis=AX.X,
                                op=ALU.max, negate=True)
        for h in range(NH):
            nc.scalar.activation(
                out=att8[:, h, :], in_=pss[:, h, :], func=AF.Exp,
                bias=negmax[:, h : h + 1], scale=1.0,
            )
        nc.vector.reduce_sum(out=sums_t[:], in_=att8[:], axis=AX.X)
        nc.vector.reciprocal(out=recip_t[:], in_=sums_t[:])
        sc_stack.close()

        # per-head: scale, transpose, ctx^T = V-block @ att^T, ao accum
        ctxT = p2.tile([DH, NH, NCL], BF16, tag="ctxT")
        attn_out = p2.tile([NCL, H], F32, tag="attnout")
        xhat = p2.tile([NCL, H], F32, tag="xhat")
        ln_in = p2.tile([NCL, H], F32, tag="lnin1")
        with tc.tile_pool(name="aops", bufs=1, space="PSUM") as aops:
            pa1 = aops.tile([NCL, 512], F32, tag="ao1")
            pa2 = aops.tile([NCL, 256], F32, tag="ao2")
            with tc.tile_pool(name="ctps", bufs=3, space="PSUM") as ctps, \
                 tc.tile_pool(name="atts", bufs=3) as atts:
                for h in range(NH):
                    nc.scalar.mul(out=attS[:, h, :], in_=att8[:, h, :], mul=recip_t[:, h : h + 1])
                    pst = ctps.tile([NCL, NCL], BF16, tag="attT", name=f"attT{h}")
                    nc.tensor.transpose(out=pst[:], in_=attS[:, h, :], identity=ident_b[:64, :64])
                    asb = atts.tile([NCL, NCL], BF16, tag="attTs", name=f"attTs{h}")
                    nc.vector.tensor_copy(out=asb[:], in_=pst[:])
                    pctx = ctps.tile([DH, NCL], F32, tag="ctx", name=f"ctx{h}")
                    nc.tensor.matmul(out=pctx[:], lhsT=Vn[:, h * DH : h * DH + DH], rhs=asb[:],
                                     start=True, stop=True)
                    if h % 2 == 0:
                        nc.vector.tensor_copy(out=ctxT[:, h, :], in_=pctx[:])
                    else:
                        nc.scalar.copy(out=ctxT[:, h, :], in_=pctx[:])
                for h in range(NH):
                    nc.tensor.matmul(out=pa1[:], lhsT=ctxT[:, h, :], rhs=aow[:, h, 0:512],
                                     start=(h == 0), stop=(h == NH - 1))
                    nc.tensor.matmul(out=pa2[:], lhsT=ctxT[:, h, :], rhs=aow[:, h, 512:768],
                                     start=(h == 0), stop=(h == NH - 1))

            def layer_norm(x_nat, g_bc, b_bc, y_nat, lnp):
                stats = lnp.tile([NCL, 3, 6], F32, tag="lnstats")
                for i in range(3):
                    nc.vector.bn_stats(out=stats[:, i, :], in_=x_nat[:, i * 256 : i * 256 + 256])
                mv = lnp.tile([NCL, 2], F32, tag="lnmv")
                nc.vector.bn_aggr(out=mv[:], in_=stats[:])
                sd = lnp.tile([NCL, 1], F32, tag="lnsd")
                nc.scalar.activation(out=sd[:], in_=mv[:, 1:2], func=AF.Sqrt, bias=eps_t[:, :1], scale=1.0)
                rstd = lnp.tile([NCL, 1], F32, tag="lnrstd")
                nc.vector.reciprocal(out=rstd[:], in_=sd[:])
                xc = lnp.tile([NCL, H], F32, tag="lnxc")
                nc.vector.tensor_scalar(out=xc[:], in0=x_nat[:], scalar1=mv[:, 0:1],
                                        scalar2=rstd[:, :1], op0=ALU.subtract, op1=ALU.mult)
                nc.vector.tensor_tensor(out=xc[:], in0=xc[:], in1=g_bc, op=ALU.mult)
                nc.vector.tensor_tensor(out=y_nat[:], in0=xc[:], in1=b_bc, op=ALU.add)

            with tc.tile_pool(name="ln1p", bufs=1) as lnp:
                nc.vector.tensor_tensor(out=ln_in[:, 0:512], in0=pa1[:], in1=cv_pa[:, 0:512], op=ALU.add)
                nc.vector.tensor_tensor(out=ln_in[:, 512:768], in0=pa2[:], in1=cv_pa[:, 512:768], op=ALU.add)
                # LN1 split: FFN consumes xhat directly (ln1_g/ln1_b are
                # host-folded into int_w/int_b); the residual (xhat*g+b)
                # is computed off the critical path.
                stats = lnp.tile([NCL, 3, 6], F32, tag="lnstats")
                for i in range(3):
                    nc.vector.bn_stats(out=stats[:, i, :], in_=ln_in[:, i * 256 : i * 256 + 256])
                mv = lnp.tile([NCL, 2], F32, tag="lnmv")
                nc.vector.bn_aggr(out=mv[:], in_=stats[:])
                sd = lnp.tile([NCL, 1], F32, tag="lnsd")
                nc.scalar.activation(out=sd[:], in_=mv[:, 1:2], func=AF.Sqrt, bias=eps_t[:, :1], scale=1.0)
                rstd = lnp.tile([NCL, 1], F32, tag="lnrstd")
                nc.vector.reciprocal(out=rstd[:], in_=sd[:])
                nc.vector.tensor_scalar(out=xhat[:], in0=ln_in[:], scalar1=mv[:, 0:1],
                                        scalar2=rstd[:, :1], op0=ALU.subtract, op1=ALU.mult)
        if DEBUG:
            nc.sync.dma_start(out=dbg["attn"][:, :], in_=attn_out[:])

        # aoT = xhat^T (bf16); the residual attn_out = xhat*g1 + b1 runs on
        # DVE in parallel with the FFN matmuls
        aoT = p2.tile([128, HC, NCL], BF16, tag="aoT")
        with tc.tile_pool(name="aotps", bufs=1, space="PSUM") as aotps:
            psA = aotps.tile([128, HC * NCL], F32, tag="psA")
            for c in range(HC):
                nc.tensor.transpose(out=psA[:, c * NCL : c * NCL + NCL],
                                    in_=xhat[:, c * 128 : c * 128 + 128],
                                    identity=ident[:64, :64])
            nc.vector.tensor_copy(out=aoT[:].rearrange("p a b -> p (a b)"), in_=psA[:])
        nc.vector.tensor_tensor(out=attn_out[:], in0=xhat[:], in1=l1g_bc, op=ALU.mult)
        nc.vector.tensor_tensor(out=attn_out[:], in0=attn_out[:], in1=l1b_bc, op=ALU.add)

        # FFN: inter produced transposed, chunk by chunk, feeding out-proj
        ln_in2 = p2.tile([NCL, H], F32, tag="lnin2")
        with tc.tile_pool(name="fips", bufs=4, space="PSUM") as fips, \
             tc.tile_pool(name="fit", bufs=4) as fit, \
             tc.tile_pool(name="fops", bufs=1, space="PSUM") as fops, \
             tc.tile_pool(name="ln2p", bufs=1) as lnp2:
            po1 = fops.tile([NCL, 512], F32, tag="o1")
            po2 = fops.tile([NCL, 256], F32, tag="o2")
            for cc in range(IC):
                ip = fips.tile([128, NCL], F32, tag="fi", name=f"fi{cc}")
                for c in range(HC):
                    nc.tensor.matmul(out=ip[:], lhsT=iw[:, c, cc * 128 : cc * 128 + 128],
                                     rhs=aoT[:, c, :], start=(c == 0), stop=(c == HC - 1))
                it = fit.tile([128, NCL], BF16, tag="it", name=f"it{cc}")
                nc.scalar.activation(out=it[:], in_=ip[:], func=AF.Gelu,
                                     bias=bias_sb[:, 18 + cc : 19 + cc], scale=1.0)
                nc.tensor.matmul(out=po1[:], lhsT=it[:], rhs=ow[:, cc, 0:512],
                                 start=(cc == 0), stop=(cc == IC - 1))
                nc.tensor.matmul(out=po2[:], lhsT=it[:], rhs=ow[:, cc, 512:768],
                                 start=(cc == 0), stop=(cc == IC - 1))
            nc.vector.tensor_tensor(out=ln_in2[:, 0:512], in0=po1[:], in1=attn_out[:, 0:512], op=ALU.add)
            nc.vector.tensor_tensor(out=ln_in2[:, 512:768], in0=po2[:], in1=attn_out[:, 512:768], op=ALU.add)
            nc.vector.tensor_tensor(out=ln_in2[:], in0=ln_in2[:], in1=outb_bc[:], op=ALU.add)
            # LN2 folded into the decoder: pred = rstd*(ln_in2 @ gdec2) + bdec
            # (gdec2 = ln2_g*dec_w - colsum/H host-folded). Only the rstd
            # stats chain remains; the transpose runs in parallel with it.
            stats2 = lnp2.tile([NCL, 3, 6], F32, tag="lnstats2")
            for i in range(3):
                nc.vector.bn_stats(out=stats2[:, i, :], in_=ln_in2[:, i * 256 : i * 256 + 256])
            mv2 = lnp2.tile([NCL, 2], F32, tag="lnmv2")
            nc.vector.bn_aggr(out=mv2[:], in_=stats2[:])
            sd2 = lnp2.tile([NCL, 1], F32, tag="lnsd2")
            nc.scalar.activation(out=sd2[:], in_=mv2[:, 1:2], func=AF.Sqrt, bias=eps_t[:, :1], scale=1.0)
            rstd2 = p2.tile([NCL, 1], F32, tag="lnrstd2")
            nc.vector.reciprocal(out=rstd2[:], in_=sd2[:])
        wf_stack.close()

        # decoder + KL
        outT = p2.tile([128, HC, NCL], F32, tag="outT")
        with tc.tile_pool(name="otps", bufs=1, space="PSUM") as otps:
            psO = otps.tile([128, HC * NCL], F32, tag="psO")
            for c in range(HC):
                nc.tensor.transpose(out=psO[:, c * NCL : c * NCL + NCL],
                                    in_=ln_in2[:, c * 128 : c * 128 + 128],
                                    identity=ident[:64, :64])
            nc.vector.tensor_copy(out=outT[:].rearrange("p a b -> p (a b)"), in_=psO[:])

        pair = p2.tile([NCL, 2], F32, tag="pair")
        fin_sb = p2.tile([2, 1], F32, tag="fin")
        with tc.tile_pool(name="klps", bufs=1, space="PSUM") as klps, \
             tc.tile_pool(name="klsc", bufs=1) as klsc:
            pd = klps.tile([NCL, NL], F32, tag="pred")
            for c in range(HC):
                nc.tensor.matmul(out=pd[:], lhsT=outT[:, c, :], rhs=dw[:, c, :],
                                 start=(c == 0), stop=(c == HC - 1))
            predm = klsc.tile([NCL, NL], F32, tag="predm")
            nc.scalar.mul(out=predm[:], in_=pd[:], mul=rstd2[:, :1])
            pred = klsc.tile([NCL, NL], F32, tag="pred_sb")
            nc.vector.tensor_tensor(out=pred[:], in0=predm[:], in1=decb_bc[:], op=ALU.add)
            if DEBUG:
                nc.sync.dma_start(out=dbg["pred"][:, :], in_=pred[:])
            # KL with sum_l(t)=1: kl = sum_l t*(ln t - pred) - negm + ln(ssum)
            negm = klsc.tile([NCL, 1], F32, tag="negm")
            nc.vector.tensor_reduce(out=negm[:], in_=pred[:], axis=AX.X, op=ALU.max, negate=True)
            esc = klsc.tile([NCL, NL], F32, tag="esc")
            ssum = klsc.tile([NCL, 1], F32, tag="ssum")
            nc.scalar.activation(out=esc[:], in_=pred[:], func=AF.Exp,
                                 bias=negm[:, :1], scale=1.0, accum_out=ssum[:, :1])
            lnS = klsc.tile([NCL, 1], F32, tag="lnS")
            nc.scalar.activation(out=lnS[:], in_=ssum[:], func=AF.Ln)
            lnt = klsc.tile([NCL, NL], F32, tag="lnt")
            nc.scalar.activation(out=lnt[:], in_=tgt_sb[:], func=AF.Ln)
            a1 = klsc.tile([NCL, NL], F32, tag="a1")
            nc.vector.tensor_tensor(out=a1[:], in0=lnt[:], in1=pred[:], op=ALU.subtract)
            nc.vector.tensor_tensor(out=a1[:], in0=a1[:], in1=tgt_sb[:], op=ALU.mult)
            kl = klsc.tile([NCL, 1], F32, tag="kl")
            nc.vector.reduce_sum(out=kl[:], in_=a1[:], axis=AX.X)
            nc.vector.tensor_tensor(out=kl[:], in0=kl[:], in1=negm[:], op=ALU.subtract)
            nc.vector.tensor_tensor(out=kl[:], in0=kl[:], in1=lnS[:], op=ALU.add)
            nc.vector.tensor_tensor(out=pair[:, 0:1], in0=kl[:], in1=cnm_pp[:], op=ALU.mult)
            nc.vector.tensor_copy(out=pair[:, 1:2], in_=cnm_pp[:])
            pf = klps.tile([2, 1], F32, tag="fin_ps")
            nc.tensor.matmul(out=pf[:], lhsT=pair[:], rhs=ones_t[:], start=True, stop=True)
            nc.vector.tensor_copy(out=fin_sb[:], in_=pf[:])
            nc.sync.dma_start(out=out_d[:, None], in_=fin_sb[:])


_CACHE = {}


def _get_program():
    if "nc" not in _CACHE:
        nc, nfix = build_program()
        _CACHE["nc"] = nc
    return _CACHE["nc"]


def shard_inputs(inputs):
    import ml_dtypes
    bf16 = ml_dtypes.bfloat16
    enc = np.ascontiguousarray(inputs["encoder_hs"], dtype=np.float32)
    wr = np.asarray(inputs["word_recovery"], dtype=np.int32)
    wm = np.asarray(inputs["word_recovery_mask"], dtype=np.int32)
    cn = np.asarray(inputs["clause_num_mask"], dtype=np.int32)
    adj = np.ascontiguousarray(inputs["adj_matrix"], dtype=np.float32)
    tl = np.ascontiguousarray(inputs["target_labels"], dtype=np.float32)

    f32w = lambda k: np.asarray(inputs[k], dtype=np.float32)
    bf16w = lambda k: np.ascontiguousarray(f32w(k).astype(bf16))

    # shared (identical across cores) weight arrays. All matmul weights are
    # host-packed into their SBUF layout [128, chunks*cols] so each DMA is
    # 128 contiguous descriptors. LayerNorm affine params are folded into
    # the consumers: ln1_g/ln1_b into int_w/int_b, ln2_g/ln2_b + dec_b into
    # the decoder (gdec2 / bdec).
    def pack128(a):
        # [C*128, X] -> [128, C*X]
        C = a.shape[0] // 128
        return np.ascontiguousarray(a.reshape(C, 128, a.shape[1]).transpose(1, 0, 2).reshape(128, -1))

    shared = {}
    for k in ("gc1_w", "gc2_w", "proj_w", "q_w", "k_w", "v_w", "out_w"):
        shared[k] = pack128(bf16w(k))
    ao96 = f32w("ao_w").reshape(NH, DH, H).transpose(1, 0, 2).reshape(DH, NH * H)
    shared["ao_w"] = np.ascontiguousarray(ao96.astype(bf16))
    int_w2 = f32w("ln1_g")[:, None] * f32w("int_w")
    shared["int_w"] = pack128(np.ascontiguousarray(int_w2.astype(bf16)))
    intb2 = f32w("ln1_b") @ f32w("int_w") + f32w("int_b")
    gdec = f32w("ln2_g")[:, None] * f32w("dec_w")
    gdec2 = gdec - gdec.sum(0, keepdims=True) / H
    shared["dec_w"] = pack128(np.ascontiguousarray(gdec2.astype(np.float32)))
    bdec = f32w("ln2_b") @ f32w("dec_w") + f32w("dec_b")
    bias_pk = np.concatenate([f32w("gc1_b"), f32w("gc2_b"), f32w("proj_b"),
                              intb2]).reshape(42, 128).T
    shared["bias_pk"] = np.ascontiguousarray(bias_pk.astype(np.float32))
    qkb = np.concatenate([f32w("q_b"), f32w("k_b")]).reshape(16, DH).T
    shared["qkb_pk"] = np.ascontiguousarray(qkb)
    bcast = np.concatenate([f32w("ao_b"), f32w("v_b"), f32w("out_b"),
                            f32w("ln1_g"), f32w("ln1_b"), bdec])
    shared["bcast_pk"] = np.ascontiguousarray(bcast.astype(bf16))

    in_maps = []
    boff = (np.arange(BB) * S).astype(np.int32)[:, None, None]
    for i in range(NCORES):
        sl = slice(BB * i, BB * i + BB)
        cnm_i = cn[sl].astype(np.float32).reshape(NCL)
        am = np.zeros((NCL, NCL), dtype=np.float32)
        for b in range(BB):
            blk = (1.0 - cnm_i[b * M : (b + 1) * M]) * -10000.0
            am[b * M : (b + 1) * M, b * M : (b + 1) * M] = blk[None, :]
        wrm_i = wm[sl].astype(np.float32).reshape(NROW)
        lens_i = wrm_i.reshape(NCL, LC).sum(-1)
        lens_r = 1.0 / np.maximum(lens_i, 1.0)
        # block-diag wrm*lens_r for PE avg-pool: wseg[p, r, j] nonzero only
        # when row 128r+p belongs to clause 4r+j (j = p//32)
        wseg = np.zeros((128, RT, 4), dtype=np.float32)
        pidx = np.arange(128)
        for r in range(RT):
            rows = 128 * r + pidx
            j = pidx // 32
            wseg[pidx, r, j] = wrm_i[rows] * lens_r[rows // LC]
        adj_i = adj[sl].reshape(NCL, LC, LC)
        adjt = np.zeros((128, RT, 128), dtype=np.float32)
        for b in range(NCL):
            c, t = b % 4, b // 4
            adjt[32 * c : 32 * c + 32, t, 32 * c : 32 * c + 32] = adj_i[b].T
        enc_i = enc[sl].reshape(BB * S, H)
        d = dict(
            enc=np.ascontiguousarray(enc_i),
            gidx=np.ascontiguousarray((wr[sl] + boff).reshape(NROW).reshape(RT, 128).T),
            **{f"enc_s{i}": np.ascontiguousarray(enc_i[:, 192 * i : 192 * i + 192])
               for i in range(4)},
            wrm=np.ascontiguousarray(wrm_i.reshape(RT, 128).T),
            wrm_b=np.ascontiguousarray(wrm_i.astype(bf16)),
            lens=np.ascontiguousarray(lens_r),
            wseg=np.ascontiguousarray(wseg.astype(bf16).reshape(128, RT * 4)),
            cnm=np.ascontiguousarray(cnm_i),
            amask=np.ascontiguousarray(np.tile(am, (1, NH)).astype(bf16)),
            adjt=np.ascontiguousarray(adjt.astype(bf16)),
            tgt=np.ascontiguousarray(tl[sl].reshape(NCL, NL)),
        )
        d.update(shared)
        in_maps.append(d)
    return in_maps


def run_spmd(inputs, trace=False):
    nc = _get_program()
    in_maps = shard_inputs(inputs)
    kw = {}
    if trace:
        import types
        from trn_agent_boot.trn_boot import _ntff_profile_via_ctypes
        mod = types.ModuleType("antenv.axon_hooks")
        hook = _ntff_profile_via_ctypes("/opt/axon/libaxon_pjrt.so")
        mod.get_axon_ntff_profile_hook = lambda: hook
        mod.set_axon_ntff_profile_hook = lambda h: None
        sys.modules["antenv.axon_hooks"] = mod
        bass_utils.upload_artifacts = lambda tmpdir: "local://" + tmpdir
        kw["trace"] = True
    res = bass_utils.run_bass_kernel_spmd(nc, in_maps, core_ids=list(range(NCORES)), **kw)
    return res


def kernel(**inputs):
    res = run_spmd(inputs)
    num = 0.0
    den = 0.0
    for i in range(NCORES):
        o = res.results[i]["out"]
        num += float(o[0])
        den += float(o[1])
    loss = (num / NL) / den
    return np.asarray(loss, dtype=np.float32)


# revision 57
# speedup vs baseline: 1.1663x; 1.0220x over previous
"""Trainium2 Bass kernel for nn_MESGM_15857019256842.

Data-parallel over batch: 16 batches -> 8 cores x 2 batches.
Per core: gather clause tokens (indirect DMA), 2 GCN layers, max/avg pooling,
projection, 8-head self-attention over 32 clauses/batch, FFN, label decoder,
soft-label KL loss. Each core emits (sum kl*mask, sum mask); host combines.

v2: trace-driven rewrite of the baseline.
 - host packs block-diag transposed adjacency + bias vectors (layout prep)
 - one big 3D DMA per weight matrix, issued early and spread across queues
 - bf16 PE transposes for the gathered tokens
 - pooling done per 512-col block, split across DVE and GpSimd, overlapped
   with GCN matmuls; no H2T materialization (pool consumes z2 chunks)
 - z2 runs c-outer so each pooled chunk immediately feeds the projection
 - attention: full-row exp (no b-split/memset), transposed-ctx path that
   feeds ao directly, FFN produces inter transposed (no 24 transposes)
"""
import sys
sys.path.insert(0, '/opt/trn_rl_repo')
import numpy as np

from concourse import bass, mybir, tile
from concourse import bass_utils
from concourse.masks import make_identity
from concourse.vector_clock import ScopedClock

F32 = mybir.dt.float32
BF16 = mybir.dt.bfloat16
I32 = mybir.dt.int32
AF = mybir.ActivationFunctionType
AX = mybir.AxisListType
ALU = mybir.AluOpType

B, S, H, M, LC, NL, II, NH, DH = 16, 512, 768, 32, 32, 7, 3072, 8, 96
NCORES = 8
BB = B // NCORES          # 2 batches per core
NCL = BB * M              # 64 clauses per core
NROW = NCL * LC           # 2048 clause-token rows per core
RT = NROW // 128          # 16 row tiles
HC = H // 128             # 6 H chunks
IC = II // 128            # 24 intermediate chunks
LN_EPS = 1e-12
SQD = float(np.sqrt(DH))
NBC = 5 * H + NL          # broadcast-packed bias columns

_MAX_WAITS = 1


def _patched_drain_and_barrier(self, tick_clock, wait_clock):
    nc = self.nc
    drain_inst = nc.sync.drain()
    wait_clock.add_sem_waits(
        drain_inst.ins, ScopedClock({None: tick_clock.global_clock})
    )
    si = drain_inst.ins.sync_info
    waits = list(si.on_wait or [])
    if len(waits) > _MAX_WAITS:
        si.on_wait = waits[:_MAX_WAITS]
        rest = waits[_MAX_WAITS:]
        for i in range(0, len(rest), _MAX_WAITS):
            nop = nc.sync.nop(nofuse=True)
            nop.ins.sync_info = mybir.SyncInfo(
                on_wait=rest[i : i + _MAX_WAITS], on_update=[]
            )
    nc.all_engine_barrier()
    assert self.sems is not None
    popped = nc._tile_sem_poison_stack.pop()
    assert popped is self._sem_poison
    nc.clear_and_free_semaphores(list(self.sems.allocated().values()))
    nc.all_engine_barrier()


tile.TileContext._drain_and_barrier = _patched_drain_and_barrier


def legalize_waits(nc, limit=1):
    """TRN2 instructions carry at most one sem wait; hoist extras onto nops."""
    nfix = 0
    for blk in nc.main_func.blocks:
        insts = list(blk.instructions)
        pos = 0
        for inst in insts:
            si = inst.sync_info
            waits = list(si.on_wait) if si is not None and si.on_wait else []
            if len(waits) > limit:
                si.on_wait = waits[-limit:]
                rest = waits[:-limit]
                eng = nc.engines[inst.engine]
                for j in range(0, len(rest), limit):
                    nop = eng.nop(nofuse=True)
                    nop.ins.sync_info = mybir.SyncInfo(
                        on_wait=rest[j : j + limit], on_update=[]
                    )
                    src_blk = nc.cur_bb.bb
                    popped = src_blk.instructions.pop()
                    assert popped.name == nop.ins.name
                    blk.instructions.insert(pos, nop.ins)
                    pos += 1
                nfix += 1
            pos += 1
    return nfix


DEBUG = False


def build_program():
    nc = bass.Bass(trn_type="TRN2")

    # ---- DRAM I/O --------------------------------------------------------
    enc = nc.dram_tensor("enc", [BB * S, H], F32, kind="ExternalInput")
    enc_s = [nc.dram_tensor(f"enc_s{i}", [BB * S, 192], F32, kind="ExternalInput")
             for i in range(4)]
    gidx = nc.dram_tensor("gidx", [128, RT], I32, kind="ExternalInput")
    wrm = nc.dram_tensor("wrm", [128, RT], F32, kind="ExternalInput")
    wrm_b = nc.dram_tensor("wrm_b", [NROW], BF16, kind="ExternalInput")
    lens = nc.dram_tensor("lens", [NCL], F32, kind="ExternalInput")
    wseg = nc.dram_tensor("wseg", [128, RT * 4], BF16, kind="ExternalInput")
    cnm = nc.dram_tensor("cnm", [NCL], F32, kind="ExternalInput")
    amask = nc.dram_tensor("amask", [NCL, NH * NCL], BF16, kind="ExternalInput")
    adjt = nc.dram_tensor("adjt", [128, RT, 128], BF16, kind="ExternalInput")
    tgt = nc.dram_tensor("tgt", [NCL, NL], F32, kind="ExternalInput")
    bias_pk = nc.dram_tensor("bias_pk", [128, 42], F32, kind="ExternalInput")
    qkb_pk = nc.dram_tensor("qkb_pk", [DH, 16], F32, kind="ExternalInput")
    bcast_pk = nc.dram_tensor("bcast_pk", [NBC], BF16, kind="ExternalInput")

    # all weights arrive host-packed in their SBUF layout: [128, chunks*cols]
    w = {}
    for name, shp, dt in [
        ("gc1_w", [128, HC * H], BF16), ("gc2_w", [128, HC * H], BF16),
        ("proj_w", [128, IC * H], BF16),
        ("q_w", [128, HC * H], BF16), ("k_w", [128, HC * H], BF16),
        ("v_w", [128, HC * H], BF16),
        ("ao_w", [DH, NH * H], BF16),
        ("int_w", [128, HC * II], BF16), ("out_w", [128, IC * H], BF16),
        ("dec_w", [128, HC * NL], F32),
    ]:
        w[name] = nc.dram_tensor(name, shp, dt, kind="ExternalInput")

    out_d = nc.dram_tensor("out", [2], F32, kind="ExternalOutput")
    dbg = {}
    if DEBUG:
        dbg["cv"] = nc.dram_tensor("dbg_cv", [NCL, H], F32, kind="ExternalOutput")
        dbg["attn"] = nc.dram_tensor("dbg_attn", [NCL, H], F32, kind="ExternalOutput")
        dbg["pred"] = nc.dram_tensor("dbg_pred", [NCL, NL], F32, kind="ExternalOutput")
        dbg["pool"] = nc.dram_tensor("dbg_pool", [128, 24, NCL], F32, kind="ExternalOutput")

    with tile.TileContext(nc) as tc:
        _body(nc, tc, enc, enc_s, gidx, wrm, wrm_b, lens, wseg, cnm, amask, adjt, tgt,
              bias_pk, qkb_pk, bcast_pk, w, out_d, dbg)

    nfix = legalize_waits(nc)
    return nc, nfix


def _body(nc, tc, enc, enc_s, gidx, wrm, wrm_b, lens, wseg, cnm, amask, adjt, tgt,
          bias_pk, qkb_pk, bcast_pk, w, out_d, dbg):
    from contextlib import ExitStack
    ctx = ExitStack()
    with ctx:
        # pool stack (LIFO). Pools reserve their full footprint at open, so
        # big pools open only for their live window:
        #   pp -> wat -> pjps -> wgcn -> yn16 -> pjwp -> ph1 -> pxm
        #   closes: pxm (GCN1 end), ph1 (y2 end), pjwp/yn16/wgcn (z2 end),
        #   pjps (after cvT), then p2 opens for the attention phase.
        pp = ctx.enter_context(tc.tile_pool(name="persist", bufs=1))

        # identities
        ident = pp.tile([128, 128], F32, tag="ident")
        make_identity(nc, ident[:])
        ident_b = pp.tile([128, 128], BF16, tag="identb")
        nc.vector.tensor_copy(out=ident_b[:], in_=ident[:])
        tprime = pp.tile([1, 1], F32, tag="tprime")
        nc.scalar.copy(out=tprime[:], in_=ident[0:1, 0:1])  # prime ACT table

        # --- input / small-tensor DMAs (spread across queues) -------------
        gidx_t = pp.tile([128, RT], I32, tag="gidx")
        nc.sync.dma_start(out=gidx_t[:], in_=bass.AP(tensor=gidx, offset=0, ap=[[RT, 128], [1, RT]]))
        wrm_pp = pp.tile([128, RT], F32, tag="wrmpp")
        nc.sync.dma_start(out=wrm_pp[:], in_=bass.AP(tensor=wrm, offset=0, ap=[[RT, 128], [1, RT]]))
        wseg_sb = pp.tile([128, RT, 4], BF16, tag="wseg")
        nc.sync.dma_start(out=wseg_sb[:].rearrange("p a b -> p (a b)"), in_=bass.AP(tensor=wseg, offset=0, ap=[[RT * 4, 128], [1, RT * 4]]))

        # phase-2 matmul weights + cvT (live to the end; reserved at open)
        wat = ctx.enter_context(tc.tile_pool(name="wat", bufs=1))

        # gcn weights + adjacency (space frees after z2)
        wgcn_stack = ExitStack()
        wg = wgcn_stack.enter_context(tc.tile_pool(name="wgcn", bufs=1))
        gc1w = wg.tile([128, HC, H], BF16, tag="gc1w")
        nc.sync.dma_start(out=gc1w[:].rearrange("p a b -> p (a b)"), in_=bass.AP(tensor=w["gc1_w"], offset=0, ap=[[HC * H, 128], [1, HC * H]]))
        adjt_sb = wg.tile([128, RT, 128], BF16, tag="adjt")
        nc.sync.dma_start(out=adjt_sb[:].rearrange("p a b -> p (a b)"), in_=bass.AP(tensor=adjt, offset=0, ap=[[RT * 128, 128], [1, RT * 128]]))
        gc2w = wg.tile([128, HC, H], BF16, tag="gc2w")
        nc.sync.dma_start(out=gc2w[:].rearrange("p a b -> p (a b)"), in_=bass.AP(tensor=w["gc2_w"], offset=0, ap=[[HC * H, 128], [1, HC * H]]))
        bcast_all = pp.tile([NCL, NBC], BF16, tag="bcall")
        nc.sync.dma_start(out=bcast_all[:], in_=bass.AP(tensor=bcast_pk, offset=0, ap=[[0, NCL], [1, NBC]]))
        amask8 = pp.tile([NCL, NH, NCL], BF16, tag="amask8")
        nc.sync.dma_start(out=amask8[:].rearrange("p a b -> p (a b)"), in_=bass.AP(tensor=amask, offset=0, ap=[[NH * NCL, NCL], [1, NH * NCL]]))
        wrm_bcb = pp.tile([128, NROW], BF16, tag="wrmbcb")
        nc.sync.dma_start(out=wrm_bcb[:], in_=bass.AP(tensor=wrm_b, offset=0, ap=[[0, 128], [1, NROW]]))
        # small tensors on the sync queue, priority order; scalar queue is
        # kept free for the gather masks
        bias_sb = pp.tile([128, 42], F32, tag="biaspk")
        nc.sync.dma_start(out=bias_sb[:], in_=bass.AP(tensor=bias_pk, offset=0, ap=[[42, 128], [1, 42]]))
        qkb_sb = pp.tile([DH, 16], F32, tag="qkb")
        nc.sync.dma_start(out=qkb_sb[:], in_=bass.AP(tensor=qkb_pk, offset=0, ap=[[16, DH], [1, 16]]))
        cnm_pp = pp.tile([NCL, 1], F32, tag="cnmpp")
        nc.sync.dma_start(out=cnm_pp[:], in_=cnm[:, None])
        lens_bc = pp.tile([128, NCL], F32, tag="lensbc")
        nc.sync.dma_start(out=lens_bc[:], in_=bass.AP(tensor=lens, offset=0, ap=[[0, 128], [1, NCL]]))
        tgt_sb = pp.tile([NCL, NL], F32, tag="tgtsb")
        nc.sync.dma_start(out=tgt_sb[:], in_=tgt[:, :])
        eps_t = pp.tile([NCL, 1], F32, tag="epst")
        nc.vector.memset(eps_t[:], LN_EPS)
        ones_t = pp.tile([NCL, 1], F32, tag="onest")
        nc.vector.memset(ones_t[:], 1.0)

        # bias_pk columns: gc1_b 0:6, gc2_b 6:12, proj_b 12:18, int_b 18:42
        gb1 = bias_sb[:, 0:HC]
        gb2 = bias_sb[:, HC : 2 * HC]
        projb = bias_sb[:, 2 * HC : 3 * HC]

        PT = pp.tile([128, 24, NCL], BF16, tag="PT")

        # attention weights early: wat space is reserved from open, so these
        # carry no space-reuse waits and transfer during the gather
        qw = wat.tile([128, HC, H], BF16, tag="qw")
        nc.sync.dma_start(out=qw[:].rearrange("p a b -> p (a b)"), in_=bass.AP(tensor=w["q_w"], offset=0, ap=[[HC * H, 128], [1, HC * H]]))
        kw = wat.tile([128, HC, H], BF16, tag="kw")
        nc.sync.dma_start(out=kw[:].rearrange("p a b -> p (a b)"), in_=bass.AP(tensor=w["k_w"], offset=0, ap=[[HC * H, 128], [1, HC * H]]))
        dw = wat.tile([128, HC, NL], F32, tag="dw")
        nc.sync.dma_start(out=dw[:].rearrange("p a b -> p (a b)"), in_=bass.AP(tensor=w["dec_w"], offset=0, ap=[[HC * NL, 128], [1, HC * NL]]))

        # big transposed activation tiles + rotating proj_w quarters
        pjwp_stack = ExitStack()
        pjwp = pjwp_stack.enter_context(tc.tile_pool(name="pjwp", bufs=3))
        ph1_stack = ExitStack()
        ph1 = ph1_stack.enter_context(tc.tile_pool(name="ph1", bufs=1))
        H1T = ph1.tile([128, HC, NROW], BF16, tag="H1T")
        pxm_stack = ExitStack()
        pxm = pxm_stack.enter_context(tc.tile_pool(name="pxm", bufs=1))
        XmT = pxm.tile([128, HC, NROW], BF16, tag="XmT")

        # =================== phase 1: gather + transpose + x-pool =========
        def y_block(XT, wt, g, tag, ypool, psum_pool, evac="mixed"):
            yns = []
            for rr in range(4):
                r = 4 * g + rr
                p1 = psum_pool.tile([128, 512], F32, tag="y1", name=f"y1_{tag}{r}")
                p2 = psum_pool.tile([128, 256], F32, tag="y2", name=f"y2_{tag}{r}")
                for c in range(HC):
                    lhs = XT[:, c, r * 128 : r * 128 + 128]
                    nc.tensor.matmul(out=p1[:], lhsT=lhs, rhs=wt[:, c, 0:512],
                                     start=(c == 0), stop=(c == HC - 1))
                    nc.tensor.matmul(out=p2[:], lhsT=lhs, rhs=wt[:, c, 512:768],
                                     start=(c == 0), stop=(c == HC - 1))
                yr = ypool.tile([128, H], BF16, tag=f"yn{rr}", name=f"yn_{tag}{r}")
                if evac == "scalar":
                    nc.scalar.copy(out=yr[:, 0:512], in_=p1[:])
                else:
                    nc.vector.tensor_copy(out=yr[:, 0:512], in_=p1[:])
                nc.scalar.copy(out=yr[:, 512:768], in_=p2[:])
                yns.append(yr)
            return yns

        with tc.tile_pool(name="xg", bufs=2) as xgp, \
             tc.tile_pool(name="xb", bufs=2) as xbp, \
             tc.tile_pool(name="xps", bufs=1, space="PSUM") as xps, \
             tc.tile_pool(name="tps", bufs=2, space="PSUM") as tps, \
             tc.tile_pool(name="ynat", bufs=2) as gcn_yn, \
             tc.tile_pool(name="gps", bufs=2, space="PSUM") as gps, \
             tc.tile_pool(name="zps", bufs=1, space="PSUM") as zps:

            def z1_block(g, yns):
                for c in range(HC):
                    zp = zps.tile([128, 512], F32, tag="z", name=f"z_l1{g}_{c}")
                    for rr in range(4):
                        nc.tensor.matmul(
                            out=zp[:, rr * 128 : rr * 128 + 128],
                            lhsT=yns[rr][:, c * 128 : c * 128 + 128],
                            rhs=adjt_sb[:, 4 * g + rr, :],
                            start=True, stop=True,
                        )
                    nc.scalar.activation(
                        out=H1T[:, c, g * 512 : g * 512 + 512], in_=zp[:],
                        func=AF.Relu, bias=gb1[:, c : c + 1], scale=1.0,
                    )

            prev = None
            for g in range(4):
                xt = xgp.tile([128, 4, H], F32, tag="xg", name=f"xg{g}")
                xb = xbp.tile([128, 4, H], BF16, tag="xb", name=f"xb{g}")
                for rr in range(4):
                    r = 4 * g + rr
                    if r == 0:
                        for piece in range(4):
                            a, b = piece * 192, piece * 192 + 192
                            nc.gpsimd.indirect_dma_start(
                                out=xt[:, rr, a:b], out_offset=None, in_=enc_s[piece][:],
                                in_offset=bass.IndirectOffsetOnAxis(ap=gidx_t[:, r : r + 1], axis=0),
                            )
                            nc.scalar.mul(out=xb[:, rr, a:b], in_=xt[:, rr, a:b],
                                          mul=wrm_pp[:, r : r + 1])
                    else:
                        nc.gpsimd.indirect_dma_start(
                            out=xt[:, rr, :], out_offset=None, in_=enc[:],
                            in_offset=bass.IndirectOffsetOnAxis(ap=gidx_t[:, r : r + 1], axis=0),
                        )
                        nc.scalar.mul(out=xb[:, rr, :], in_=xt[:, rr, :], mul=wrm_pp[:, r : r + 1])
                for c in range(HC):
                    ps = tps.tile([128, 512], BF16, tag="tp", name=f"tp{g}_{c}")
                    for rr in range(4):
                        nc.tensor.transpose(
                            out=ps[:, rr * 128 : rr * 128 + 128],
                            in_=xb[:, rr, c * 128 : c * 128 + 128],
                            identity=ident_b[:],
                        )
                    psx = xps.tile([128, 16], F32, tag="psx", name=f"psx{g}_{c}")
                    for rr in range(4):
                        nc.tensor.matmul(out=psx[:, rr * 4 : rr * 4 + 4],
                                         lhsT=xb[:, rr, c * 128 : c * 128 + 128],
                                         rhs=wseg_sb[:, 4 * g + rr, :],
                                         start=True, stop=True)
                    if (c + g) % 2 == 0:
                        nc.vector.tensor_copy(out=XmT[:, c, g * 512 : g * 512 + 512], in_=ps[:])
                        nc.scalar.copy(out=PT[:, 12 + c, g * 16 : g * 16 + 16], in_=psx[:])
                    else:
                        nc.scalar.copy(out=XmT[:, c, g * 512 : g * 512 + 512], in_=ps[:])
                        nc.vector.tensor_copy(out=PT[:, 12 + c, g * 16 : g * 16 + 16], in_=psx[:])
                    v = XmT[:, c, g * 512 : g * 512 + 512].rearrange("p (n l) -> p n l", l=LC)
                    nc.vector.reduce_max(out=PT[:, c, g * 16 : g * 16 + 16], in_=v, axis=AX.X)
                yns = y_block(XmT, gc1w, g, "l1", gcn_yn, gps)
                if prev is not None:
                    z1_block(prev[0], prev[1])
                prev = (g, yns)
            z1_block(prev[0], prev[1])

        # proj_w quarters 0-2 (sync queue; gpsimd slots stay free for gathers)
        pjq = []
        for q in range(3):
            t = pjwp.tile([128, HC, H], BF16, tag="pjw", name=f"pjw{q}")
            nc.sync.dma_start(out=t[:].rearrange("p a b -> p (a b)"), in_=bass.AP(
                tensor=w["proj_w"], offset=q * HC * H,
                ap=[[IC * H, 128], [1, HC * H]]))
            pjq.append(t)

        pxm_stack.close()

        # projection psum (phase-1 psum pools closed now; spans GCN2..cvT)
        pjps_stack = ExitStack()
        pjps = pjps_stack.enter_context(tc.tile_pool(name="pjps", bufs=1, space="PSUM"))
        pcs = pjps.tile([128, HC, NCL], F32, tag="pj")

        # v / ao weights
        vw = wat.tile([128, HC, H], BF16, tag="vw")
        nc.scalar.dma_start(out=vw[:].rearrange("p a b -> p (a b)"), in_=bass.AP(tensor=w["v_w"], offset=0, ap=[[HC * H, 128], [1, HC * H]]))
        aow = wat.tile([DH, NH, H], BF16, tag="aow")
        nc.scalar.dma_start(out=aow[:].rearrange("p a b -> p (a b)"), in_=bass.AP(tensor=w["ao_w"], offset=0, ap=[[NH * H, DH], [1, NH * H]]))

        korder = list(range(12, 18)) + list(range(0, 6)) + \
            [k for c in range(HC) for k in (6 + c, 18 + c)]

        def proj_chunk(k, ki):
            kq, kr = divmod(k, HC)
            for m in range(HC):
                nc.tensor.matmul(out=pcs[:, m, :], lhsT=pjq[kq][:, kr, m * 128 : m * 128 + 128],
                                 rhs=PT[:, k, :], start=(ki == 0), stop=(ki == 23))
        for ki in range(12):
            proj_chunk(korder[ki], ki)
        t = pjwp.tile([128, HC, H], BF16, tag="pjw", name="pjw3")
        nc.sync.dma_start(out=t[:].rearrange("p a b -> p (a b)"), in_=bass.AP(
            tensor=w["proj_w"], offset=3 * HC * H, ap=[[IC * H, 128], [1, HC * H]]))
        pjq.append(t)

        # layer 2: per group g, y2(g) then z2 blocks (c, g) + pooling;
        # proj h2 chunks fire inside the last group as chunks complete.
        with tc.tile_pool(name="gps2", bufs=2, space="PSUM") as gps2, \
             tc.tile_pool(name="yn16", bufs=2) as yn16, \
             tc.tile_pool(name="h2b", bufs=3) as h2bp, \
             tc.tile_pool(name="h2m", bufs=3) as h2mp, \
             tc.tile_pool(name="xs2", bufs=3) as xs2p, \
             tc.tile_pool(name="z2ps", bufs=3, space="PSUM") as z2ps:
            for g in range(4):
                yg = y_block(H1T, gc2w, g, "l2", yn16, gps2, evac="scalar")
                for c in range(HC):
                    zp = z2ps.tile([128, 512], F32, tag="z2", name=f"z_l2{g}_{c}")
                    for rr in range(4):
                        nc.tensor.matmul(
                            out=zp[:, rr * 128 : rr * 128 + 128],
                            lhsT=yg[rr][:, c * 128 : c * 128 + 128],
                            rhs=adjt_sb[:, 4 * g + rr, :],
                            start=True, stop=True,
                        )
                    hb = h2bp.tile([128, 512], BF16, tag="h2b", name=f"h2b{g}_{c}")
                    nc.scalar.activation(out=hb[:], in_=zp[:], func=AF.Relu,
                                         bias=gb2[:, c : c + 1], scale=1.0)
                    hm = h2mp.tile([128, 512], BF16, tag="h2m", name=f"h2m{g}_{c}")
                    nc.gpsimd.tensor_tensor(out=hm[:], in0=hb[:],
                                            in1=wrm_bcb[:, g * 512 : g * 512 + 512], op=ALU.mult)
                    v = hm[:].rearrange("p (n l) -> p n l", l=LC)
                    nc.vector.reduce_max(out=PT[:, 6 + c, g * 16 : g * 16 + 16], in_=v, axis=AX.X)
                    xs = xs2p.tile([128, 16], F32, tag="xs2", name=f"xs2{g}_{c}")
                    nc.vector.reduce_sum(out=xs[:], in_=v, axis=AX.X)
                    nc.gpsimd.tensor_tensor(
                        out=PT[:, 18 + c, g * 16 : g * 16 + 16], in0=xs[:],
                        in1=lens_bc[:, g * 16 : g * 16 + 16], op=ALU.mult)
                    if g == 3:
                        proj_chunk(6 + c, 12 + 2 * c)
                        proj_chunk(18 + c, 13 + 2 * c)

        ph1_stack.close()

        # cv^T = relu(proj + b) straight out of the proj psum
        cvT = wat.tile([128, HC, NCL], BF16, tag="cvT")
        for m in range(HC):
            nc.scalar.activation(out=cvT[:, m, :], in_=pcs[:, m, :], func=AF.Relu,
                                 bias=projb[:, m : m + 1], scale=1.0)
        pjps_stack.close()
        pjwp_stack.close()
        wgcn_stack.close()
        if DEBUG:
            with tc.tile_pool(name="dbgp", bufs=1) as dp:
                ptf = dp.tile([128, 24, NCL], F32, tag="ptdbg")
                nc.vector.tensor_copy(out=ptf[:], in_=PT[:])
                nc.sync.dma_start(out=dbg["pool"][:, :, :], in_=ptf[:])

        # =================== phase 2: attention + FFN + KL ================
        p2 = ctx.enter_context(tc.tile_pool(name="p2", bufs=1))
        # bcast_pk: ao_b, v_b, out_b, ln1_g, ln1_b, ln2_g, ln2_b, bdec
        aob_bc = bcast_all[:, 0:H]
        vb_bc = bcast_all[:, H : 2 * H]
        outb_bc = bcast_all[:, 2 * H : 3 * H]
        l1g_bc = bcast_all[:, 3 * H : 4 * H]
        l1b_bc = bcast_all[:, 4 * H : 5 * H]
        decb_bc = bcast_all[:, 5 * H : 5 * H + NL]

        # FFN weights stream into space freed by H1T/yn16/pjw
        wf_stack = ExitStack()
        wf = wf_stack.enter_context(tc.tile_pool(name="wf", bufs=1))
        iw = wf.tile([128, HC, II], BF16, tag="iw")
        for half in range(2):
            nc.sync.dma_start(
                out=iw[:, 3 * half : 3 * half + 3, :].rearrange("p a b -> p (a b)"),
                in_=bass.AP(tensor=w["int_w"], offset=half * 3 * II,
                            ap=[[HC * II, 128], [1, 3 * II]]))
        ow = wf.tile([128, IC, H], BF16, tag="ow")
        for qtr in range(4):
            nc.sync.dma_start(
                out=ow[:, 6 * qtr : 6 * qtr + 6, :].rearrange("p a b -> p (a b)"),
                in_=bass.AP(tensor=w["out_w"], offset=qtr * 6 * H,
                            ap=[[IC * H, 128], [1, 6 * H]]))

        QT = p2.tile([DH, NH, NCL], BF16, tag="QT")
        KT = p2.tile([DH, NH, NCL], BF16, tag="KT")
        Vn = p2.tile([NCL, H], BF16, tag="Vn")
        att8 = p2.tile([NCL, NH, NCL], BF16, tag="att8")
        attS = p2.tile([NCL, NH, NCL], BF16, tag="attS")
        sums_t = p2.tile([NCL, NH], F32, tag="sums")
        recip_t = p2.tile([NCL, NH], F32, tag="recip")
        negmax = p2.tile([NCL, NH], F32, tag="negmax")
        cv_pa = p2.tile([NCL, H], F32, tag="cvpa")

        # scores psum prefilled with the attention mask (matmuls accumulate
        # on top); the copy is issued early so it never gates the scores.
        sc_stack = ExitStack()
        scps = sc_stack.enter_context(tc.tile_pool(name="scps", bufs=1, space="PSUM"))
        pss = scps.tile([NCL, NH, NCL], F32, tag="scores")
        nc.vector.tensor_copy(out=pss[:], in_=amask8[:])

        with tc.tile_pool(name="qkps", bufs=2, space="PSUM") as qkps, \
             tc.tile_pool(name="vps", bufs=1, space="PSUM") as vps, \
             tc.tile_pool(name="cvt2", bufs=3, space="PSUM") as cvt2:
            psq = qkps.tile([DH, NH * NCL], F32, tag="qk", name="psq")
            for h in range(NH):
                for c in range(HC):
                    nc.tensor.matmul(out=psq[:, h * NCL : h * NCL + NCL],
                                     lhsT=qw[:, c, h * DH : h * DH + DH],
                                     rhs=cvT[:, c, :], start=(c == 0), stop=(c == HC - 1))
            # Q evacs on DVE: (psq + q_b) / sqrt(dh)
            for h in range(NH):
                nc.vector.tensor_scalar(out=QT[:, h, :], in0=psq[:, h * NCL : h * NCL + NCL],
                                        scalar1=qkb_sb[:, h : h + 1], scalar2=1.0 / SQD,
                                        op0=ALU.add, op1=ALU.mult)
            psk = qkps.tile([DH, NH * NCL], F32, tag="qk", name="psk")
            for h in range(NH):
                for c in range(HC):
                    nc.tensor.matmul(out=psk[:, h * NCL : h * NCL + NCL],
                                     lhsT=kw[:, c, h * DH : h * DH + DH],
                                     rhs=cvT[:, c, :], start=(c == 0), stop=(c == HC - 1))
            for h in range(NH):
                nc.scalar.activation(out=KT[:, h, :], in_=psk[:, h * NCL : h * NCL + NCL],
                                     func=AF.Identity, bias=qkb_sb[:, 8 + h : 9 + h], scale=1.0)

            # scores straight after K; V runs on the PE behind them
            for h in range(NH):
                nc.tensor.matmul(out=pss[:, h, :], lhsT=QT[:, h, :], rhs=KT[:, h, :],
                                 start=False, stop=True, skip_group_check=True)
            pv1 = vps.tile([NCL, 512], F32, tag="v1")
            pv2 = vps.tile([NCL, 256], F32, tag="v2")
            for c in range(HC):
                nc.tensor.matmul(out=pv1[:], lhsT=cvT[:, c, :], rhs=vw[:, c, 0:512],
                                 start=(c == 0), stop=(c == HC - 1))
                nc.tensor.matmul(out=pv2[:], lhsT=cvT[:, c, :], rhs=vw[:, c, 512:768],
                                 start=(c == 0), stop=(c == HC - 1))
            # cv natural + ao_b residual base (PE transposes behind V)
            for c in range(HC):
                ps = cvt2.tile([NCL, 128], BF16, tag="cvn", name=f"cvn{c}")
                nc.tensor.transpose(out=ps[:], in_=cvT[:, c, :], identity=ident_b[:])
                nc.vector.tensor_tensor(out=cv_pa[:, c * 128 : c * 128 + 128], in0=ps[:],
                                        in1=aob_bc[:, c * 128 : c * 128 + 128], op=ALU.add)
            nc.vector.tensor_tensor(out=Vn[:, 0:512], in0=pv1[:], in1=vb_bc[:, 0:512], op=ALU.add)
            nc.vector.tensor_tensor(out=Vn[:, 512:768], in0=pv2[:], in1=vb_bc[:, 512:768], op=ALU.add)

        nc.vector.tensor_reduce(out=negmax[:], in_=pss[:], axis=AX.X,
                                op=ALU.max, negate=True)
        for h in range(NH):
            nc.scalar.activation(
                out=att8[:, h, :], in_=pss[:, h, :], func=AF.Exp,
                bias=negmax[:, h : h + 1], scale=1.0,
            )
        nc.vector.reduce_sum(out=sums_t[:], in_=att8[:], axis=AX.X)
        nc.vector.reciprocal(out=recip_t[:], in_=sums_t[:])
        sc_stack.close()

        # per-head: scale, transpose, ctx^T = V-block @ att^T, ao accum
        ctxT = p2.tile([DH, NH, NCL], BF16, tag="ctxT")
        attn_out = p2.tile([NCL, H], F32, tag="attnout")
        xhat = p2.tile([NCL, H], F32, tag="xhat")
        ln_in = p2.tile([NCL, H], F32, tag="lnin1")
        with tc.tile_pool(name="aops", bufs=1, space="PSUM") as aops:
            pa1 = aops.tile([NCL, 512], F32, tag="ao1")
            pa2 = aops.tile([NCL, 256], F32, tag="ao2")
            with tc.tile_pool(name="ctps", bufs=3, space="PSUM") as ctps, \
                 tc.tile_pool(name="atts", bufs=3) as atts:
                for h in range(NH):
                    nc.scalar.mul(out=attS[:, h, :], in_=att8[:, h, :], mul=recip_t[:, h : h + 1])
                    pst = ctps.tile([NCL, NCL], BF16, tag="attT", name=f"attT{h}")
                    nc.tensor.transpose(out=pst[:], in_=attS[:, h, :], identity=ident_b[:64, :64])
                    asb = atts.tile([NCL, NCL], BF16, tag="attTs", name=f"attTs{h}")
                    nc.vector.tensor_copy(out=asb[:], in_=pst[:])
                    pctx = ctps.tile([DH, NCL], F32, tag="ctx", name=f"ctx{h}")
                    nc.tensor.matmul(out=pctx[:], lhsT=Vn[:, h * DH : h * DH + DH], rhs=asb[:],
                                     start=True, stop=True)
                    if h % 2 == 0:
                        nc.vector.tensor_copy(out=ctxT[:, h, :], in_=pctx[:])
                    else:
                        nc.scalar.copy(out=ctxT[:, h, :], in_=pctx[:])
                for h in range(NH):
                    nc.tensor.matmul(out=pa1[:], lhsT=ctxT[:, h, :], rhs=aow[:, h, 0:512],
                                     start=(h == 0), stop=(h == NH - 1))
                    nc.tensor.matmul(out=pa2[:], lhsT=ctxT[:, h, :], rhs=aow[:, h, 512:768],
                                     start=(h == 0), stop=(h == NH - 1))

            def layer_norm(x_nat, g_bc, b_bc, y_nat, lnp):
                stats = lnp.tile([NCL, 3, 6], F32, tag="lnstats")
                for i in range(3):
                    nc.vector.bn_stats(out=stats[:, i, :], in_=x_nat[:, i * 256 : i * 256 + 256])
                mv = lnp.tile([NCL, 2], F32, tag="lnmv")
                nc.vector.bn_aggr(out=mv[:], in_=stats[:])
                sd = lnp.tile([NCL, 1], F32, tag="lnsd")
                nc.scalar.activation(out=sd[:], in_=mv[:, 1:2], func=AF.Sqrt, bias=eps_t[:, :1], scale=1.0)
                rstd = lnp.tile([NCL, 1], F32, tag="lnrstd")
                nc.vector.reciprocal(out=rstd[:], in_=sd[:])
                xc = lnp.tile([NCL, H], F32, tag="lnxc")
                nc.vector.tensor_scalar(out=xc[:], in0=x_nat[:], scalar1=mv[:, 0:1],
                                        scalar2=rstd[:, :1], op0=ALU.subtract, op1=ALU.mult)
                nc.vector.tensor_tensor(out=xc[:], in0=xc[:], in1=g_bc, op=ALU.mult)
                nc.vector.tensor_tensor(out=y_nat[:], in0=xc[:], in1=b_bc, op=ALU.add)

            with tc.tile_pool(name="ln1p", bufs=1) as lnp:
                nc.vector.tensor_tensor(out=ln_in[:, 0:512], in0=pa1[:], in1=cv_pa[:, 0:512], op=ALU.add)
                nc.vector.tensor_tensor(out=ln_in[:, 512:768], in0=pa2[:], in1=cv_pa[:, 512:768], op=ALU.add)
                # LN1 split: FFN consumes xhat directly (ln1_g/ln1_b are
                # host-folded into int_w/int_b); the residual (xhat*g+b)
                # is computed off the critical path.
                stats = lnp.tile([NCL, 3, 6], F32, tag="lnstats")
                for i in range(3):
                    nc.vector.bn_stats(out=stats[:, i, :], in_=ln_in[:, i * 256 : i * 256 + 256])
                mv = lnp.tile([NCL, 2], F32, tag="lnmv")
                nc.vector.bn_aggr(out=mv[:], in_=stats[:])
                sd = lnp.tile([NCL, 1], F32, tag="lnsd")
                nc.scalar.activation(out=sd[:], in_=mv[:, 1:2], func=AF.Sqrt, bias=eps_t[:, :1], scale=1.0)
                rstd = lnp.tile([NCL, 1], F32, tag="lnrstd")
                nc.vector.reciprocal(out=rstd[:], in_=sd[:])
                nc.vector.tensor_scalar(out=xhat[:], in0=ln_in[:], scalar1=mv[:, 0:1],
                                        scalar2=rstd[:, :1], op0=ALU.subtract, op1=ALU.mult)
        if DEBUG:
            nc.sync.dma_start(out=dbg["attn"][:, :], in_=attn_out[:])

        # aoT = xhat^T (bf16); the residual attn_out = xhat*g1 + b1 runs on
        # DVE in parallel with the FFN matmuls
        aoT = p2.tile([128, HC, NCL], BF16, tag="aoT")
        with tc.tile_pool(name="aotps", bufs=1, space="PSUM") as aotps:
            psA = aotps.tile([128, HC * NCL], F32, tag="psA")
            for c in range(HC):
                nc.tensor.transpose(out=psA[:, c * NCL : c * NCL + NCL],
                                    in_=xhat[:, c * 128 : c * 128 + 128],
                                    identity=ident[:64, :64])
            nc.vector.tensor_copy(out=aoT[:].rearrange("p a b -> p (a b)"), in_=psA[:])
        nc.vector.tensor_tensor(out=attn_out[:], in0=xhat[:], in1=l1g_bc, op=ALU.mult)
        nc.vector.tensor_tensor(out=attn_out[:], in0=attn_out[:], in1=l1b_bc, op=ALU.add)

        # FFN: inter produced transposed, chunk by chunk, feeding out-proj
        ln_in2 = p2.tile([NCL, H], F32, tag="lnin2")
        with tc.tile_pool(name="fips", bufs=6, space="PSUM") as fips, \
             tc.tile_pool(name="fit", bufs=6) as fit, \
             tc.tile_pool(name="fops", bufs=1, space="PSUM") as fops, \
             tc.tile_pool(name="ln2p", bufs=1) as lnp2:
            po1 = fops.tile([NCL, 512], F32, tag="o1")
            po2 = fops.tile([NCL, 256], F32, tag="o2")
            for cc in range(IC):
                ip = fips.tile([128, NCL], F32, tag="fi", name=f"fi{cc}")
                for c in range(HC):
                    nc.tensor.matmul(out=ip[:], lhsT=iw[:, c, cc * 128 : cc * 128 + 128],
                                     rhs=aoT[:, c, :], start=(c == 0), stop=(c == HC - 1))
                it = fit.tile([128, NCL], BF16, tag="it", name=f"it{cc}")
                nc.scalar.activation(out=it[:], in_=ip[:], func=AF.Gelu,
                                     bias=bias_sb[:, 18 + cc : 19 + cc], scale=1.0)
                nc.tensor.matmul(out=po1[:], lhsT=it[:], rhs=ow[:, cc, 0:512],
                                 start=(cc == 0), stop=(cc == IC - 1))
                nc.tensor.matmul(out=po2[:], lhsT=it[:], rhs=ow[:, cc, 512:768],
                                 start=(cc == 0), stop=(cc == IC - 1))
            nc.vector.tensor_tensor(out=ln_in2[:, 0:512], in0=po1[:], in1=attn_out[:, 0:512], op=ALU.add)
            nc.vector.tensor_tensor(out=ln_in2[:, 512:768], in0=po2[:], in1=attn_out[:, 512:768], op=ALU.add)
            nc.vector.tensor_tensor(out=ln_in2[:], in0=ln_in2[:], in1=outb_bc[:], op=ALU.add)
            # LN2 folded into the decoder: pred = rstd*(ln_in2 @ gdec2) + bdec
            # (gdec2 = ln2_g*dec_w - colsum/H host-folded). Only the rstd
            # stats chain remains; the transpose runs in parallel with it.
            stats2 = lnp2.tile([NCL, 3, 6], F32, tag="lnstats2")
            for i in range(3):
                nc.vector.bn_stats(out=stats2[:, i, :], in_=ln_in2[:, i * 256 : i * 256 + 256])
            mv2 = lnp2.tile([NCL, 2], F32, tag="lnmv2")
            nc.vector.bn_aggr(out=mv2[:], in_=stats2[:])
            sd2 = lnp2.tile([NCL, 1], F32, tag="lnsd2")
            nc.scalar.activation(out=sd2[:], in_=mv2[:, 1:2], func=AF.Sqrt, bias=eps_t[:, :1], scale=1.0)
            rstd2 = p2.tile([NCL, 1], F32, tag="lnrstd2")
            nc.vector.reciprocal(out=rstd2[:], in_=sd2[:])
        wf_stack.close()

        # decoder + KL
        outT = p2.tile([128, HC, NCL], F32, tag="outT")
        with tc.tile_pool(name="otps", bufs=1, space="PSUM") as otps:
            psO = otps.tile([128, HC * NCL], F32, tag="psO")
            for c in range(HC):
                nc.tensor.transpose(out=psO[:, c * NCL : c * NCL + NCL],
                                    in_=ln_in2[:, c * 128 : c * 128 + 128],
                                    identity=ident[:64, :64])
            nc.vector.tensor_copy(out=outT[:].rearrange("p a b -> p (a b)"), in_=psO[:])

        pair = p2.tile([NCL, 2], F32, tag="pair")
        fin_sb = p2.tile([2, 1], F32, tag="fin")
        with tc.tile_pool(name="klps", bufs=1, space="PSUM") as klps, \
             tc.tile_pool(name="klsc", bufs=1) as klsc:
            pd = klps.tile([NCL, NL], F32, tag="pred")
            for c in range(HC):
                nc.tensor.matmul(out=pd[:], lhsT=outT[:, c, :], rhs=dw[:, c, :],
                                 start=(c == 0), stop=(c == HC - 1))
            predm = klsc.tile([NCL, NL], F32, tag="predm")
            nc.scalar.mul(out=predm[:], in_=pd[:], mul=rstd2[:, :1])
            pred = klsc.tile([NCL, NL], F32, tag="pred_sb")
            nc.vector.tensor_tensor(out=pred[:], in0=predm[:], in1=decb_bc[:], op=ALU.add)
            if DEBUG:
                nc.sync.dma_start(out=dbg["pred"][:, :], in_=pred[:])
            # KL with sum_l(t)=1: kl = sum_l t*(ln t - pred) - negm + ln(ssum)
            negm = klsc.tile([NCL, 1], F32, tag="negm")
            nc.vector.tensor_reduce(out=negm[:], in_=pred[:], axis=AX.X, op=ALU.max, negate=True)
            esc = klsc.tile([NCL, NL], F32, tag="esc")
            ssum = klsc.tile([NCL, 1], F32, tag="ssum")
            nc.scalar.activation(out=esc[:], in_=pred[:], func=AF.Exp,
                                 bias=negm[:, :1], scale=1.0, accum_out=ssum[:, :1])
            lnS = klsc.tile([NCL, 1], F32, tag="lnS")
            nc.scalar.activation(out=lnS[:], in_=ssum[:], func=AF.Ln)
            lnt = klsc.tile([NCL, NL], F32, tag="lnt")
            nc.scalar.activation(out=lnt[:], in_=tgt_sb[:], func=AF.Ln)
            a1 = klsc.tile([NCL, NL], F32, tag="a1")
            nc.vector.tensor_tensor(out=a1[:], in0=lnt[:], in1=pred[:], op=ALU.subtract)
            nc.vector.tensor_tensor(out=a1[:], in0=a1[:], in1=tgt_sb[:], op=ALU.mult)
            kl = klsc.tile([NCL, 1], F32, tag="kl")
            nc.vector.reduce_sum(out=kl[:], in_=a1[:], axis=AX.X)
            nc.vector.tensor_tensor(out=kl[:], in0=kl[:], in1=negm[:], op=ALU.subtract)
            nc.vector.tensor_tensor(out=kl[:], in0=kl[:], in1=lnS[:], op=ALU.add)
            nc.vector.tensor_tensor(out=pair[:, 0:1], in0=kl[:], in1=cnm_pp[:], op=ALU.mult)
            nc.vector.tensor_copy(out=pair[:, 1:2], in_=cnm_pp[:])
            pf = klps.tile([2, 1], F32, tag="fin_ps")
            nc.tensor.matmul(out=pf[:], lhsT=pair[:], rhs=ones_t[:], start=True, stop=True)
            nc.vector.tensor_copy(out=fin_sb[:], in_=pf[:])
            nc.sync.dma_start(out=out_d[:, None], in_=fin_sb[:])


_CACHE = {}


def _get_program():
    if "nc" not in _CACHE:
        nc, nfix = build_program()
        _CACHE["nc"] = nc
    return _CACHE["nc"]


def shard_inputs(inputs):
    import ml_dtypes
    bf16 = ml_dtypes.bfloat16
    enc = np.ascontiguousarray(inputs["encoder_hs"], dtype=np.float32)
    wr = np.asarray(inputs["word_recovery"], dtype=np.int32)
    wm = np.asarray(inputs["word_recovery_mask"], dtype=np.int32)
    cn = np.asarray(inputs["clause_num_mask"], dtype=np.int32)
    adj = np.ascontiguousarray(inputs["adj_matrix"], dtype=np.float32)
    tl = np.ascontiguousarray(inputs["target_labels"], dtype=np.float32)

    f32w = lambda k: np.asarray(inputs[k], dtype=np.float32)
    bf16w = lambda k: np.ascontiguousarray(f32w(k).astype(bf16))

    # shared (identical across cores) weight arrays. All matmul weights are
    # host-packed into their SBUF layout [128, chunks*cols] so each DMA is
    # 128 contiguous descriptors. LayerNorm affine params are folded into
    # the consumers: ln1_g/ln1_b into int_w/int_b, ln2_g/ln2_b + dec_b into
    # the decoder (gdec2 / bdec).
    def pack128(a):
        # [C*128, X] -> [128, C*X]
        C = a.shape[0] // 128
        return np.ascontiguousarray(a.reshape(C, 128, a.shape[1]).transpose(1, 0, 2).reshape(128, -1))

    shared = {}
    for k in ("gc1_w", "gc2_w", "proj_w", "q_w", "k_w", "v_w", "out_w"):
        shared[k] = pack128(bf16w(k))
    ao96 = f32w("ao_w").reshape(NH, DH, H).transpose(1, 0, 2).reshape(DH, NH * H)
    shared["ao_w"] = np.ascontiguousarray(ao96.astype(bf16))
    int_w2 = f32w("ln1_g")[:, None] * f32w("int_w")
    shared["int_w"] = pack128(np.ascontiguousarray(int_w2.astype(bf16)))
    intb2 = f32w("ln1_b") @ f32w("int_w") + f32w("int_b")
    gdec = f32w("ln2_g")[:, None] * f32w("dec_w")
    gdec2 = gdec - gdec.sum(0, keepdims=True) / H
    shared["dec_w"] = pack128(np.ascontiguousarray(gdec2.astype(np.float32)))
    bdec = f32w("ln2_b") @ f32w("dec_w") + f32w("dec_b")
    bias_pk = np.concatenate([f32w("gc1_b"), f32w("gc2_b"), f32w("proj_b"),
                              intb2]).reshape(42, 128).T
    shared["bias_pk"] = np.ascontiguousarray(bias_pk.astype(np.float32))
    qkb = np.concatenate([f32w("q_b"), f32w("k_b")]).reshape(16, DH).T
    shared["qkb_pk"] = np.ascontiguousarray(qkb)
    bcast = np.concatenate([f32w("ao_b"), f32w("v_b"), f32w("out_b"),
                            f32w("ln1_g"), f32w("ln1_b"), bdec])
    shared["bcast_pk"] = np.ascontiguousarray(bcast.astype(bf16))

    in_maps = []
    boff = (np.arange(BB) * S).astype(np.int32)[:, None, None]
    for i in range(NCORES):
        sl = slice(BB * i, BB * i + BB)
        cnm_i = cn[sl].astype(np.float32).reshape(NCL)
        am = np.zeros((NCL, NCL), dtype=np.float32)
        for b in range(BB):
            blk = (1.0 - cnm_i[b * M : (b + 1) * M]) * -10000.0
            am[b * M : (b + 1) * M, b * M : (b + 1) * M] = blk[None, :]
        wrm_i = wm[sl].astype(np.float32).reshape(NROW)
        lens_i = wrm_i.reshape(NCL, LC).sum(-1)
        lens_r = 1.0 / np.maximum(lens_i, 1.0)
        # block-diag wrm*lens_r for PE avg-pool: wseg[p, r, j] nonzero only
        # when row 128r+p belongs to clause 4r+j (j = p//32)
        wseg = np.zeros((128, RT, 4), dtype=np.float32)
        pidx = np.arange(128)
        for r in range(RT):
            rows = 128 * r + pidx
            j = pidx // 32
            wseg[pidx, r, j] = wrm_i[rows] * lens_r[rows // LC]
        adj_i = adj[sl].reshape(NCL, LC, LC)
        adjt = np.zeros((128, RT, 128), dtype=np.float32)
        for b in range(NCL):
            c, t = b % 4, b // 4
            adjt[32 * c : 32 * c + 32, t, 32 * c : 32 * c + 32] = adj_i[b].T
        enc_i = enc[sl].reshape(BB * S, H)
        d = dict(
            enc=np.ascontiguousarray(enc_i),
            gidx=np.ascontiguousarray((wr[sl] + boff).reshape(NROW).reshape(RT, 128).T),
            **{f"enc_s{i}": np.ascontiguousarray(enc_i[:, 192 * i : 192 * i + 192])
               for i in range(4)},
            wrm=np.ascontiguousarray(wrm_i.reshape(RT, 128).T),
            wrm_b=np.ascontiguousarray(wrm_i.astype(bf16)),
            lens=np.ascontiguousarray(lens_r),
            wseg=np.ascontiguousarray(wseg.astype(bf16).reshape(128, RT * 4)),
            cnm=np.ascontiguousarray(cnm_i),
            amask=np.ascontiguousarray(np.tile(am, (1, NH)).astype(bf16)),
            adjt=np.ascontiguousarray(adjt.astype(bf16)),
            tgt=np.ascontiguousarray(tl[sl].reshape(NCL, NL)),
        )
        d.update(shared)
        in_maps.append(d)
    return in_maps


def run_spmd(inputs, trace=False):
    nc = _get_program()
    in_maps = shard_inputs(inputs)
    kw = {}
    if trace:
        import types
        from trn_agent_boot.trn_boot import _ntff_profile_via_ctypes
        mod = types.ModuleType("antenv.axon_hooks")
        hook = _ntff_profile_via_ctypes("/opt/axon/libaxon_pjrt.so")
        mod.get_axon_ntff_profile_hook = lambda: hook
        mod.set_axon_ntff_profile_hook = lambda h: None
        sys.modules["antenv.axon_hooks"] = mod
        bass_utils.upload_artifacts = lambda tmpdir: "local://" + tmpdir
        kw["trace"] = True
    res = bass_utils.run_bass_kernel_spmd(nc, in_maps, core_ids=list(range(NCORES)), **kw)
    return res


def kernel(**inputs):
    res = run_spmd(inputs)
    num = 0.0
    den = 0.0
    for i in range(NCORES):
        o = res.results[i]["out"]
        num += float(o[0])
        den += float(o[1])
    loss = (num / NL) / den
    return np.asarray(loss, dtype=np.float32)


# revision 59
# speedup vs baseline: 1.1686x; 1.0020x over previous
"""Trainium2 Bass kernel for nn_MESGM_15857019256842.

Data-parallel over batch: 16 batches -> 8 cores x 2 batches.
Per core: gather clause tokens (indirect DMA), 2 GCN layers, max/avg pooling,
projection, 8-head self-attention over 32 clauses/batch, FFN, label decoder,
soft-label KL loss. Each core emits (sum kl*mask, sum mask); host combines.

v2: trace-driven rewrite of the baseline.
 - host packs block-diag transposed adjacency + bias vectors (layout prep)
 - one big 3D DMA per weight matrix, issued early and spread across queues
 - bf16 PE transposes for the gathered tokens
 - pooling done per 512-col block, split across DVE and GpSimd, overlapped
   with GCN matmuls; no H2T materialization (pool consumes z2 chunks)
 - z2 runs c-outer so each pooled chunk immediately feeds the projection
 - attention: full-row exp (no b-split/memset), transposed-ctx path that
   feeds ao directly, FFN produces inter transposed (no 24 transposes)
"""
import sys
sys.path.insert(0, '/opt/trn_rl_repo')
import numpy as np

from concourse import bass, mybir, tile
from concourse import bass_utils
from concourse.masks import make_identity
from concourse.vector_clock import ScopedClock

F32 = mybir.dt.float32
BF16 = mybir.dt.bfloat16
I32 = mybir.dt.int32
AF = mybir.ActivationFunctionType
AX = mybir.AxisListType
ALU = mybir.AluOpType

B, S, H, M, LC, NL, II, NH, DH = 16, 512, 768, 32, 32, 7, 3072, 8, 96
NCORES = 8
BB = B // NCORES          # 2 batches per core
NCL = BB * M              # 64 clauses per core
NROW = NCL * LC           # 2048 clause-token rows per core
RT = NROW // 128          # 16 row tiles
HC = H // 128             # 6 H chunks
IC = II // 128            # 24 intermediate chunks
LN_EPS = 1e-12
SQD = float(np.sqrt(DH))
NBC = 5 * H + NL          # broadcast-packed bias columns

_MAX_WAITS = 1


def _patched_drain_and_barrier(self, tick_clock, wait_clock):
    nc = self.nc
    drain_inst = nc.sync.drain()
    wait_clock.add_sem_waits(
        drain_inst.ins, ScopedClock({None: tick_clock.global_clock})
    )
    si = drain_inst.ins.sync_info
    waits = list(si.on_wait or [])
    if len(waits) > _MAX_WAITS:
        si.on_wait = waits[:_MAX_WAITS]
        rest = waits[_MAX_WAITS:]
        for i in range(0, len(rest), _MAX_WAITS):
            nop = nc.sync.nop(nofuse=True)
            nop.ins.sync_info = mybir.SyncInfo(
                on_wait=rest[i : i + _MAX_WAITS], on_update=[]
            )
    nc.all_engine_barrier()
    assert self.sems is not None
    popped = nc._tile_sem_poison_stack.pop()
    assert popped is self._sem_poison
    nc.clear_and_free_semaphores(list(self.sems.allocated().values()))
    nc.all_engine_barrier()


tile.TileContext._drain_and_barrier = _patched_drain_and_barrier


def legalize_waits(nc, limit=1):
    """TRN2 instructions carry at most one sem wait; hoist extras onto nops."""
    nfix = 0
    for blk in nc.main_func.blocks:
        insts = list(blk.instructions)
        pos = 0
        for inst in insts:
            si = inst.sync_info
            waits = list(si.on_wait) if si is not None and si.on_wait else []
            if len(waits) > limit:
                si.on_wait = waits[-limit:]
                rest = waits[:-limit]
                eng = nc.engines[inst.engine]
                for j in range(0, len(rest), limit):
                    nop = eng.nop(nofuse=True)
                    nop.ins.sync_info = mybir.SyncInfo(
                        on_wait=rest[j : j + limit], on_update=[]
                    )
                    src_blk = nc.cur_bb.bb
                    popped = src_blk.instructions.pop()
                    assert popped.name == nop.ins.name
                    blk.instructions.insert(pos, nop.ins)
                    pos += 1
                nfix += 1
            pos += 1
    return nfix


DEBUG = False


def build_program():
    nc = bass.Bass(trn_type="TRN2")

    # ---- DRAM I/O --------------------------------------------------------
    enc = nc.dram_tensor("enc", [BB * S, H], F32, kind="ExternalInput")
    enc_s = [nc.dram_tensor(f"enc_s{i}", [BB * S, 192], F32, kind="ExternalInput")
             for i in range(4)]
    gidx = nc.dram_tensor("gidx", [128, RT], I32, kind="ExternalInput")
    wrm = nc.dram_tensor("wrm", [128, RT], F32, kind="ExternalInput")
    wrm_b = nc.dram_tensor("wrm_b", [NROW], BF16, kind="ExternalInput")
    lens = nc.dram_tensor("lens", [NCL], F32, kind="ExternalInput")
    wseg = nc.dram_tensor("wseg", [128, RT * 4], BF16, kind="ExternalInput")
    cnm = nc.dram_tensor("cnm", [NCL], F32, kind="ExternalInput")
    amask = nc.dram_tensor("amask", [NCL, NH * NCL], BF16, kind="ExternalInput")
    adjt = nc.dram_tensor("adjt", [128, RT, 128], BF16, kind="ExternalInput")
    tgt = nc.dram_tensor("tgt", [NCL, NL], F32, kind="ExternalInput")
    bias_pk = nc.dram_tensor("bias_pk", [128, 42], F32, kind="ExternalInput")
    qkb_pk = nc.dram_tensor("qkb_pk", [DH, 16], F32, kind="ExternalInput")
    bcast_pk = nc.dram_tensor("bcast_pk", [NBC], BF16, kind="ExternalInput")

    # all weights arrive host-packed in their SBUF layout: [128, chunks*cols]
    w = {}
    for name, shp, dt in [
        ("gc1_w", [128, HC * H], BF16), ("gc2_w", [128, HC * H], BF16),
        ("proj_w", [128, IC * H], BF16),
        ("q_w", [128, HC * H], BF16), ("k_w", [128, HC * H], BF16),
        ("v_w", [128, HC * H], BF16),
        ("ao_w", [DH, NH * H], BF16),
        ("int_w", [128, HC * II], BF16), ("out_w", [128, IC * H], BF16),
        ("dec_w", [128, HC * NL], F32),
    ]:
        w[name] = nc.dram_tensor(name, shp, dt, kind="ExternalInput")

    out_d = nc.dram_tensor("out", [2], F32, kind="ExternalOutput")
    dbg = {}
    if DEBUG:
        dbg["cv"] = nc.dram_tensor("dbg_cv", [NCL, H], F32, kind="ExternalOutput")
        dbg["attn"] = nc.dram_tensor("dbg_attn", [NCL, H], F32, kind="ExternalOutput")
        dbg["pred"] = nc.dram_tensor("dbg_pred", [NCL, NL], F32, kind="ExternalOutput")
        dbg["pool"] = nc.dram_tensor("dbg_pool", [128, 24, NCL], F32, kind="ExternalOutput")

    with tile.TileContext(nc) as tc:
        _body(nc, tc, enc, enc_s, gidx, wrm, wrm_b, lens, wseg, cnm, amask, adjt, tgt,
              bias_pk, qkb_pk, bcast_pk, w, out_d, dbg)

    nfix = legalize_waits(nc)
    return nc, nfix


def _body(nc, tc, enc, enc_s, gidx, wrm, wrm_b, lens, wseg, cnm, amask, adjt, tgt,
          bias_pk, qkb_pk, bcast_pk, w, out_d, dbg):
    from contextlib import ExitStack
    ctx = ExitStack()
    with ctx:
        # pool stack (LIFO). Pools reserve their full footprint at open, so
        # big pools open only for their live window:
        #   pp -> wat -> pjps -> wgcn -> yn16 -> pjwp -> ph1 -> pxm
        #   closes: pxm (GCN1 end), ph1 (y2 end), pjwp/yn16/wgcn (z2 end),
        #   pjps (after cvT), then p2 opens for the attention phase.
        pp = ctx.enter_context(tc.tile_pool(name="persist", bufs=1))

        # identities
        ident = pp.tile([128, 128], F32, tag="ident")
        make_identity(nc, ident[:])
        ident_b = pp.tile([128, 128], BF16, tag="identb")
        nc.vector.tensor_copy(out=ident_b[:], in_=ident[:])
        tprime = pp.tile([1, 1], F32, tag="tprime")
        nc.scalar.copy(out=tprime[:], in_=ident[0:1, 0:1])  # prime ACT table

        # --- input / small-tensor DMAs (spread across queues) -------------
        gidx_t = pp.tile([128, RT], I32, tag="gidx")
        nc.sync.dma_start(out=gidx_t[:], in_=bass.AP(tensor=gidx, offset=0, ap=[[RT, 128], [1, RT]]))
        wrm_pp = pp.tile([128, RT], F32, tag="wrmpp")
        nc.sync.dma_start(out=wrm_pp[:], in_=bass.AP(tensor=wrm, offset=0, ap=[[RT, 128], [1, RT]]))
        wseg_sb = pp.tile([128, RT, 4], BF16, tag="wseg")
        nc.sync.dma_start(out=wseg_sb[:].rearrange("p a b -> p (a b)"), in_=bass.AP(tensor=wseg, offset=0, ap=[[RT * 4, 128], [1, RT * 4]]))

        # phase-2 matmul weights + cvT (live to the end; reserved at open)
        wat = ctx.enter_context(tc.tile_pool(name="wat", bufs=1))

        # gcn weights + adjacency (space frees after z2)
        wgcn_stack = ExitStack()
        wg = wgcn_stack.enter_context(tc.tile_pool(name="wgcn", bufs=1))
        gc1w = wg.tile([128, HC, H], BF16, tag="gc1w")
        nc.sync.dma_start(out=gc1w[:].rearrange("p a b -> p (a b)"), in_=bass.AP(tensor=w["gc1_w"], offset=0, ap=[[HC * H, 128], [1, HC * H]]))
        adjt_sb = wg.tile([128, RT, 128], BF16, tag="adjt")
        nc.sync.dma_start(out=adjt_sb[:].rearrange("p a b -> p (a b)"), in_=bass.AP(tensor=adjt, offset=0, ap=[[RT * 128, 128], [1, RT * 128]]))
        gc2w = wg.tile([128, HC, H], BF16, tag="gc2w")
        nc.sync.dma_start(out=gc2w[:].rearrange("p a b -> p (a b)"), in_=bass.AP(tensor=w["gc2_w"], offset=0, ap=[[HC * H, 128], [1, HC * H]]))
        bcast_all = pp.tile([NCL, NBC], BF16, tag="bcall")
        nc.sync.dma_start(out=bcast_all[:], in_=bass.AP(tensor=bcast_pk, offset=0, ap=[[0, NCL], [1, NBC]]))
        amask8 = pp.tile([NCL, NH, NCL], BF16, tag="amask8")
        nc.sync.dma_start(out=amask8[:].rearrange("p a b -> p (a b)"), in_=bass.AP(tensor=amask, offset=0, ap=[[NH * NCL, NCL], [1, NH * NCL]]))
        wrm_bcb = pp.tile([128, NROW], BF16, tag="wrmbcb")
        nc.sync.dma_start(out=wrm_bcb[:], in_=bass.AP(tensor=wrm_b, offset=0, ap=[[0, 128], [1, NROW]]))
        # small tensors on the sync queue, priority order; scalar queue is
        # kept free for the gather masks
        bias_sb = pp.tile([128, 42], F32, tag="biaspk")
        nc.sync.dma_start(out=bias_sb[:], in_=bass.AP(tensor=bias_pk, offset=0, ap=[[42, 128], [1, 42]]))
        qkb_sb = pp.tile([DH, 16], F32, tag="qkb")
        nc.sync.dma_start(out=qkb_sb[:], in_=bass.AP(tensor=qkb_pk, offset=0, ap=[[16, DH], [1, 16]]))
        cnm_pp = pp.tile([NCL, 1], F32, tag="cnmpp")
        nc.sync.dma_start(out=cnm_pp[:], in_=cnm[:, None])
        lens_bc = pp.tile([128, NCL], F32, tag="lensbc")
        nc.sync.dma_start(out=lens_bc[:], in_=bass.AP(tensor=lens, offset=0, ap=[[0, 128], [1, NCL]]))
        tgt_sb = pp.tile([NCL, NL], F32, tag="tgtsb")
        nc.sync.dma_start(out=tgt_sb[:], in_=tgt[:, :])
        eps_t = pp.tile([NCL, 1], F32, tag="epst")
        nc.vector.memset(eps_t[:], LN_EPS)
        ones_t = pp.tile([NCL, 1], F32, tag="onest")
        nc.vector.memset(ones_t[:], 1.0)

        # bias_pk columns: gc1_b 0:6, gc2_b 6:12, proj_b 12:18, int_b 18:42
        gb1 = bias_sb[:, 0:HC]
        gb2 = bias_sb[:, HC : 2 * HC]
        projb = bias_sb[:, 2 * HC : 3 * HC]

        PT = pp.tile([128, 24, NCL], BF16, tag="PT")

        # attention weights early: wat space is reserved from open, so these
        # carry no space-reuse waits and transfer during the gather
        qw = wat.tile([128, HC, H], BF16, tag="qw")
        nc.sync.dma_start(out=qw[:].rearrange("p a b -> p (a b)"), in_=bass.AP(tensor=w["q_w"], offset=0, ap=[[HC * H, 128], [1, HC * H]]))
        kw = wat.tile([128, HC, H], BF16, tag="kw")
        nc.sync.dma_start(out=kw[:].rearrange("p a b -> p (a b)"), in_=bass.AP(tensor=w["k_w"], offset=0, ap=[[HC * H, 128], [1, HC * H]]))
        dw = wat.tile([128, HC, NL], F32, tag="dw")
        nc.sync.dma_start(out=dw[:].rearrange("p a b -> p (a b)"), in_=bass.AP(tensor=w["dec_w"], offset=0, ap=[[HC * NL, 128], [1, HC * NL]]))

        # big transposed activation tiles + rotating proj_w quarters
        pjwp_stack = ExitStack()
        pjwp = pjwp_stack.enter_context(tc.tile_pool(name="pjwp", bufs=3))
        ph1_stack = ExitStack()
        ph1 = ph1_stack.enter_context(tc.tile_pool(name="ph1", bufs=1))
        H1T = ph1.tile([128, HC, NROW], BF16, tag="H1T")
        pxm_stack = ExitStack()
        pxm = pxm_stack.enter_context(tc.tile_pool(name="pxm", bufs=1))
        XmT = pxm.tile([128, HC, NROW], BF16, tag="XmT")

        # =================== phase 1: gather + transpose + x-pool =========
        def y_block(XT, wt, g, tag, ypool, psum_pool, evac="mixed"):
            yns = []
            for rr in range(4):
                r = 4 * g + rr
                p1 = psum_pool.tile([128, 512], F32, tag="y1", name=f"y1_{tag}{r}")
                p2 = psum_pool.tile([128, 256], F32, tag="y2", name=f"y2_{tag}{r}")
                for c in range(HC):
                    lhs = XT[:, c, r * 128 : r * 128 + 128]
                    nc.tensor.matmul(out=p1[:], lhsT=lhs, rhs=wt[:, c, 0:512],
                                     start=(c == 0), stop=(c == HC - 1))
                    nc.tensor.matmul(out=p2[:], lhsT=lhs, rhs=wt[:, c, 512:768],
                                     start=(c == 0), stop=(c == HC - 1))
                yr = ypool.tile([128, H], BF16, tag=f"yn{rr}", name=f"yn_{tag}{r}")
                if evac == "scalar":
                    nc.scalar.copy(out=yr[:, 0:512], in_=p1[:])
                else:
                    nc.vector.tensor_copy(out=yr[:, 0:512], in_=p1[:])
                nc.scalar.copy(out=yr[:, 512:768], in_=p2[:])
                yns.append(yr)
            return yns

        with tc.tile_pool(name="xg", bufs=2) as xgp, \
             tc.tile_pool(name="xb", bufs=2) as xbp, \
             tc.tile_pool(name="xps", bufs=1, space="PSUM") as xps, \
             tc.tile_pool(name="tps", bufs=2, space="PSUM") as tps, \
             tc.tile_pool(name="ynat", bufs=2) as gcn_yn, \
             tc.tile_pool(name="gps", bufs=2, space="PSUM") as gps, \
             tc.tile_pool(name="zps", bufs=1, space="PSUM") as zps:

            def z1_block(g, yns):
                for c in range(HC):
                    zp = zps.tile([128, 512], F32, tag="z", name=f"z_l1{g}_{c}")
                    for rr in range(4):
                        nc.tensor.matmul(
                            out=zp[:, rr * 128 : rr * 128 + 128],
                            lhsT=yns[rr][:, c * 128 : c * 128 + 128],
                            rhs=adjt_sb[:, 4 * g + rr, :],
                            start=True, stop=True,
                        )
                    nc.scalar.activation(
                        out=H1T[:, c, g * 512 : g * 512 + 512], in_=zp[:],
                        func=AF.Relu, bias=gb1[:, c : c + 1], scale=1.0,
                    )

            prev = None
            for g in range(4):
                xt = xgp.tile([128, 4, H], F32, tag="xg", name=f"xg{g}")
                xb = xbp.tile([128, 4, H], BF16, tag="xb", name=f"xb{g}")
                for rr in range(4):
                    r = 4 * g + rr
                    if r == 0:
                        for piece in range(4):
                            a, b = piece * 192, piece * 192 + 192
                            nc.gpsimd.indirect_dma_start(
                                out=xt[:, rr, a:b], out_offset=None, in_=enc_s[piece][:],
                                in_offset=bass.IndirectOffsetOnAxis(ap=gidx_t[:, r : r + 1], axis=0),
                            )
                            nc.scalar.mul(out=xb[:, rr, a:b], in_=xt[:, rr, a:b],
                                          mul=wrm_pp[:, r : r + 1])
                    else:
                        nc.gpsimd.indirect_dma_start(
                            out=xt[:, rr, :], out_offset=None, in_=enc[:],
                            in_offset=bass.IndirectOffsetOnAxis(ap=gidx_t[:, r : r + 1], axis=0),
                        )
                        nc.scalar.mul(out=xb[:, rr, :], in_=xt[:, rr, :], mul=wrm_pp[:, r : r + 1])
                for c in range(HC):
                    ps = tps.tile([128, 512], BF16, tag="tp", name=f"tp{g}_{c}")
                    for rr in range(4):
                        nc.tensor.transpose(
                            out=ps[:, rr * 128 : rr * 128 + 128],
                            in_=xb[:, rr, c * 128 : c * 128 + 128],
                            identity=ident_b[:],
                        )
                    psx = xps.tile([128, 16], F32, tag="psx", name=f"psx{g}_{c}")
                    for rr in range(4):
                        nc.tensor.matmul(out=psx[:, rr * 4 : rr * 4 + 4],
                                         lhsT=xb[:, rr, c * 128 : c * 128 + 128],
                                         rhs=wseg_sb[:, 4 * g + rr, :],
                                         start=True, stop=True)
                    if (c + g) % 2 == 0:
                        nc.vector.tensor_copy(out=XmT[:, c, g * 512 : g * 512 + 512], in_=ps[:])
                        nc.scalar.copy(out=PT[:, 12 + c, g * 16 : g * 16 + 16], in_=psx[:])
                    else:
                        nc.scalar.copy(out=XmT[:, c, g * 512 : g * 512 + 512], in_=ps[:])
                        nc.vector.tensor_copy(out=PT[:, 12 + c, g * 16 : g * 16 + 16], in_=psx[:])
                    v = XmT[:, c, g * 512 : g * 512 + 512].rearrange("p (n l) -> p n l", l=LC)
                    nc.vector.reduce_max(out=PT[:, c, g * 16 : g * 16 + 16], in_=v, axis=AX.X)
                yns = y_block(XmT, gc1w, g, "l1", gcn_yn, gps)
                if prev is not None:
                    z1_block(prev[0], prev[1])
                prev = (g, yns)
            z1_block(prev[0], prev[1])

        # proj_w quarters 0-2 (sync queue; gpsimd slots stay free for gathers)
        pjq = []
        for q in range(3):
            t = pjwp.tile([128, HC, H], BF16, tag="pjw", name=f"pjw{q}")
            nc.sync.dma_start(out=t[:].rearrange("p a b -> p (a b)"), in_=bass.AP(
                tensor=w["proj_w"], offset=q * HC * H,
                ap=[[IC * H, 128], [1, HC * H]]))
            pjq.append(t)

        pxm_stack.close()

        # projection psum (phase-1 psum pools closed now; spans GCN2..cvT)
        pjps_stack = ExitStack()
        pjps = pjps_stack.enter_context(tc.tile_pool(name="pjps", bufs=1, space="PSUM"))
        pcs = pjps.tile([128, HC, NCL], F32, tag="pj")

        # v / ao weights
        vw = wat.tile([128, HC, H], BF16, tag="vw")
        nc.scalar.dma_start(out=vw[:].rearrange("p a b -> p (a b)"), in_=bass.AP(tensor=w["v_w"], offset=0, ap=[[HC * H, 128], [1, HC * H]]))
        aow = wat.tile([DH, NH, H], BF16, tag="aow")
        nc.scalar.dma_start(out=aow[:].rearrange("p a b -> p (a b)"), in_=bass.AP(tensor=w["ao_w"], offset=0, ap=[[NH * H, DH], [1, NH * H]]))

        korder = list(range(12, 18)) + list(range(0, 6)) + \
            [k for c in range(HC) for k in (6 + c, 18 + c)]

        def proj_chunk(k, ki):
            kq, kr = divmod(k, HC)
            for m in range(HC):
                nc.tensor.matmul(out=pcs[:, m, :], lhsT=pjq[kq][:, kr, m * 128 : m * 128 + 128],
                                 rhs=PT[:, k, :], start=(ki == 0), stop=(ki == 23))
        for ki in range(12):
            proj_chunk(korder[ki], ki)
        t = pjwp.tile([128, HC, H], BF16, tag="pjw", name="pjw3")
        nc.sync.dma_start(out=t[:].rearrange("p a b -> p (a b)"), in_=bass.AP(
            tensor=w["proj_w"], offset=3 * HC * H, ap=[[IC * H, 128], [1, HC * H]]))
        pjq.append(t)

        # layer 2: per group g, y2(g) then z2 blocks (c, g) + pooling;
        # proj h2 chunks fire inside the last group as chunks complete.
        with tc.tile_pool(name="gps2", bufs=2, space="PSUM") as gps2, \
             tc.tile_pool(name="yn16", bufs=2) as yn16, \
             tc.tile_pool(name="h2b", bufs=3) as h2bp, \
             tc.tile_pool(name="h2m", bufs=3) as h2mp, \
             tc.tile_pool(name="xs2", bufs=3) as xs2p, \
             tc.tile_pool(name="z2ps", bufs=3, space="PSUM") as z2ps:
            for g in range(4):
                yg = y_block(H1T, gc2w, g, "l2", yn16, gps2, evac="scalar")
                for c in range(HC):
                    zp = z2ps.tile([128, 512], F32, tag="z2", name=f"z_l2{g}_{c}")
                    for rr in range(4):
                        nc.tensor.matmul(
                            out=zp[:, rr * 128 : rr * 128 + 128],
                            lhsT=yg[rr][:, c * 128 : c * 128 + 128],
                            rhs=adjt_sb[:, 4 * g + rr, :],
                            start=True, stop=True,
                        )
                    hb = h2bp.tile([128, 512], BF16, tag="h2b", name=f"h2b{g}_{c}")
                    nc.scalar.activation(out=hb[:], in_=zp[:], func=AF.Relu,
                                         bias=gb2[:, c : c + 1], scale=1.0)
                    hm = h2mp.tile([128, 512], BF16, tag="h2m", name=f"h2m{g}_{c}")
                    nc.gpsimd.tensor_tensor(out=hm[:], in0=hb[:],
                                            in1=wrm_bcb[:, g * 512 : g * 512 + 512], op=ALU.mult)
                    v = hm[:].rearrange("p (n l) -> p n l", l=LC)
                    nc.vector.reduce_max(out=PT[:, 6 + c, g * 16 : g * 16 + 16], in_=v, axis=AX.X)
                    xs = xs2p.tile([128, 16], F32, tag="xs2", name=f"xs2{g}_{c}")
                    nc.vector.reduce_sum(out=xs[:], in_=v, axis=AX.X)
                    nc.gpsimd.tensor_tensor(
                        out=PT[:, 18 + c, g * 16 : g * 16 + 16], in0=xs[:],
                        in1=lens_bc[:, g * 16 : g * 16 + 16], op=ALU.mult)
                    if g == 3:
                        proj_chunk(6 + c, 12 + 2 * c)
                        proj_chunk(18 + c, 13 + 2 * c)

        ph1_stack.close()

        # cv^T = relu(proj + b) straight out of the proj psum
        cvT = wat.tile([128, HC, NCL], BF16, tag="cvT")
        for m in range(HC):
            nc.scalar.activation(out=cvT[:, m, :], in_=pcs[:, m, :], func=AF.Relu,
                                 bias=projb[:, m : m + 1], scale=1.0)
        pjps_stack.close()
        pjwp_stack.close()
        wgcn_stack.close()
        if DEBUG:
            with tc.tile_pool(name="dbgp", bufs=1) as dp:
                ptf = dp.tile([128, 24, NCL], F32, tag="ptdbg")
                nc.vector.tensor_copy(out=ptf[:], in_=PT[:])
                nc.sync.dma_start(out=dbg["pool"][:, :, :], in_=ptf[:])

        # =================== phase 2: attention + FFN + KL ================
        p2 = ctx.enter_context(tc.tile_pool(name="p2", bufs=1))
        # bcast_pk: ao_b, v_b, out_b, ln1_g, ln1_b, ln2_g, ln2_b, bdec
        aob_bc = bcast_all[:, 0:H]
        vb_bc = bcast_all[:, H : 2 * H]
        outb_bc = bcast_all[:, 2 * H : 3 * H]
        l1g_bc = bcast_all[:, 3 * H : 4 * H]
        l1b_bc = bcast_all[:, 4 * H : 5 * H]
        decb_bc = bcast_all[:, 5 * H : 5 * H + NL]

        # FFN weights stream into space freed by H1T/yn16/pjw
        wf_stack = ExitStack()
        wf = wf_stack.enter_context(tc.tile_pool(name="wf", bufs=1))
        iw = wf.tile([128, HC, II], BF16, tag="iw")
        for half in range(2):
            nc.sync.dma_start(
                out=iw[:, 3 * half : 3 * half + 3, :].rearrange("p a b -> p (a b)"),
                in_=bass.AP(tensor=w["int_w"], offset=half * 3 * II,
                            ap=[[HC * II, 128], [1, 3 * II]]))
        ow = wf.tile([128, IC, H], BF16, tag="ow")
        for qtr in range(4):
            nc.sync.dma_start(
                out=ow[:, 6 * qtr : 6 * qtr + 6, :].rearrange("p a b -> p (a b)"),
                in_=bass.AP(tensor=w["out_w"], offset=qtr * 6 * H,
                            ap=[[IC * H, 128], [1, 6 * H]]))

        QT = p2.tile([DH, NH, NCL], BF16, tag="QT")
        KT = p2.tile([DH, NH, NCL], BF16, tag="KT")
        Vn = p2.tile([NCL, H], BF16, tag="Vn")
        att8 = p2.tile([NCL, NH, NCL], BF16, tag="att8")
        attS = p2.tile([NCL, NH, NCL], BF16, tag="attS")
        sums_t = p2.tile([NCL, NH], F32, tag="sums")
        recip_t = p2.tile([NCL, NH], F32, tag="recip")
        negmax = p2.tile([NCL, NH], F32, tag="negmax")
        cv_pa = p2.tile([NCL, H], F32, tag="cvpa")

        # scores psum prefilled with the attention mask (matmuls accumulate
        # on top); the copy is issued early so it never gates the scores.
        sc_stack = ExitStack()
        scps = sc_stack.enter_context(tc.tile_pool(name="scps", bufs=1, space="PSUM"))
        pss = scps.tile([NCL, NH, NCL], F32, tag="scores")
        nc.vector.tensor_copy(out=pss[:], in_=amask8[:])

        with tc.tile_pool(name="qkps", bufs=2, space="PSUM") as qkps, \
             tc.tile_pool(name="vps", bufs=1, space="PSUM") as vps, \
             tc.tile_pool(name="cvt2", bufs=3, space="PSUM") as cvt2:
            psq = qkps.tile([DH, NH * NCL], F32, tag="qk", name="psq")
            for h in range(NH):
                for c in range(HC):
                    nc.tensor.matmul(out=psq[:, h * NCL : h * NCL + NCL],
                                     lhsT=qw[:, c, h * DH : h * DH + DH],
                                     rhs=cvT[:, c, :], start=(c == 0), stop=(c == HC - 1))
            # Q evacs on DVE: (psq + q_b) / sqrt(dh)
            for h in range(NH):
                nc.vector.tensor_scalar(out=QT[:, h, :], in0=psq[:, h * NCL : h * NCL + NCL],
                                        scalar1=qkb_sb[:, h : h + 1], scalar2=1.0 / SQD,
                                        op0=ALU.add, op1=ALU.mult)
            psk = qkps.tile([DH, NH * NCL], F32, tag="qk", name="psk")
            for h in range(NH):
                for c in range(HC):
                    nc.tensor.matmul(out=psk[:, h * NCL : h * NCL + NCL],
                                     lhsT=kw[:, c, h * DH : h * DH + DH],
                                     rhs=cvT[:, c, :], start=(c == 0), stop=(c == HC - 1))
            for h in range(NH):
                nc.scalar.activation(out=KT[:, h, :], in_=psk[:, h * NCL : h * NCL + NCL],
                                     func=AF.Identity, bias=qkb_sb[:, 8 + h : 9 + h], scale=1.0)

            # scores straight after K; V runs on the PE behind them
            for h in range(NH):
                nc.tensor.matmul(out=pss[:, h, :], lhsT=QT[:, h, :], rhs=KT[:, h, :],
                                 start=False, stop=True, skip_group_check=True)
            pv1 = vps.tile([NCL, 512], F32, tag="v1")
            pv2 = vps.tile([NCL, 256], F32, tag="v2")
            for c in range(HC):
                nc.tensor.matmul(out=pv1[:], lhsT=cvT[:, c, :], rhs=vw[:, c, 0:512],
                                 start=(c == 0), stop=(c == HC - 1))
                nc.tensor.matmul(out=pv2[:], lhsT=cvT[:, c, :], rhs=vw[:, c, 512:768],
                                 start=(c == 0), stop=(c == HC - 1))
            # cv natural + ao_b residual base (PE transposes behind V)
            for c in range(HC):
                ps = cvt2.tile([NCL, 128], BF16, tag="cvn", name=f"cvn{c}")
                nc.tensor.transpose(out=ps[:], in_=cvT[:, c, :], identity=ident_b[:])
                nc.vector.tensor_tensor(out=cv_pa[:, c * 128 : c * 128 + 128], in0=ps[:],
                                        in1=aob_bc[:, c * 128 : c * 128 + 128], op=ALU.add)
            nc.vector.tensor_tensor(out=Vn[:, 0:512], in0=pv1[:], in1=vb_bc[:, 0:512], op=ALU.add)
            nc.vector.tensor_tensor(out=Vn[:, 512:768], in0=pv2[:], in1=vb_bc[:, 512:768], op=ALU.add)

        nc.vector.tensor_reduce(out=negmax[:], in_=pss[:], axis=AX.X,
                                op=ALU.max, negate=True)
        for h in range(NH):
            nc.scalar.activation(
                out=att8[:, h, :], in_=pss[:, h, :], func=AF.Exp,
                bias=negmax[:, h : h + 1], scale=1.0,
            )
        nc.vector.reduce_sum(out=sums_t[:], in_=att8[:], axis=AX.X)
        nc.vector.reciprocal(out=recip_t[:], in_=sums_t[:])
        sc_stack.close()

        # per-head: scale, transpose, ctx^T = V-block @ att^T, ao accum
        ctxT = p2.tile([DH, NH, NCL], BF16, tag="ctxT")
        attn_out = p2.tile([NCL, H], F32, tag="attnout")
        xhat = p2.tile([NCL, H], F32, tag="xhat")
        ln_in = p2.tile([NCL, H], F32, tag="lnin1")
        with tc.tile_pool(name="aops", bufs=1, space="PSUM") as aops:
            pa1 = aops.tile([NCL, 512], F32, tag="ao1")
            pa2 = aops.tile([NCL, 256], F32, tag="ao2")
            with tc.tile_pool(name="ctps", bufs=3, space="PSUM") as ctps, \
                 tc.tile_pool(name="atts", bufs=3) as atts:
                for h in range(NH):
                    nc.scalar.mul(out=attS[:, h, :], in_=att8[:, h, :], mul=recip_t[:, h : h + 1])
                    pst = ctps.tile([NCL, NCL], BF16, tag="attT", name=f"attT{h}")
                    nc.tensor.transpose(out=pst[:], in_=attS[:, h, :], identity=ident_b[:64, :64])
                    asb = atts.tile([NCL, NCL], BF16, tag="attTs", name=f"attTs{h}")
                    nc.vector.tensor_copy(out=asb[:], in_=pst[:])
                    pctx = ctps.tile([DH, NCL], F32, tag="ctx", name=f"ctx{h}")
                    nc.tensor.matmul(out=pctx[:], lhsT=Vn[:, h * DH : h * DH + DH], rhs=asb[:],
                                     start=True, stop=True)
                    if h % 2 == 0:
                        nc.vector.tensor_copy(out=ctxT[:, h, :], in_=pctx[:])
                    else:
                        nc.scalar.copy(out=ctxT[:, h, :], in_=pctx[:])
                for h in range(NH):
                    nc.tensor.matmul(out=pa1[:], lhsT=ctxT[:, h, :], rhs=aow[:, h, 0:512],
                                     start=(h == 0), stop=(h == NH - 1))
                    nc.tensor.matmul(out=pa2[:], lhsT=ctxT[:, h, :], rhs=aow[:, h, 512:768],
                                     start=(h == 0), stop=(h == NH - 1))

            def layer_norm(x_nat, g_bc, b_bc, y_nat, lnp):
                stats = lnp.tile([NCL, 3, 6], F32, tag="lnstats")
                for i in range(3):
                    nc.vector.bn_stats(out=stats[:, i, :], in_=x_nat[:, i * 256 : i * 256 + 256])
                mv = lnp.tile([NCL, 2], F32, tag="lnmv")
                nc.vector.bn_aggr(out=mv[:], in_=stats[:])
                sd = lnp.tile([NCL, 1], F32, tag="lnsd")
                nc.scalar.activation(out=sd[:], in_=mv[:, 1:2], func=AF.Sqrt, bias=eps_t[:, :1], scale=1.0)
                rstd = lnp.tile([NCL, 1], F32, tag="lnrstd")
                nc.vector.reciprocal(out=rstd[:], in_=sd[:])
                xc = lnp.tile([NCL, H], F32, tag="lnxc")
                nc.vector.tensor_scalar(out=xc[:], in0=x_nat[:], scalar1=mv[:, 0:1],
                                        scalar2=rstd[:, :1], op0=ALU.subtract, op1=ALU.mult)
                nc.vector.tensor_tensor(out=xc[:], in0=xc[:], in1=g_bc, op=ALU.mult)
                nc.vector.tensor_tensor(out=y_nat[:], in0=xc[:], in1=b_bc, op=ALU.add)

            with tc.tile_pool(name="ln1p", bufs=1) as lnp:
                nc.vector.tensor_tensor(out=ln_in[:, 0:512], in0=pa1[:], in1=cv_pa[:, 0:512], op=ALU.add)
                nc.vector.tensor_tensor(out=ln_in[:, 512:768], in0=pa2[:], in1=cv_pa[:, 512:768], op=ALU.add)
                # LN1 split: FFN consumes xhat directly (ln1_g/ln1_b are
                # host-folded into int_w/int_b); the residual (xhat*g+b)
                # is computed off the critical path.
                stats = lnp.tile([NCL, 3, 6], F32, tag="lnstats")
                for i in range(3):
                    nc.vector.bn_stats(out=stats[:, i, :], in_=ln_in[:, i * 256 : i * 256 + 256])
                mv = lnp.tile([NCL, 2], F32, tag="lnmv")
                nc.vector.bn_aggr(out=mv[:], in_=stats[:])
                sd = lnp.tile([NCL, 1], F32, tag="lnsd")
                nc.scalar.activation(out=sd[:], in_=mv[:, 1:2], func=AF.Sqrt, bias=eps_t[:, :1], scale=1.0)
                rstd = lnp.tile([NCL, 1], F32, tag="lnrstd")
                nc.vector.reciprocal(out=rstd[:], in_=sd[:])
                nc.vector.tensor_scalar(out=xhat[:], in0=ln_in[:], scalar1=mv[:, 0:1],
                                        scalar2=rstd[:, :1], op0=ALU.subtract, op1=ALU.mult)
        if DEBUG:
            nc.sync.dma_start(out=dbg["attn"][:, :], in_=attn_out[:])

        # aoT = xhat^T (bf16); the residual attn_out = xhat*g1 + b1 runs on
        # DVE in parallel with the FFN matmuls
        aoT = p2.tile([128, HC, NCL], BF16, tag="aoT")
        with tc.tile_pool(name="aotps", bufs=1, space="PSUM") as aotps:
            psA = aotps.tile([128, HC * NCL], F32, tag="psA")
            for c in range(HC):
                nc.tensor.transpose(out=psA[:, c * NCL : c * NCL + NCL],
                                    in_=xhat[:, c * 128 : c * 128 + 128],
                                    identity=ident[:64, :64])
            nc.vector.tensor_copy(out=aoT[:].rearrange("p a b -> p (a b)"), in_=psA[:])
        nc.vector.tensor_tensor(out=attn_out[:], in0=xhat[:], in1=l1g_bc, op=ALU.mult)
        nc.vector.tensor_tensor(out=attn_out[:], in0=attn_out[:], in1=l1b_bc, op=ALU.add)

        # FFN: inter produced transposed, chunk by chunk, feeding out-proj
        ln_in2 = p2.tile([NCL, H], F32, tag="lnin2")
        with tc.tile_pool(name="fips", bufs=6, space="PSUM") as fips, \
             tc.tile_pool(name="fit", bufs=6) as fit, \
             tc.tile_pool(name="fops", bufs=1, space="PSUM") as fops, \
             tc.tile_pool(name="ln2p", bufs=1) as lnp2:
            po1 = fops.tile([NCL, 512], F32, tag="o1")
            po2 = fops.tile([NCL, 256], F32, tag="o2")
            for cc in range(IC):
                ip = fips.tile([128, NCL], F32, tag="fi", name=f"fi{cc}")
                for c in range(HC):
                    nc.tensor.matmul(out=ip[:], lhsT=iw[:, c, cc * 128 : cc * 128 + 128],
                                     rhs=aoT[:, c, :], start=(c == 0), stop=(c == HC - 1))
                it = fit.tile([128, NCL], BF16, tag="it", name=f"it{cc}")
                nc.scalar.activation(out=it[:], in_=ip[:], func=AF.Gelu,
                                     bias=bias_sb[:, 18 + cc : 19 + cc], scale=1.0)
                nc.tensor.matmul(out=po1[:], lhsT=it[:], rhs=ow[:, cc, 0:512],
                                 start=(cc == 0), stop=(cc == IC - 1))
                nc.tensor.matmul(out=po2[:], lhsT=it[:], rhs=ow[:, cc, 512:768],
                                 start=(cc == 0), stop=(cc == IC - 1))
            nc.vector.tensor_tensor(out=ln_in2[:, 0:512], in0=po1[:], in1=attn_out[:, 0:512], op=ALU.add)
            nc.vector.tensor_tensor(out=ln_in2[:, 512:768], in0=po2[:], in1=attn_out[:, 512:768], op=ALU.add)
            nc.vector.tensor_tensor(out=ln_in2[:], in0=ln_in2[:], in1=outb_bc[:], op=ALU.add)
            # LN2 folded into the decoder: pred = rstd*(ln_in2 @ gdec2) + bdec
            # (gdec2 = ln2_g*dec_w - colsum/H host-folded). Only the rstd
            # stats chain remains; the transpose runs in parallel with it.
            stats2 = lnp2.tile([NCL, 3, 6], F32, tag="lnstats2")
            for i in range(3):
                nc.vector.bn_stats(out=stats2[:, i, :], in_=ln_in2[:, i * 256 : i * 256 + 256])
            mv2 = lnp2.tile([NCL, 2], F32, tag="lnmv2")
            nc.vector.bn_aggr(out=mv2[:], in_=stats2[:])
            sd2 = lnp2.tile([NCL, 1], F32, tag="lnsd2")
            nc.scalar.activation(out=sd2[:], in_=mv2[:, 1:2], func=AF.Sqrt, bias=eps_t[:, :1], scale=1.0)
            rstd2 = p2.tile([NCL, 1], F32, tag="lnrstd2")
            nc.vector.reciprocal(out=rstd2[:], in_=sd2[:])
        wf_stack.close()

        # decoder + KL
        outT = p2.tile([128, HC, NCL], F32, tag="outT")
        with tc.tile_pool(name="otps", bufs=1, space="PSUM") as otps:
            psO = otps.tile([128, HC * NCL], F32, tag="psO")
            for c in range(HC):
                nc.tensor.transpose(out=psO[:, c * NCL : c * NCL + NCL],
                                    in_=ln_in2[:, c * 128 : c * 128 + 128],
                                    identity=ident[:64, :64])
            nc.vector.tensor_copy(out=outT[:].rearrange("p a b -> p (a b)"), in_=psO[:])

        pair = p2.tile([NCL, 2], F32, tag="pair")
        fin_sb = p2.tile([2, 1], F32, tag="fin")
        with tc.tile_pool(name="klps", bufs=1, space="PSUM") as klps, \
             tc.tile_pool(name="klsc", bufs=1) as klsc:
            pd = klps.tile([NCL, NL], F32, tag="pred")
            for c in range(HC):
                nc.tensor.matmul(out=pd[:], lhsT=outT[:, c, :], rhs=dw[:, c, :],
                                 start=(c == 0), stop=(c == HC - 1))
            predm = klsc.tile([NCL, NL], F32, tag="predm")
            nc.scalar.mul(out=predm[:], in_=pd[:], mul=rstd2[:, :1])
            pred = klsc.tile([NCL, NL], F32, tag="pred_sb")
            nc.vector.tensor_tensor(out=pred[:], in0=predm[:], in1=decb_bc[:], op=ALU.add)
            if DEBUG:
                nc.sync.dma_start(out=dbg["pred"][:, :], in_=pred[:])
            # KL with sum_l(t)=1: kl = sum_l t*(ln t - pred) - negm + ln(ssum)
            negm = klsc.tile([NCL, 1], F32, tag="negm")
            nc.vector.tensor_reduce(out=negm[:], in_=pred[:], axis=AX.X, op=ALU.max, negate=True)
            esc = klsc.tile([NCL, NL], F32, tag="esc")
            ssum = klsc.tile([NCL, 1], F32, tag="ssum")
            nc.scalar.activation(out=esc[:], in_=pred[:], func=AF.Exp,
                                 bias=negm[:, :1], scale=1.0, accum_out=ssum[:, :1])
            lnS = klsc.tile([NCL, 1], F32, tag="lnS")
            nc.scalar.activation(out=lnS[:], in_=ssum[:], func=AF.Ln)
            lnt = klsc.tile([NCL, NL], F32, tag="lnt")
            nc.scalar.activation(out=lnt[:], in_=tgt_sb[:], func=AF.Ln)
            a1 = klsc.tile([NCL, NL], F32, tag="a1")
            nc.vector.tensor_tensor(out=a1[:], in0=lnt[:], in1=pred[:], op=ALU.subtract)
            nc.vector.tensor_tensor(out=a1[:], in0=a1[:], in1=tgt_sb[:], op=ALU.mult)
            kl = klsc.tile([NCL, 1], F32, tag="kl")
            nc.vector.reduce_sum(out=kl[:], in_=a1[:], axis=AX.X)
            nc.vector.tensor_tensor(out=kl[:], in0=kl[:], in1=negm[:], op=ALU.subtract)
            nc.vector.tensor_tensor(out=kl[:], in0=kl[:], in1=lnS[:], op=ALU.add)
            nc.vector.tensor_tensor(out=pair[:, 0:1], in0=kl[:], in1=cnm_pp[:], op=ALU.mult)
            nc.vector.tensor_copy(out=pair[:, 1:2], in_=cnm_pp[:])
            pf = klps.tile([2, 1], F32, tag="fin_ps")
            nc.tensor.matmul(out=pf[:], lhsT=pair[:], rhs=ones_t[:], start=True, stop=True)
            nc.vector.tensor_copy(out=fin_sb[:], in_=pf[:])
            nc.sync.dma_start(out=out_d[:, None], in_=fin_sb[:])


_CACHE = {}


def _get_program():
    if "nc" not in _CACHE:
        nc, nfix = build_program()
        _CACHE["nc"] = nc
    return _CACHE["nc"]


def shard_inputs(inputs):
    import ml_dtypes
    bf16 = ml_dtypes.bfloat16
    enc = np.ascontiguousarray(inputs["encoder_hs"], dtype=np.float32)
    wr = np.asarray(inputs["word_recovery"], dtype=np.int32)
    wm = np.asarray(inputs["word_recovery_mask"], dtype=np.int32)
    cn = np.asarray(inputs["clause_num_mask"], dtype=np.int32)
    adj = np.ascontiguousarray(inputs["adj_matrix"], dtype=np.float32)
    tl = np.ascontiguousarray(inputs["target_labels"], dtype=np.float32)

    f32w = lambda k: np.asarray(inputs[k], dtype=np.float32)
    bf16w = lambda k: np.ascontiguousarray(f32w(k).astype(bf16))

    # shared (identical across cores) weight arrays. All matmul weights are
    # host-packed into their SBUF layout [128, chunks*cols] so each DMA is
    # 128 contiguous descriptors. LayerNorm affine params are folded into
    # the consumers: ln1_g/ln1_b into int_w/int_b, ln2_g/ln2_b + dec_b into
    # the decoder (gdec2 / bdec).
    def pack128(a):
        # [C*128, X] -> [128, C*X]
        C = a.shape[0] // 128
        return np.ascontiguousarray(a.reshape(C, 128, a.shape[1]).transpose(1, 0, 2).reshape(128, -1))

    shared = {}
    for k in ("gc1_w", "gc2_w", "proj_w", "q_w", "k_w", "v_w", "out_w"):
        shared[k] = pack128(bf16w(k))
    ao96 = f32w("ao_w").reshape(NH, DH, H).transpose(1, 0, 2).reshape(DH, NH * H)
    shared["ao_w"] = np.ascontiguousarray(ao96.astype(bf16))
    int_w2 = f32w("ln1_g")[:, None] * f32w("int_w")
    shared["int_w"] = pack128(np.ascontiguousarray(int_w2.astype(bf16)))
    intb2 = f32w("ln1_b") @ f32w("int_w") + f32w("int_b")
    gdec = f32w("ln2_g")[:, None] * f32w("dec_w")
    gdec2 = gdec - gdec.sum(0, keepdims=True) / H
    shared["dec_w"] = pack128(np.ascontiguousarray(gdec2.astype(np.float32)))
    bdec = f32w("ln2_b") @ f32w("dec_w") + f32w("dec_b")
    bias_pk = np.concatenate([f32w("gc1_b"), f32w("gc2_b"), f32w("proj_b"),
                              intb2]).reshape(42, 128).T
    shared["bias_pk"] = np.ascontiguousarray(bias_pk.astype(np.float32))
    qkb = np.concatenate([f32w("q_b"), f32w("k_b")]).reshape(16, DH).T
    shared["qkb_pk"] = np.ascontiguousarray(qkb)
    bcast = np.concatenate([f32w("ao_b"), f32w("v_b"), f32w("out_b"),
                            f32w("ln1_g"), f32w("ln1_b"), bdec])
    shared["bcast_pk"] = np.ascontiguousarray(bcast.astype(bf16))

    in_maps = []
    boff = (np.arange(BB) * S).astype(np.int32)[:, None, None]
    for i in range(NCORES):
        sl = slice(BB * i, BB * i + BB)
        cnm_i = cn[sl].astype(np.float32).reshape(NCL)
        am = np.zeros((NCL, NCL), dtype=np.float32)
        for b in range(BB):
            blk = (1.0 - cnm_i[b * M : (b + 1) * M]) * -10000.0
            am[b * M : (b + 1) * M, b * M : (b + 1) * M] = blk[None, :]
        wrm_i = wm[sl].astype(np.float32).reshape(NROW)
        lens_i = wrm_i.reshape(NCL, LC).sum(-1)
        lens_r = 1.0 / np.maximum(lens_i, 1.0)
        # block-diag wrm*lens_r for PE avg-pool: wseg[p, r, j] nonzero only
        # when row 128r+p belongs to clause 4r+j (j = p//32)
        wseg = np.zeros((128, RT, 4), dtype=np.float32)
        pidx = np.arange(128)
        for r in range(RT):
            rows = 128 * r + pidx
            j = pidx // 32
            wseg[pidx, r, j] = wrm_i[rows] * lens_r[rows // LC]
        adj_i = adj[sl].reshape(NCL, LC, LC)
        adjt = np.zeros((128, RT, 128), dtype=np.float32)
        for b in range(NCL):
            c, t = b % 4, b // 4
            adjt[32 * c : 32 * c + 32, t, 32 * c : 32 * c + 32] = adj_i[b].T
        enc_i = enc[sl].reshape(BB * S, H)
        d = dict(
            enc=np.ascontiguousarray(enc_i),
            gidx=np.ascontiguousarray((wr[sl] + boff).reshape(NROW).reshape(RT, 128).T),
            **{f"enc_s{i}": np.ascontiguousarray(enc_i[:, 192 * i : 192 * i + 192])
               for i in range(4)},
            wrm=np.ascontiguousarray(wrm_i.reshape(RT, 128).T),
            wrm_b=np.ascontiguousarray(wrm_i.astype(bf16)),
            lens=np.ascontiguousarray(lens_r),
            wseg=np.ascontiguousarray(wseg.astype(bf16).reshape(128, RT * 4)),
            cnm=np.ascontiguousarray(cnm_i),
            amask=np.ascontiguousarray(np.tile(am, (1, NH)).astype(bf16)),
            adjt=np.ascontiguousarray(adjt.astype(bf16)),
            tgt=np.ascontiguousarray(tl[sl].reshape(NCL, NL)),
        )
        d.update(shared)
        in_maps.append(d)
    return in_maps


def run_spmd(inputs, trace=False):
    nc = _get_program()
    in_maps = shard_inputs(inputs)
    kw = {}
    if trace:
        import types
        from trn_agent_boot.trn_boot import _ntff_profile_via_ctypes
        mod = types.ModuleType("antenv.axon_hooks")
        hook = _ntff_profile_via_ctypes("/opt/axon/libaxon_pjrt.so")
        mod.get_axon_ntff_profile_hook = lambda: hook
        mod.set_axon_ntff_profile_hook = lambda h: None
        sys.modules["antenv.axon_hooks"] = mod
        bass_utils.upload_artifacts = lambda tmpdir: "local://" + tmpdir
        kw["trace"] = True
    res = bass_utils.run_bass_kernel_spmd(nc, in_maps, core_ids=list(range(NCORES)), **kw)
    return res


def kernel(**inputs):
    res = run_spmd(inputs)
    num = 0.0
    den = 0.0
    for i in range(NCORES):
        o = res.results[i]["out"]
        num += float(o[0])
        den += float(o[1])
    loss = (num / NL) / den
    return np.asarray(loss, dtype=np.float32)
